# revision 43
# baseline (speedup 1.0000x reference)
"""Two-layer GAT (PyG GATConv-style) on 8 Trainium2 NeuronCores via Bass/Tile.

Strategy (dst-sharded, data-parallel, fp16 data path):
  - Nodes are assigned to (core, group) "strata": all 8 cores' group g hold
    nodes with similar in-degree statistics (snake-ordered by (cA, cB)), so a
    single SPMD program with uniform per-group slot counts serves all cores.
  - Each core redundantly computes the dense part h = x @ W (plus fused
    alpha_src/alpha_dst columns) for ALL nodes and writes a DRAM row table
    (fp16 row, 768B pitch = [h(256) | a_src(4) | a_dst(4) | pad]); the edge
    phase (gather h[src], per-dst softmax over incoming edges, weighted sum)
    runs only on the core's own 1/8 of destination nodes via dma_gather with
    host-precomputed int16 index lists (slot-major, per-dst-row layout).
  - int16 gather indices only reach 32767, so the table is split into an A
    range and a B range (split aligned to a stratum boundary); every dst row's
    slots are [A-slots | B-slots] with per-group uniform counts KA/KB. Slot 0
    of a node's own side is its self-loop, which supplies alpha_dst.
  - Edge math: e = leaky(as+ad) (no max-shift; dummy slots use as=-60000 so
    exp underflows to 0), p = exp(e), den = sum_k p, alpha = p/den fused into
    an Act-engine per-head expansion to [k, c] fp16, weighted products via a
    DVE 2x-mode tensor_tensor, then a binary-tree fp16 reduction over slots.
  - Between layers, each core's elu(out1)^T block is AllGathered (fp16) so
    layer 2's replicated dense phase can read every node's features.
  - Final sigmoid(fc) output is computed per-core and unpermuted on the host.
"""

import sys

sys.path.insert(0, "/opt/trn_rl_repo")

from dataclasses import dataclass, field

import numpy as np

import concourse.bass as bass
import concourse.bacc as bacc
import concourse.tile as tile
from concourse import mybir
from concourse.library_config import mlp as MLP_LIB

F32 = mybir.dt.float32
F16 = mybir.dt.float16
I16 = mybir.dt.int16
AX = mybir.AxisListType
OP = mybir.AluOpType
ACT = mybir.ActivationFunctionType

HEADS = 4
CH = 64
HC = HEADS * CH          # 256
DIN = 128
NEG_SLOPE = 0.2
EWP = 384                # table row pitch in fp16 elems (768 B, %256==0)
PAY = 272                # payload elems per row: h(256) | as(4) | ad(4) | pad8
AS_OFF = 256             # alpha_src column offset within a row
AD_OFF = 260             # alpha_dst column offset
DUM_AS = -60000.0        # dummy-slot alpha_src (finite; exp underflows to 0)


@dataclass
class Cfg:
    n_real: int = 50000
    nc: int = 8                  # cores
    g: int = 49                  # groups (strata) per core
    split_strat: int = 31        # strata in table range A
    ka: list = field(default_factory=list)   # per-group A slots
    kb: list = field(default_factory=list)   # per-group B slots

    @property
    def stratum(self):
        return self.nc * 128

    @property
    def np_(self):
        return self.g * self.stratum

    @property
    def pc(self):
        return self.g * 128

    @property
    def split_sp(self):
        return self.split_strat * self.stratum

    @property
    def nb(self):
        return self.np_ - self.split_sp

    @property
    def rows(self):
        return self.np_ + 3      # dumA + dumB + padself

    @property
    def row_b0(self):
        return self.split_sp + 1   # dummyB row

    @property
    def padself_idx(self):
        return self.nb + 1         # local B index of the pad-self row


def build_layout(edge_index, cfg: Cfg):
    """Host-side graph layout. Returns per-core index arrays + permutation."""
    n = cfg.n_real
    src = np.asarray(edge_index[0], dtype=np.int64)
    dst = np.asarray(edge_index[1], dtype=np.int64)
    src = np.concatenate([src, np.arange(n, dtype=np.int64)])
    dst = np.concatenate([dst, np.arange(n, dtype=np.int64)])
    deg = np.bincount(dst, minlength=n)

    split = cfg.split_sp
    order1 = np.argsort(-deg, kind="stable")
    a_nodes = order1[:split]
    b_nodes = order1[split:]
    is_a = np.zeros(n, bool)
    is_a[a_nodes] = True
    ca = np.bincount(dst[is_a[src]], minlength=n)
    cb = deg - ca

    def snake(nodes):
        out = []
        flip = False
        avals = ca[nodes]
        for v in np.sort(np.unique(avals))[::-1]:
            grp = nodes[avals == v]
            grp = grp[np.argsort(cb[grp], kind="stable")]
            if flip:
                grp = grp[::-1]
            flip = not flip
            out.append(grp)
        return np.concatenate(out) if out else nodes

    seq = np.concatenate([snake(a_nodes), snake(b_nodes)])
    # node_of_sp: storage position -> node (-1 = pad). pads go at the end.
    node_of_sp = np.full(cfg.np_, -1, np.int64)
    node_of_sp[: len(seq)] = seq
    sp_of = np.full(n, -1, np.int64)
    sp_of[seq] = np.arange(len(seq))

    # per-group slot counts
    ka = np.zeros(cfg.g, np.int64)
    kb = np.zeros(cfg.g, np.int64)
    for s in range(cfg.g):
        nodes = node_of_sp[s * cfg.stratum : (s + 1) * cfg.stratum]
        nodes = nodes[nodes >= 0]
        if len(nodes):
            ka[s] = ca[nodes].max()
            kb[s] = cb[nodes].max()
    ka = np.maximum(ka, 1)
    kb = np.maximum(kb, 1)
    # pads live in the last stratum; their pad-self slot is B slot 0
    cfg.ka = ka.tolist()
    cfg.kb = kb.tolist()

    # group edges by dst
    eorder = np.argsort(dst, kind="stable")
    s_sorted = src[eorder]
    starts = np.searchsorted(dst, np.arange(n + 1), sorter=eorder)

    sp_src = sp_of[s_sorted]           # storage pos of each edge's src
    src_is_a = sp_src < split
    idx_a_val = (sp_src + 1).astype(np.int64)            # A-range row index
    idx_b_val = (sp_src - split + 1).astype(np.int64)    # B-range local row

    def pack(flat):
        # idx element i lives at [i % 16, i // 16]; the two Q7 DGE cores read
        # partitions 0-15 and 16-31 respectively, so replicate into both.
        ln = len(flat)
        f = np.zeros((128, ln // 16), np.int16)
        ii = np.arange(ln)
        f[ii % 16, ii // 16] = flat
        f[16 + (ii % 16), ii // 16] = flat
        return f

    idx_cores = []
    for c in range(cfg.nc):
        flats = []
        for s in range(cfg.g):
            KA, KB = int(ka[s]), int(kb[s])
            ma = np.zeros((KA, 128), np.int64)   # [slot, partition]
            mb = np.zeros((KB, 128), np.int64)
            base_sp = s * cfg.stratum + c * 128
            for p in range(128):
                node = node_of_sp[base_sp + p]
                if node < 0:
                    # pad position: pad-self row keeps den positive
                    mb[0, p] = cfg.padself_idx
                    continue
                lo, hi = starts[node], starts[node + 1]
                aa = idx_a_val[lo:hi][src_is_a[lo:hi]]
                bb = idx_b_val[lo:hi][~src_is_a[lo:hi]]
                # self-loop first in its own side (supplies alpha_dst)
                own_sp = base_sp + p
                if own_sp < split:
                    self_idx = own_sp + 1
                    w = np.where(aa == self_idx)[0]
                    if len(w) == 0:
                        raise RuntimeError("self-loop missing in A list")
                    aa[[0, w[0]]] = aa[[w[0], 0]]
                else:
                    self_idx = own_sp - split + 1
                    w = np.where(bb == self_idx)[0]
                    if len(w) == 0:
                        raise RuntimeError("self-loop missing in B list")
                    bb[[0, w[0]]] = bb[[w[0], 0]]
                ma[: len(aa), p] = aa
                mb[: len(bb), p] = bb
            flats.append(
                np.concatenate([pack(ma.reshape(-1)), pack(mb.reshape(-1))],
                               axis=1)
            )
        idx_cores.append(np.concatenate(flats, axis=1))

    return dict(node_of_sp=node_of_sp, sp_of=sp_of, idx=idx_cores)


def build_program(cfg: Cfg, shared_out: bool = True):
    nc_b = bacc.Bacc(None, num_devices=cfg.nc)
    NP, G, NC = cfg.np_, cfg.g, cfg.nc
    NT = NP // 128                 # global tiles (stratum-major)
    SPLIT_T = cfg.split_sp // 128  # first B tile
    R = cfg.rows
    ROW_B0 = cfg.row_b0
    PC = cfg.pc
    sumK8 = 8 * int(np.sum(cfg.ka) + np.sum(cfg.kb))

    xT = nc_b.dram_tensor("xT", [DIN, NP], F16, kind="ExternalInput")
    W1AUG = nc_b.dram_tensor("W1AUG", [DIN, PAY], F16, kind="ExternalInput")
    W2AUG = nc_b.dram_tensor("W2AUG", [HC, PAY], F16, kind="ExternalInput")
    b1t = nc_b.dram_tensor("b1t", [128, HC], F16, kind="ExternalInput")
    b2t = nc_b.dram_tensor("b2t", [128, HC], F16, kind="ExternalInput")
    fcwt = nc_b.dram_tensor("fcwt", [128, HC], F32, kind="ExternalInput")
    fcbt = nc_b.dram_tensor("fcbt", [128, 1], F32, kind="ExternalInput")
    identh = nc_b.dram_tensor("identh", [128, 128], F16, kind="ExternalInput")
    dumrow = nc_b.dram_tensor("dumrow", [3, EWP], F16, kind="ExternalInput")
    idx = nc_b.dram_tensor("idx", [128, sumK8], I16, kind="ExternalInput")
    y = nc_b.dram_tensor("y", [PC, 1], F32, kind="ExternalOutput")

    table1 = nc_b.dram_tensor("table1", [R, EWP], F16)
    table2 = nc_b.dram_tensor("table2", [R, EWP], F16)
    elu1T = nc_b.dram_tensor("elu1T", [HC, NP // NC], F16)
    gath = nc_b.dram_tensor(
        "gath", [NC * HC, NP // NC], F16,
        addr_space="Shared" if shared_out else "Local",
    )

    with tile.TileContext(nc_b) as tc:
        ctxmgrs = [
            tc.tile_pool(name="cst", bufs=1),
            tc.tile_pool(name="lw", bufs=2),
            tc.tile_pool(name="hp", bufs=2),
            tc.tile_pool(name="ge", bufs=3),
            tc.tile_pool(name="px", bufs=2),
            tc.tile_pool(name="th", bufs=1),
            tc.tile_pool(name="sm", bufs=2),
            tc.tile_pool(name="ix", bufs=2),
            tc.tile_pool(name="ph", bufs=1, space="PSUM"),
            tc.tile_pool(name="pt", bufs=2, space="PSUM"),
        ]
        import contextlib

        with contextlib.ExitStack() as st:
            cst, lw, hp, ge, px, th, sm, ix, ph, pt = [
                st.enter_context(m) for m in ctxmgrs
            ]
            nc_b.gpsimd.load_library(MLP_LIB)

            # ---- constants ----
            idt = cst.tile([128, 128], F16)
            nc_b.sync.dma_start(out=idt[:], in_=identh[:, :])
            b1s = cst.tile([128, HC], F16)
            nc_b.sync.dma_start(out=b1s[:], in_=b1t[:, :])
            b2s = cst.tile([128, HC], F16)
            nc_b.sync.dma_start(out=b2s[:], in_=b2t[:, :])
            fcws = cst.tile([128, HC], F32)
            nc_b.sync.dma_start(out=fcws[:], in_=fcwt[:, :])
            fcbs = cst.tile([128, 1], F32)
            nc_b.sync.dma_start(out=fcbs[:], in_=fcbt[:, :])
            w1s = cst.tile([128, PAY], F16)
            nc_b.sync.dma_start(out=w1s[:], in_=W1AUG[:, :])
            w2s = cst.tile([128, 2, PAY], F16)
            nc_b.sync.dma_start(out=w2s[:, 0, :], in_=W2AUG[0:128, :])
            nc_b.sync.dma_start(out=w2s[:, 1, :], in_=W2AUG[128:HC, :])
            zall = cst.tile([128, G], F32)

            # dummy + pad-self rows for both tables (DRAM -> DRAM)
            for tab in (table1, table2):
                nc_b.sync.dma_start(out=tab[0:1, :], in_=dumrow[0:1, :])
                nc_b.sync.dma_start(
                    out=tab[ROW_B0 : ROW_B0 + 1, :], in_=dumrow[1:2, :]
                )
                nc_b.sync.dma_start(out=tab[R - 1 : R, :], in_=dumrow[2:3, :])

            # ---- dense phase (replicated): h|alphas -> table ----
            def h_phase(layer):
                tab = table1 if layer == 1 else table2
                cp_rr = [0]

                def copy_psum(dst_ap, src_ap):
                    e = cp_rr[0] % 2
                    cp_rr[0] += 1
                    if e == 0:
                        nc_b.scalar.copy(out=dst_ap, in_=src_ap)
                    else:
                        nc_b.vector.tensor_copy(out=dst_ap, in_=src_ap)

                def write_rows(ht_slice, row0, nt):
                    # ht_slice[:, j, :] (j in 0..nt) -> rows row0 + j*128 + p
                    nc_b.sync.dma_start(
                        out=bass.AP(
                            tensor=tab[:, :].tensor,
                            offset=row0 * EWP,
                            ap=[[EWP, 128], [128 * EWP, nt], [1, PAY]],
                        ),
                        in_=ht_slice,
                    )

                if layer == 1:
                    TB = 12
                    t0 = 0
                    while t0 < NT:
                        nt = min(TB, NT - t0)
                        lh = lw.tile([128, TB * 128], F16, tag="gh")
                        nc_b.sync.dma_start(
                            out=lh[:, 0 : nt * 128],
                            in_=xT[:, t0 * 128 : (t0 + nt) * 128],
                        )
                        ht = hp.tile([128, TB, PAY], F16, tag="ht")
                        for j in range(nt):
                            pj = ph.tile([128, PAY], F32, space="PSUM",
                                         tag=f"pj{j % 5}")
                            nc_b.tensor.matmul(
                                out=pj[:], lhsT=lh[:, j * 128 : (j + 1) * 128],
                                rhs=w1s[:], start=True, stop=True,
                            )
                            copy_psum(ht[:, j, :], pj[:])
                        # contiguous row runs (split at the A/B boundary)
                        runs = []
                        for j in range(nt):
                            t = t0 + j
                            sh = 1 if t < SPLIT_T else 2
                            if runs and runs[-1][2] == sh:
                                runs[-1][1] += 1
                            else:
                                runs.append([j, 1, sh])
                        for j0, nj, sh in runs:
                            write_rows(ht[:, j0 : j0 + nj, :],
                                       (t0 + j0) * 128 + sh, nj)
                        t0 += nt
                else:
                    TB = 11
                    for c in range(NC):
                        s0 = 0
                        while s0 < G:
                            ns = min(TB, G - s0)
                            gh = lw.tile([128, 2, TB * 128], F16, tag="gh")
                            nc_b.sync.dma_start(
                                out=gh[:, :, 0 : ns * 128],
                                in_=bass.AP(
                                    tensor=gath[:, :].tensor,
                                    offset=(c * HC) * (NP // NC) + s0 * 128,
                                    ap=[[NP // NC, 128],
                                        [128 * (NP // NC), 2],
                                        [1, ns * 128]],
                                ),
                            )
                            ht = hp.tile([128, TB, PAY], F16, tag="ht")
                            for j in range(ns):
                                pj = ph.tile([128, PAY], F32, space="PSUM",
                                             tag=f"pj{j % 5}")
                                nc_b.tensor.matmul(
                                    out=pj[:],
                                    lhsT=gh[:, 0, j * 128 : (j + 1) * 128],
                                    rhs=w2s[:, 0, :], start=True, stop=False,
                                )
                                nc_b.tensor.matmul(
                                    out=pj[:],
                                    lhsT=gh[:, 1, j * 128 : (j + 1) * 128],
                                    rhs=w2s[:, 1, :], start=False, stop=True,
                                )
                                copy_psum(ht[:, j, :], pj[:])
                            runs = []
                            for j in range(ns):
                                sv = s0 + j
                                sh = 1 if sv < cfg.split_strat else 2
                                if runs and runs[-1][2] == sh:
                                    runs[-1][1] += 1
                                else:
                                    runs.append([j, 1, sh])
                            for j0, nj, sh in runs:
                                sv = s0 + j0
                                nc_b.sync.dma_start(
                                    out=bass.AP(
                                        tensor=tab[:, :].tensor,
                                        offset=(sv * cfg.stratum + c * 128 + sh)
                                        * EWP,
                                        ap=[[EWP, 128],
                                            [cfg.stratum * EWP, nj],
                                            [1, PAY]],
                                    ),
                                    in_=ht[:, j0 : j0 + nj, :],
                                )
                            s0 += ns

            # ---- edge phase ----
            # Software-pipelined: gathers run 2 groups ahead, softmax prep
            # (e/leaky/exp/den/recip/alpha-expansion) 1 group ahead of the
            # heavy weighted-sum work, so no engine head-of-line blocks.
            GCH = 8

            def edge_phase(layer):
                tab = table1 if layer == 1 else table2
                offs = []
                off = 0
                for g in range(G):
                    offs.append(off)
                    off += 8 * (cfg.ka[g] + cfg.kb[g])
                state = {}

                def stage_gather(g):
                    KA, KB = cfg.ka[g], cfg.kb[g]
                    K = KA + KB
                    ixg = ix.tile([128, 8 * K], I16, tag="ixg")
                    nc_b.sync.dma_start(
                        out=ixg[:], in_=idx[:, offs[g] : offs[g] + 8 * K]
                    )
                    gt = ge.tile([128, K * EWP], F16, tag="gt")
                    gta = gt[:]
                    for base, kn, ioff, in_ap in (
                        (0, KA, 0, tab[0:ROW_B0, :]),
                        (KA, KB, 8 * KA, tab[ROW_B0:R, :]),
                    ):
                        for c0 in range(0, kn, GCH):
                            cw = min(GCH, kn - c0)
                            nc_b.gpsimd.dma_gather(
                                out_ap=bass.AP(
                                    tensor=gta.tensor,
                                    offset=gta.offset + (base + c0) * EWP,
                                    ap=[gta.ap[0], [EWP, cw], [1, EWP]],
                                ),
                                in_ap=in_ap,
                                idxs_ap=ixg[:, ioff + 8 * c0 : ioff + 8 * (c0 + cw)],
                                num_idxs=128 * cw,
                                num_idxs_reg=128 * cw,
                                elem_size=EWP,
                            )
                    state[g] = dict(gt=gt)

                def stage_prep(g):
                    KA, KB = cfg.ka[g], cfg.kb[g]
                    K = KA + KB
                    gta = state[g]["gt"][:]
                    # alpha_dst from own self-loop slot (slot 0 of own side)
                    ad_off = AD_OFF if g < cfg.split_strat else KA * EWP + AD_OFF
                    # e[p, h*K+k] = as(slot k, h) + ad(h)   [Act x4,
                    # ad supplied as a per-partition bias column]
                    e = sm.tile([128, HEADS * K], F32, tag="e")
                    for h in range(HEADS):
                        nc_b.scalar.activation(
                            out=e[:, h * K : (h + 1) * K],
                            in_=bass.AP(
                                tensor=gta.tensor,
                                offset=gta.offset + AS_OFF + h,
                                ap=[gta.ap[0], [EWP, K]],
                            ),
                            func=ACT.Identity,
                            bias=bass.AP(
                                tensor=gta.tensor,
                                offset=gta.offset + ad_off + h,
                                ap=[gta.ap[0], [1, 1]],
                            ),
                        )
                    # leaky relu [DVE]
                    el = sm.tile([128, HEADS * K], F32, tag="el")
                    nc_b.vector.scalar_tensor_tensor(
                        out=el[:], in0=e[:], scalar=NEG_SLOPE, in1=e[:],
                        op0=OP.mult, op1=OP.max,
                    )
                    # p = exp(el) [Act]; den = sum_k p [DVE]; rden [DVE]
                    p = sm.tile([128, HEADS * K], F32, tag="p")
                    nc_b.scalar.activation(out=p[:], in_=el[:], func=ACT.Exp)
                    den = sm.tile([128, HEADS], F32, tag="den")
                    nc_b.vector.tensor_reduce(
                        out=den[:],
                        in_=bass.AP(
                            tensor=p[:].tensor, offset=p[:].offset,
                            ap=[p[:].ap[0], [K, HEADS], [1, K]],
                        ),
                        axis=AX.X, op=OP.add,
                    )
                    rden = sm.tile([128, HEADS], F32, tag="rden")
                    nc_b.vector.reciprocal(out=rden[:], in_=den[:])
                    # pexp[p, h, k, 0:2] = alpha = p * rden_h  (fp16) [Act]
                    # Only 2 copies per alpha: the multiply broadcasts over
                    # the middle (c_hi) dim; DVE 2x only requires the LAST
                    # dim packed.
                    pex = px.tile([128, HEADS, K, 2], F16, tag="pex")
                    for h in range(HEADS):
                        nc_b.scalar.activation(
                            out=bass.AP(
                                tensor=pex[:].tensor,
                                offset=pex[:].offset + h * K * 2,
                                ap=[pex[:].ap[0], [2, K], [1, 2]],
                            ),
                            in_=bass.AP(
                                tensor=p[:].tensor, offset=p[:].offset + h * K,
                                ap=[p[:].ap[0], [1, K], [0, 2]],
                            ),
                            func=ACT.Copy,
                            scale=rden[:, h : h + 1],
                        )
                    state[g]["pex"] = pex

                def stage_mult(g):
                    KA, KB = cfg.ka[g], cfg.kb[g]
                    K = KA + KB
                    gta = state[g]["gt"][:]
                    pex = state[g]["pex"]
                    # tht[p, h, k, c] = alpha * h_src  (fp16, DVE 2x);
                    # split per head so each starts as soon as that head's
                    # alpha expansion lands
                    tht = th.tile([128, HEADS * K * CH], F16, tag="tht")
                    for h in range(HEADS):
                        nc_b.vector.tensor_tensor(
                            out=bass.AP(
                                tensor=tht[:].tensor,
                                offset=tht[:].offset + h * K * CH,
                                ap=[tht[:].ap[0], [CH, K], [2, CH // 2],
                                    [1, 2]],
                            ),
                            in0=bass.AP(
                                tensor=gta.tensor, offset=gta.offset + h * CH,
                                ap=[gta.ap[0], [EWP, K], [2, CH // 2], [1, 2]],
                            ),
                            in1=bass.AP(
                                tensor=pex[:].tensor,
                                offset=pex[:].offset + h * K * 2,
                                ap=[pex[:].ap[0], [2, K], [0, CH // 2],
                                    [1, 2]],
                            ),
                            op=OP.mult,
                        )
                    state[g]["tht"] = tht

                def stage_heavy(g):
                    KA, KB = cfg.ka[g], cfg.kb[g]
                    K = KA + KB
                    pex = state[g]["pex"]
                    tht = state[g]["tht"]
                    # binary-tree reduce over k (fp16 DVE 2x adds); ping-pong
                    # tht <-> thB
                    thB = th.tile([128, HEADS * ((K + 1) // 2) * CH], F16,
                                  tag="thB")
                    opre = sm.tile([128, HC], F16, tag="opre")
                    cur, alt = (tht, K * CH), (thB, ((K + 1) // 2) * CH)
                    n = K
                    while n > 1:
                        (cur_t, HS), (alt_t, HSa) = cur, alt
                        ca_, aa_ = cur_t[:], alt_t[:]
                        if n == 2:
                            nc_b.vector.tensor_tensor(
                                out=bass.AP(
                                    tensor=opre[:].tensor, offset=opre[:].offset,
                                    ap=[opre[:].ap[0], [CH, HEADS], [1, CH]],
                                ),
                                in0=bass.AP(
                                    tensor=ca_.tensor, offset=ca_.offset,
                                    ap=[ca_.ap[0], [HS, HEADS], [1, CH]],
                                ),
                                in1=bass.AP(
                                    tensor=ca_.tensor, offset=ca_.offset + CH,
                                    ap=[ca_.ap[0], [HS, HEADS], [1, CH]],
                                ),
                                op=OP.add,
                            )
                            n = 1
                            continue
                        if n % 2 == 1:
                            # fold straggler (slot n-1) into slot 0 in place
                            nc_b.vector.tensor_tensor(
                                out=bass.AP(
                                    tensor=ca_.tensor, offset=ca_.offset,
                                    ap=[ca_.ap[0], [HS, HEADS], [1, CH]],
                                ),
                                in0=bass.AP(
                                    tensor=ca_.tensor, offset=ca_.offset,
                                    ap=[ca_.ap[0], [HS, HEADS], [1, CH]],
                                ),
                                in1=bass.AP(
                                    tensor=ca_.tensor,
                                    offset=ca_.offset + (n - 1) * CH,
                                    ap=[ca_.ap[0], [HS, HEADS], [1, CH]],
                                ),
                                op=OP.add,
                            )
                            n -= 1
                            continue
                        half = n // 2
                        nc_b.vector.tensor_tensor(
                            out=bass.AP(
                                tensor=aa_.tensor, offset=aa_.offset,
                                ap=[aa_.ap[0], [HSa, HEADS], [CH, half],
                                    [1, CH]],
                            ),
                            in0=bass.AP(
                                tensor=ca_.tensor, offset=ca_.offset,
                                ap=[ca_.ap[0], [HS, HEADS], [2 * CH, half],
                                    [1, CH]],
                            ),
                            in1=bass.AP(
                                tensor=ca_.tensor, offset=ca_.offset + CH,
                                ap=[ca_.ap[0], [HS, HEADS], [2 * CH, half],
                                    [1, CH]],
                            ),
                            op=OP.add,
                        )
                        n = half
                        cur, alt = alt, cur

                    # + bias [DVE]
                    outb = sm.tile([128, HC], F16, tag="outb")
                    nc_b.vector.tensor_tensor(
                        out=outb[:], in0=opre[:],
                        in1=(b1s if layer == 1 else b2s)[:], op=OP.add,
                    )
                    # elu = relu(x) + exp(min(x,0)) - 1.  Layer 1 computes
                    # it exactly; layer 2 computes elu+1 = min(exp(x),1) +
                    # relu(x) (exp overflows for x>~88? no: clamp via min
                    # AFTER exp is exact since exp(x)>=1 iff x>=0) and the -1
                    # is folded into the host-adjusted fc bias.
                    rl = sm.tile([128, HC], F16, tag="rl")
                    nc_b.scalar.activation(out=rl[:], in_=outb[:], func=ACT.Relu)
                    em = sm.tile([128, HC], F16, tag="em")
                    elu = sm.tile([128, HC], F16, tag="elu")
                    if layer == 1:
                        xm = sm.tile([128, HC], F16, tag="xm")
                        nc_b.scalar.activation(out=xm[:], in_=outb[:],
                                               func=ACT.Relu, scale=-1.0)
                        nc_b.scalar.activation(out=em[:], in_=xm[:],
                                               func=ACT.Exp, scale=-1.0)
                        nc_b.vector.scalar_tensor_tensor(
                            out=elu[:], in0=em[:], scalar=-1.0, in1=rl[:],
                            op0=OP.add, op1=OP.add,
                        )
                    else:
                        nc_b.scalar.activation(out=em[:], in_=outb[:],
                                               func=ACT.Exp)
                        nc_b.vector.scalar_tensor_tensor(
                            out=elu[:], in0=em[:], scalar=1.0, in1=rl[:],
                            op0=OP.min, op1=OP.add,
                        )
                    if layer == 1:
                        et = sm.tile([128, 2, 128], F16, tag="et")
                        for half_i in range(2):
                            ptr = pt.tile([128, 128], F16, space="PSUM",
                                          tag="ptr")
                            nc_b.tensor.transpose(
                                out=ptr[:],
                                in_=elu[:, half_i * 128 : (half_i + 1) * 128],
                                identity=idt[:],
                            )
                            nc_b.scalar.copy(out=et[:, half_i, :],
                                             in_=ptr[:])
                        nc_b.sync.dma_start(
                            out=bass.AP(
                                tensor=elu1T[:, :].tensor,
                                offset=g * 128,
                                ap=[[NP // NC, 128], [128 * (NP // NC), 2],
                                    [1, 128]],
                            ),
                            in_=et[:],
                        )
                    else:
                        fsc = sm.tile([128, HC], F32, tag="xm")
                        nc_b.vector.scalar_tensor_tensor(
                            out=fsc[:], in0=elu[:], scalar=1.0, in1=fcws[:],
                            op0=OP.bypass, op1=OP.mult,
                            accum_out=zall[:, g : g + 1],
                        )
                    del state[g]

                stage_gather(0)
                if G > 1:
                    stage_gather(1)
                stage_prep(0)
                for g in range(G):
                    if g + 1 < G:
                        stage_prep(g + 1)
                    if g + 2 < G:
                        stage_gather(g + 2)
                    stage_mult(g)
                    stage_heavy(g)

            import os
            phases = os.environ.get("KM_PHASES", "h1,e1,cc,h2,e2").split(",")
            marks = {}

            def mark(label):
                marks[label] = len(nc_b.inst_map)

            nc_b._phase_marks = marks
            mark("setup_end")
            if "h1" in phases:
                h_phase(1)
            mark("h1_end")
            if "e1" in phases:
                edge_phase(1)
            mark("e1_end")
            if "cc" in phases:
                nc_b.gpsimd.collective_compute(
                    "AllGather",
                    OP.bypass,
                    replica_groups=[list(range(NC))],
                    ins=[elu1T[:, :].opt()],
                    outs=[gath[:, :].opt()],
                )
            mark("cc_end")
            if "h2" in phases:
                h_phase(2)
            mark("h2_end")
            if "e2" in phases:
                edge_phase(2)
            mark("e2_end")

            # final: y = sigmoid(z + fc_b), transposed out
            if "e2" not in phases:
                nc_b.vector.memset(zall[:], 0.0)
            zsig = cst.tile([128, G], F16)
            nc_b.scalar.activation(
                out=zsig[:], in_=zall[:], func=ACT.Sigmoid,
                bias=fcbs[:, 0:1], scale=1.0,
            )
            pz = ph.tile([G, 128], F16, space="PSUM", tag="pz")
            nc_b.tensor.transpose(out=pz[:], in_=zsig[:], identity=idt[:])
            yT = cst.tile([G, 128], F32)  # copy converts f16 psum -> f32
            nc_b.vector.tensor_copy(out=yT[:], in_=pz[:])
            nc_b.sync.dma_start(
                out=bass.AP(
                    tensor=y[:, :].tensor, offset=0, ap=[[128, G], [1, 128]]
                ),
                in_=yT[:],
            )
    nc_b.finalize()
    return nc_b


def make_block_diag(a):
    """a: [H, C] -> [H*C, H] block diagonal."""
    out = np.zeros((HC, HEADS), np.float32)
    for h in range(HEADS):
        out[h * CH : (h + 1) * CH, h] = a[h]
    return out


def _aug(W, a_src, a_dst):
    """[W | W@As_bd | W@Ad_bd | pad] as fp16, width PAY."""
    W = np.asarray(W, np.float32)
    aug = np.zeros((W.shape[0], PAY), np.float32)
    aug[:, 0:HC] = W
    aug[:, HC : HC + HEADS] = W @ make_block_diag(np.asarray(a_src, np.float32))
    aug[:, HC + HEADS : HC + 2 * HEADS] = W @ make_block_diag(
        np.asarray(a_dst, np.float32)
    )
    return aug.astype(np.float16)


def build_inputs(cfg: Cfg, layout, x, W1, a_src1, a_dst1, b1, W2, a_src2,
                 a_dst2, b2, fc_w, fc_b):
    NP = cfg.np_
    node_of_sp = layout["node_of_sp"]
    xs = np.zeros((NP, DIN), np.float32)
    valid = node_of_sp >= 0
    xs[valid] = np.asarray(x, np.float32)[node_of_sp[valid]]
    xT = np.ascontiguousarray(xs.T).astype(np.float16)

    dumrow = np.zeros((3, EWP), np.float16)
    dumrow[0:2, AS_OFF : AS_OFF + HEADS] = DUM_AS

    base = dict(
        xT=xT,
        W1AUG=_aug(W1, a_src1, a_dst1),
        W2AUG=_aug(W2, a_src2, a_dst2),
        b1t=np.broadcast_to(np.asarray(b1, np.float16), (128, HC)).copy(),
        b2t=np.broadcast_to(np.asarray(b2, np.float16), (128, HC)).copy(),
        fcwt=np.broadcast_to(
            np.asarray(fc_w, np.float32).reshape(1, HC), (128, HC)
        ).copy(),
        # layer 2 produces elu+1; the -1 contribution is folded here:
        # z = sum((elu+1) * fcw) + (fc_b - sum(fcw))
        fcbt=np.full(
            (128, 1),
            np.float32(
                np.asarray(fc_b).reshape(-1)[0]
                - np.asarray(fc_w, np.float32).sum()
            ),
        ),
        identh=np.eye(128, dtype=np.float16),
        dumrow=dumrow,
    )
    in_maps = []
    for c in range(cfg.nc):
        m = dict(base)
        m["idx"] = layout["idx"][c]
        in_maps.append(m)
    return in_maps


def assemble_output(cfg: Cfg, layout, results):
    node_of_sp = layout["node_of_sp"]
    yfull = np.zeros((cfg.n_real, 1), np.float32)
    for c in range(cfg.nc):
        yc = results[c]["y"].reshape(-1)       # [PC] local order (g*128 + p)
        loc = np.arange(cfg.pc)
        sp = (loc // 128) * cfg.stratum + c * 128 + (loc % 128)
        nodes = node_of_sp[sp]
        ok = nodes >= 0
        yfull[nodes[ok], 0] = yc[ok]
    return yfull


def _absorb_device_wedge():
    """Run a trivial 8-core kernel; a crashed prior session can leave the
    NeuronCores in NRT_EXEC_UNIT_UNRECOVERABLE state for the next session,
    which a fresh trivial execution clears."""
    try:
        from concourse.bass_utils import run_bass_kernel_spmd

        nc_t = bacc.Bacc(None, num_devices=8)
        a = nc_t.dram_tensor("a", [128, 128], F32, kind="ExternalInput")
        o = nc_t.dram_tensor("o", [128, 128], F32, kind="ExternalOutput")
        with tile.TileContext(nc_t) as tc:
            with tc.tile_pool(name="sb", bufs=1) as sb:
                t = sb.tile([128, 128], F32)
                nc_t.sync.dma_start(out=t[:], in_=a[:, :])
                nc_t.sync.dma_start(out=o[:, :], in_=t[:])
        nc_t.finalize()
        run_bass_kernel_spmd(
            nc_t, [{"a": np.zeros((128, 128), np.float32)}] * 8,
            core_ids=list(range(8)),
        )
    except Exception:
        pass


def kernel(**inputs):
    from concourse.bass_utils import run_bass_kernel_spmd

    cfg = Cfg()
    layout = build_layout(inputs["edge_index"], cfg)
    in_maps = build_inputs(
        cfg, layout,
        inputs["x"], inputs["W1"], inputs["a_src1"], inputs["a_dst1"],
        inputs["b1"], inputs["W2"], inputs["a_src2"], inputs["a_dst2"],
        inputs["b2"], inputs["fc_w"], inputs["fc_b"],
    )
    nc_b = build_program(cfg, shared_out=True)
    last_err = None
    for attempt in range(3):
        try:
            res = run_bass_kernel_spmd(
                nc_b, in_maps, core_ids=list(range(cfg.nc))
            )
            return assemble_output(cfg, layout, res.results)
        except Exception as e:  # wedged device from a prior crashed session
            last_err = e
            _absorb_device_wedge()
    raise last_err


if __name__ == "__main__":
    pass


# revision 44
# speedup vs baseline: 1.0078x; 1.0078x over previous
"""Two-layer GAT (PyG GATConv-style) on 8 Trainium2 NeuronCores via Bass/Tile.

Strategy (dst-sharded, data-parallel, fp16 data path):
  - Nodes are assigned to (core, group) "strata": all 8 cores' group g hold
    nodes with similar in-degree statistics (snake-ordered by (cA, cB)), so a
    single SPMD program with uniform per-group slot counts serves all cores.
  - Each core redundantly computes the dense part h = x @ W (plus fused
    alpha_src/alpha_dst columns) for ALL nodes and writes a DRAM row table
    (fp16 row, 768B pitch = [h(256) | a_src(4) | a_dst(4) | pad]); the edge
    phase (gather h[src], per-dst softmax over incoming edges, weighted sum)
    runs only on the core's own 1/8 of destination nodes via dma_gather with
    host-precomputed int16 index lists (slot-major, per-dst-row layout).
  - int16 gather indices only reach 32767, so the table is split into an A
    range and a B range (split aligned to a stratum boundary); every dst row's
    slots are [A-slots | B-slots] with per-group uniform counts KA/KB. Slot 0
    of a node's own side is its self-loop, which supplies alpha_dst.
  - Edge math: e = leaky(as+ad) (no max-shift; dummy slots use as=-60000 so
    exp underflows to 0), p = exp(e), den = sum_k p, alpha = p/den fused into
    an Act-engine per-head expansion to [k, c] fp16, weighted products via a
    DVE 2x-mode tensor_tensor, then a binary-tree fp16 reduction over slots.
  - Between layers, each core's elu(out1)^T block is AllGathered (fp16) so
    layer 2's replicated dense phase can read every node's features.
  - Final sigmoid(fc) output is computed per-core and unpermuted on the host.
"""

import sys

sys.path.insert(0, "/opt/trn_rl_repo")

from dataclasses import dataclass, field

import numpy as np

import concourse.bass as bass
import concourse.bacc as bacc
import concourse.tile as tile
from concourse import mybir
from concourse.library_config import mlp as MLP_LIB

F32 = mybir.dt.float32
F16 = mybir.dt.float16
I16 = mybir.dt.int16
AX = mybir.AxisListType
OP = mybir.AluOpType
ACT = mybir.ActivationFunctionType

HEADS = 4
CH = 64
HC = HEADS * CH          # 256
DIN = 128
NEG_SLOPE = 0.2
EWP = 384                # table row pitch in fp16 elems (768 B, %256==0)
PAY = 272                # payload elems per row: h(256) | as(4) | ad(4) | pad8
AS_OFF = 256             # alpha_src column offset within a row
AD_OFF = 260             # alpha_dst column offset
DUM_AS = -60000.0        # dummy-slot alpha_src (finite; exp underflows to 0)


@dataclass
class Cfg:
    n_real: int = 50000
    nc: int = 8                  # cores
    g: int = 49                  # groups (strata) per core
    split_strat: int = 31        # strata in table range A
    ka: list = field(default_factory=list)   # per-group A slots
    kb: list = field(default_factory=list)   # per-group B slots

    @property
    def stratum(self):
        return self.nc * 128

    @property
    def np_(self):
        return self.g * self.stratum

    @property
    def pc(self):
        return self.g * 128

    @property
    def split_sp(self):
        return self.split_strat * self.stratum

    @property
    def nb(self):
        return self.np_ - self.split_sp

    @property
    def rows(self):
        return self.np_ + 3      # dumA + dumB + padself

    @property
    def row_b0(self):
        return self.split_sp + 1   # dummyB row

    @property
    def padself_idx(self):
        return self.nb + 1         # local B index of the pad-self row


def build_layout(edge_index, cfg: Cfg):
    """Host-side graph layout. Returns per-core index arrays + permutation."""
    n = cfg.n_real
    src = np.asarray(edge_index[0], dtype=np.int64)
    dst = np.asarray(edge_index[1], dtype=np.int64)
    src = np.concatenate([src, np.arange(n, dtype=np.int64)])
    dst = np.concatenate([dst, np.arange(n, dtype=np.int64)])
    deg = np.bincount(dst, minlength=n)

    split = cfg.split_sp
    order1 = np.argsort(-deg, kind="stable")
    a_nodes = order1[:split]
    b_nodes = order1[split:]
    is_a = np.zeros(n, bool)
    is_a[a_nodes] = True
    ca = np.bincount(dst[is_a[src]], minlength=n)
    cb = deg - ca

    def snake(nodes):
        out = []
        flip = False
        avals = ca[nodes]
        for v in np.sort(np.unique(avals))[::-1]:
            grp = nodes[avals == v]
            grp = grp[np.argsort(cb[grp], kind="stable")]
            if flip:
                grp = grp[::-1]
            flip = not flip
            out.append(grp)
        return np.concatenate(out) if out else nodes

    seq = np.concatenate([snake(a_nodes), snake(b_nodes)])
    # node_of_sp: storage position -> node (-1 = pad). pads go at the end.
    node_of_sp = np.full(cfg.np_, -1, np.int64)
    node_of_sp[: len(seq)] = seq
    sp_of = np.full(n, -1, np.int64)
    sp_of[seq] = np.arange(len(seq))

    # per-group slot counts
    ka = np.zeros(cfg.g, np.int64)
    kb = np.zeros(cfg.g, np.int64)
    for s in range(cfg.g):
        nodes = node_of_sp[s * cfg.stratum : (s + 1) * cfg.stratum]
        nodes = nodes[nodes >= 0]
        if len(nodes):
            ka[s] = ca[nodes].max()
            kb[s] = cb[nodes].max()
    ka = np.maximum(ka, 1)
    kb = np.maximum(kb, 1)
    # pads live in the last stratum; their pad-self slot is B slot 0
    cfg.ka = ka.tolist()
    cfg.kb = kb.tolist()

    # group edges by dst
    eorder = np.argsort(dst, kind="stable")
    s_sorted = src[eorder]
    starts = np.searchsorted(dst, np.arange(n + 1), sorter=eorder)

    sp_src = sp_of[s_sorted]           # storage pos of each edge's src
    src_is_a = sp_src < split
    idx_a_val = (sp_src + 1).astype(np.int64)            # A-range row index
    idx_b_val = (sp_src - split + 1).astype(np.int64)    # B-range local row

    def pack(flat):
        # idx element i lives at [i % 16, i // 16]; the two Q7 DGE cores read
        # partitions 0-15 and 16-31 respectively, so replicate into both.
        ln = len(flat)
        f = np.zeros((128, ln // 16), np.int16)
        ii = np.arange(ln)
        f[ii % 16, ii // 16] = flat
        f[16 + (ii % 16), ii // 16] = flat
        return f

    idx_cores = []
    for c in range(cfg.nc):
        flats = []
        for s in range(cfg.g):
            KA, KB = int(ka[s]), int(kb[s])
            ma = np.zeros((KA, 128), np.int64)   # [slot, partition]
            mb = np.zeros((KB, 128), np.int64)
            base_sp = s * cfg.stratum + c * 128
            for p in range(128):
                node = node_of_sp[base_sp + p]
                if node < 0:
                    # pad position: pad-self row keeps den positive
                    mb[0, p] = cfg.padself_idx
                    continue
                lo, hi = starts[node], starts[node + 1]
                aa = idx_a_val[lo:hi][src_is_a[lo:hi]]
                bb = idx_b_val[lo:hi][~src_is_a[lo:hi]]
                # self-loop first in its own side (supplies alpha_dst)
                own_sp = base_sp + p
                if own_sp < split:
                    self_idx = own_sp + 1
                    w = np.where(aa == self_idx)[0]
                    if len(w) == 0:
                        raise RuntimeError("self-loop missing in A list")
                    aa[[0, w[0]]] = aa[[w[0], 0]]
                else:
                    self_idx = own_sp - split + 1
                    w = np.where(bb == self_idx)[0]
                    if len(w) == 0:
                        raise RuntimeError("self-loop missing in B list")
                    bb[[0, w[0]]] = bb[[w[0], 0]]
                ma[: len(aa), p] = aa
                mb[: len(bb), p] = bb
            flats.append(
                np.concatenate([pack(ma.reshape(-1)), pack(mb.reshape(-1))],
                               axis=1)
            )
        idx_cores.append(np.concatenate(flats, axis=1))

    return dict(node_of_sp=node_of_sp, sp_of=sp_of, idx=idx_cores)


def build_program(cfg: Cfg, shared_out: bool = True):
    nc_b = bacc.Bacc(None, num_devices=cfg.nc)
    NP, G, NC = cfg.np_, cfg.g, cfg.nc
    NT = NP // 128                 # global tiles (stratum-major)
    SPLIT_T = cfg.split_sp // 128  # first B tile
    R = cfg.rows
    ROW_B0 = cfg.row_b0
    PC = cfg.pc
    sumK8 = 8 * int(np.sum(cfg.ka) + np.sum(cfg.kb))

    xT = nc_b.dram_tensor("xT", [DIN, NP], F16, kind="ExternalInput")
    W1AUG = nc_b.dram_tensor("W1AUG", [DIN, PAY], F16, kind="ExternalInput")
    W2AUG = nc_b.dram_tensor("W2AUG", [HC, PAY], F16, kind="ExternalInput")
    bmt = nc_b.dram_tensor("bmt", [1, 128], F16, kind="ExternalInput")
    br1 = nc_b.dram_tensor("br1", [1, PAY], F16, kind="ExternalInput")
    br2 = nc_b.dram_tensor("br2", [1, PAY], F16, kind="ExternalInput")
    fcwt = nc_b.dram_tensor("fcwt", [128, HC], F32, kind="ExternalInput")
    fcbt = nc_b.dram_tensor("fcbt", [128, 1], F32, kind="ExternalInput")
    identh = nc_b.dram_tensor("identh", [128, 128], F16, kind="ExternalInput")
    dumrow = nc_b.dram_tensor("dumrow", [3, EWP], F16, kind="ExternalInput")
    idx = nc_b.dram_tensor("idx", [128, sumK8], I16, kind="ExternalInput")
    y = nc_b.dram_tensor("y", [PC, 1], F32, kind="ExternalOutput")

    table1 = nc_b.dram_tensor("table1", [R, EWP], F16)
    table2 = nc_b.dram_tensor("table2", [R, EWP], F16)
    elu1T = nc_b.dram_tensor("elu1T", [HC, NP // NC], F16)
    gath = nc_b.dram_tensor(
        "gath", [NC * HC, NP // NC], F16,
        addr_space="Shared" if shared_out else "Local",
    )

    with tile.TileContext(nc_b) as tc:
        ctxmgrs = [
            tc.tile_pool(name="cst", bufs=1),
            tc.tile_pool(name="lw", bufs=2),
            tc.tile_pool(name="hp", bufs=2),
            tc.tile_pool(name="ge", bufs=3),
            tc.tile_pool(name="px", bufs=2),
            tc.tile_pool(name="th", bufs=1),
            tc.tile_pool(name="sm", bufs=2),
            tc.tile_pool(name="ix", bufs=2),
            tc.tile_pool(name="ph", bufs=1, space="PSUM"),
            tc.tile_pool(name="pt", bufs=2, space="PSUM"),
        ]
        import contextlib

        with contextlib.ExitStack() as st:
            cst, lw, hp, ge, px, th, sm, ix, ph, pt = [
                st.enter_context(m) for m in ctxmgrs
            ]
            nc_b.gpsimd.load_library(MLP_LIB)

            # ---- constants ----
            idt = cst.tile([128, 128], F16)
            nc_b.sync.dma_start(out=idt[:], in_=identh[:, :])
            bms = cst.tile([1, 128], F16)
            nc_b.sync.dma_start(out=bms[:], in_=bmt[:, :])
            br1s = cst.tile([1, PAY], F16)
            nc_b.sync.dma_start(out=br1s[:], in_=br1[:, :])
            br2s = cst.tile([1, PAY], F16)
            nc_b.sync.dma_start(out=br2s[:], in_=br2[:, :])
            fcws = cst.tile([128, HC], F32)
            nc_b.sync.dma_start(out=fcws[:], in_=fcwt[:, :])
            fcbs = cst.tile([128, 1], F32)
            nc_b.sync.dma_start(out=fcbs[:], in_=fcbt[:, :])
            w1s = cst.tile([128, PAY], F16)
            nc_b.sync.dma_start(out=w1s[:], in_=W1AUG[:, :])
            w2s = cst.tile([128, 2, PAY], F16)
            nc_b.sync.dma_start(out=w2s[:, 0, :], in_=W2AUG[0:128, :])
            nc_b.sync.dma_start(out=w2s[:, 1, :], in_=W2AUG[128:HC, :])
            zall = cst.tile([128, G], F32)

            # dummy + pad-self rows for both tables (DRAM -> DRAM)
            for tab in (table1, table2):
                nc_b.sync.dma_start(out=tab[0:1, :], in_=dumrow[0:1, :])
                nc_b.sync.dma_start(
                    out=tab[ROW_B0 : ROW_B0 + 1, :], in_=dumrow[1:2, :]
                )
                nc_b.sync.dma_start(out=tab[R - 1 : R, :], in_=dumrow[2:3, :])

            # ---- dense phase (replicated): h|alphas -> table ----
            def h_phase(layer):
                tab = table1 if layer == 1 else table2
                cp_rr = [0]

                def copy_psum(dst_ap, src_ap):
                    e = cp_rr[0] % 2
                    cp_rr[0] += 1
                    if e == 0:
                        nc_b.scalar.copy(out=dst_ap, in_=src_ap)
                    else:
                        nc_b.vector.tensor_copy(out=dst_ap, in_=src_ap)

                def write_rows(ht_slice, row0, nt):
                    # ht_slice[:, j, :] (j in 0..nt) -> rows row0 + j*128 + p
                    nc_b.sync.dma_start(
                        out=bass.AP(
                            tensor=tab[:, :].tensor,
                            offset=row0 * EWP,
                            ap=[[EWP, 128], [128 * EWP, nt], [1, PAY]],
                        ),
                        in_=ht_slice,
                    )

                if layer == 1:
                    TB = 12
                    t0 = 0
                    while t0 < NT:
                        nt = min(TB, NT - t0)
                        lh = lw.tile([128, TB * 128], F16, tag="gh")
                        nc_b.sync.dma_start(
                            out=lh[:, 0 : nt * 128],
                            in_=xT[:, t0 * 128 : (t0 + nt) * 128],
                        )
                        ht = hp.tile([128, TB, PAY], F16, tag="ht")
                        for j in range(nt):
                            pj = ph.tile([128, PAY], F32, space="PSUM",
                                         tag=f"pj{j % 5}")
                            nc_b.tensor.matmul(
                                out=pj[:], lhsT=lh[:, j * 128 : (j + 1) * 128],
                                rhs=w1s[:], start=True, stop=False,
                            )
                            nc_b.tensor.matmul(
                                out=pj[:], lhsT=bms[:], rhs=br1s[:],
                                start=False, stop=True,
                            )
                            copy_psum(ht[:, j, :], pj[:])
                        # contiguous row runs (split at the A/B boundary)
                        runs = []
                        for j in range(nt):
                            t = t0 + j
                            sh = 1 if t < SPLIT_T else 2
                            if runs and runs[-1][2] == sh:
                                runs[-1][1] += 1
                            else:
                                runs.append([j, 1, sh])
                        for j0, nj, sh in runs:
                            write_rows(ht[:, j0 : j0 + nj, :],
                                       (t0 + j0) * 128 + sh, nj)
                        t0 += nt
                else:
                    TB = 11
                    for c in range(NC):
                        s0 = 0
                        while s0 < G:
                            ns = min(TB, G - s0)
                            gh = lw.tile([128, 2, TB * 128], F16, tag="gh")
                            nc_b.sync.dma_start(
                                out=gh[:, :, 0 : ns * 128],
                                in_=bass.AP(
                                    tensor=gath[:, :].tensor,
                                    offset=(c * HC) * (NP // NC) + s0 * 128,
                                    ap=[[NP // NC, 128],
                                        [128 * (NP // NC), 2],
                                        [1, ns * 128]],
                                ),
                            )
                            ht = hp.tile([128, TB, PAY], F16, tag="ht")
                            for j in range(ns):
                                pj = ph.tile([128, PAY], F32, space="PSUM",
                                             tag=f"pj{j % 5}")
                                nc_b.tensor.matmul(
                                    out=pj[:],
                                    lhsT=gh[:, 0, j * 128 : (j + 1) * 128],
                                    rhs=w2s[:, 0, :], start=True, stop=False,
                                )
                                nc_b.tensor.matmul(
                                    out=pj[:],
                                    lhsT=gh[:, 1, j * 128 : (j + 1) * 128],
                                    rhs=w2s[:, 1, :], start=False, stop=False,
                                )
                                nc_b.tensor.matmul(
                                    out=pj[:], lhsT=bms[:], rhs=br2s[:],
                                    start=False, stop=True,
                                )
                                copy_psum(ht[:, j, :], pj[:])
                            runs = []
                            for j in range(ns):
                                sv = s0 + j
                                sh = 1 if sv < cfg.split_strat else 2
                                if runs and runs[-1][2] == sh:
                                    runs[-1][1] += 1
                                else:
                                    runs.append([j, 1, sh])
                            for j0, nj, sh in runs:
                                sv = s0 + j0
                                nc_b.sync.dma_start(
                                    out=bass.AP(
                                        tensor=tab[:, :].tensor,
                                        offset=(sv * cfg.stratum + c * 128 + sh)
                                        * EWP,
                                        ap=[[EWP, 128],
                                            [cfg.stratum * EWP, nj],
                                            [1, PAY]],
                                    ),
                                    in_=ht[:, j0 : j0 + nj, :],
                                )
                            s0 += ns

            # ---- edge phase ----
            # Software-pipelined: gathers run 2 groups ahead, softmax prep
            # (e/leaky/exp/den/recip/alpha-expansion) 1 group ahead of the
            # heavy weighted-sum work, so no engine head-of-line blocks.
            GCH = 8

            def edge_phase(layer):
                tab = table1 if layer == 1 else table2
                offs = []
                off = 0
                for g in range(G):
                    offs.append(off)
                    off += 8 * (cfg.ka[g] + cfg.kb[g])
                state = {}

                def stage_gather(g):
                    KA, KB = cfg.ka[g], cfg.kb[g]
                    K = KA + KB
                    ixg = ix.tile([128, 8 * K], I16, tag="ixg")
                    nc_b.sync.dma_start(
                        out=ixg[:], in_=idx[:, offs[g] : offs[g] + 8 * K]
                    )
                    gt = ge.tile([128, K * EWP], F16, tag="gt")
                    gta = gt[:]
                    for base, kn, ioff, in_ap in (
                        (0, KA, 0, tab[0:ROW_B0, :]),
                        (KA, KB, 8 * KA, tab[ROW_B0:R, :]),
                    ):
                        for c0 in range(0, kn, GCH):
                            cw = min(GCH, kn - c0)
                            nc_b.gpsimd.dma_gather(
                                out_ap=bass.AP(
                                    tensor=gta.tensor,
                                    offset=gta.offset + (base + c0) * EWP,
                                    ap=[gta.ap[0], [EWP, cw], [1, EWP]],
                                ),
                                in_ap=in_ap,
                                idxs_ap=ixg[:, ioff + 8 * c0 : ioff + 8 * (c0 + cw)],
                                num_idxs=128 * cw,
                                num_idxs_reg=128 * cw,
                                elem_size=EWP,
                            )
                    state[g] = dict(gt=gt)

                def stage_prep(g):
                    KA, KB = cfg.ka[g], cfg.kb[g]
                    K = KA + KB
                    gta = state[g]["gt"][:]
                    # alpha_dst from own self-loop slot (slot 0 of own side)
                    ad_off = AD_OFF if g < cfg.split_strat else KA * EWP + AD_OFF
                    # e[p, h*K+k] = as(slot k, h) + ad(h)   [Act x4,
                    # ad supplied as a per-partition bias column]
                    e = sm.tile([128, HEADS * K], F32, tag="e")
                    for h in range(HEADS):
                        nc_b.scalar.activation(
                            out=e[:, h * K : (h + 1) * K],
                            in_=bass.AP(
                                tensor=gta.tensor,
                                offset=gta.offset + AS_OFF + h,
                                ap=[gta.ap[0], [EWP, K]],
                            ),
                            func=ACT.Identity,
                            bias=bass.AP(
                                tensor=gta.tensor,
                                offset=gta.offset + ad_off + h,
                                ap=[gta.ap[0], [1, 1]],
                            ),
                        )
                    # leaky relu [DVE]
                    el = sm.tile([128, HEADS * K], F32, tag="el")
                    nc_b.vector.scalar_tensor_tensor(
                        out=el[:], in0=e[:], scalar=NEG_SLOPE, in1=e[:],
                        op0=OP.mult, op1=OP.max,
                    )
                    # p = exp(el) [Act]; den = sum_k p [DVE]; rden [DVE]
                    p = sm.tile([128, HEADS * K], F32, tag="p")
                    nc_b.scalar.activation(out=p[:], in_=el[:], func=ACT.Exp)
                    den = sm.tile([128, HEADS], F32, tag="den")
                    nc_b.vector.tensor_reduce(
                        out=den[:],
                        in_=bass.AP(
                            tensor=p[:].tensor, offset=p[:].offset,
                            ap=[p[:].ap[0], [K, HEADS], [1, K]],
                        ),
                        axis=AX.X, op=OP.add,
                    )
                    rden = sm.tile([128, HEADS], F32, tag="rden")
                    nc_b.vector.reciprocal(out=rden[:], in_=den[:])
                    # pexp[p, h, k, 0:2] = alpha = p * rden_h  (fp16) [Act]
                    # Only 2 copies per alpha: the multiply broadcasts over
                    # the middle (c_hi) dim; DVE 2x only requires the LAST
                    # dim packed.
                    pex = px.tile([128, HEADS, K, 2], F16, tag="pex")
                    for h in range(HEADS):
                        nc_b.scalar.activation(
                            out=bass.AP(
                                tensor=pex[:].tensor,
                                offset=pex[:].offset + h * K * 2,
                                ap=[pex[:].ap[0], [2, K], [1, 2]],
                            ),
                            in_=bass.AP(
                                tensor=p[:].tensor, offset=p[:].offset + h * K,
                                ap=[p[:].ap[0], [1, K], [0, 2]],
                            ),
                            func=ACT.Copy,
                            scale=rden[:, h : h + 1],
                        )
                    state[g]["pex"] = pex

                def stage_mult(g):
                    KA, KB = cfg.ka[g], cfg.kb[g]
                    K = KA + KB
                    gta = state[g]["gt"][:]
                    pex = state[g]["pex"]
                    # tht[p, h, k, c] = alpha * h_src  (fp16, DVE 2x);
                    # split per head so each starts as soon as that head's
                    # alpha expansion lands
                    tht = th.tile([128, HEADS * K * CH], F16, tag="tht")
                    for h in range(HEADS):
                        nc_b.vector.tensor_tensor(
                            out=bass.AP(
                                tensor=tht[:].tensor,
                                offset=tht[:].offset + h * K * CH,
                                ap=[tht[:].ap[0], [CH, K], [2, CH // 2],
                                    [1, 2]],
                            ),
                            in0=bass.AP(
                                tensor=gta.tensor, offset=gta.offset + h * CH,
                                ap=[gta.ap[0], [EWP, K], [2, CH // 2], [1, 2]],
                            ),
                            in1=bass.AP(
                                tensor=pex[:].tensor,
                                offset=pex[:].offset + h * K * 2,
                                ap=[pex[:].ap[0], [2, K], [0, CH // 2],
                                    [1, 2]],
                            ),
                            op=OP.mult,
                        )
                    state[g]["tht"] = tht

                def stage_heavy(g):
                    KA, KB = cfg.ka[g], cfg.kb[g]
                    K = KA + KB
                    pex = state[g]["pex"]
                    tht = state[g]["tht"]
                    # binary-tree reduce over k (fp16 DVE 2x adds); ping-pong
                    # tht <-> thB
                    thB = th.tile([128, HEADS * ((K + 1) // 2) * CH], F16,
                                  tag="thB")
                    opre = sm.tile([128, HC], F16, tag="opre")
                    cur, alt = (tht, K * CH), (thB, ((K + 1) // 2) * CH)
                    n = K
                    while n > 1:
                        (cur_t, HS), (alt_t, HSa) = cur, alt
                        ca_, aa_ = cur_t[:], alt_t[:]
                        if n == 2:
                            nc_b.vector.tensor_tensor(
                                out=bass.AP(
                                    tensor=opre[:].tensor, offset=opre[:].offset,
                                    ap=[opre[:].ap[0], [CH, HEADS], [1, CH]],
                                ),
                                in0=bass.AP(
                                    tensor=ca_.tensor, offset=ca_.offset,
                                    ap=[ca_.ap[0], [HS, HEADS], [1, CH]],
                                ),
                                in1=bass.AP(
                                    tensor=ca_.tensor, offset=ca_.offset + CH,
                                    ap=[ca_.ap[0], [HS, HEADS], [1, CH]],
                                ),
                                op=OP.add,
                            )
                            n = 1
                            continue
                        if n % 2 == 1:
                            # fold straggler (slot n-1) into slot 0 in place
                            nc_b.vector.tensor_tensor(
                                out=bass.AP(
                                    tensor=ca_.tensor, offset=ca_.offset,
                                    ap=[ca_.ap[0], [HS, HEADS], [1, CH]],
                                ),
                                in0=bass.AP(
                                    tensor=ca_.tensor, offset=ca_.offset,
                                    ap=[ca_.ap[0], [HS, HEADS], [1, CH]],
                                ),
                                in1=bass.AP(
                                    tensor=ca_.tensor,
                                    offset=ca_.offset + (n - 1) * CH,
                                    ap=[ca_.ap[0], [HS, HEADS], [1, CH]],
                                ),
                                op=OP.add,
                            )
                            n -= 1
                            continue
                        half = n // 2
                        nc_b.vector.tensor_tensor(
                            out=bass.AP(
                                tensor=aa_.tensor, offset=aa_.offset,
                                ap=[aa_.ap[0], [HSa, HEADS], [CH, half],
                                    [1, CH]],
                            ),
                            in0=bass.AP(
                                tensor=ca_.tensor, offset=ca_.offset,
                                ap=[ca_.ap[0], [HS, HEADS], [2 * CH, half],
                                    [1, CH]],
                            ),
                            in1=bass.AP(
                                tensor=ca_.tensor, offset=ca_.offset + CH,
                                ap=[ca_.ap[0], [HS, HEADS], [2 * CH, half],
                                    [1, CH]],
                            ),
                            op=OP.add,
                        )
                        n = half
                        cur, alt = alt, cur

                    # bias already folded into the table rows (rank-1
                    # matmul in the dense phase; softmax weights sum to 1)
                    outb = opre
                    # elu = relu(x) + exp(min(x,0)) - 1.  Layer 1 computes
                    # it exactly; layer 2 computes elu+1 = min(exp(x),1) +
                    # relu(x) (exp overflows for x>~88? no: clamp via min
                    # AFTER exp is exact since exp(x)>=1 iff x>=0) and the -1
                    # is folded into the host-adjusted fc bias.
                    rl = sm.tile([128, HC], F16, tag="rl")
                    nc_b.scalar.activation(out=rl[:], in_=outb[:], func=ACT.Relu)
                    em = sm.tile([128, HC], F16, tag="em")
                    elu = sm.tile([128, HC], F16, tag="elu")
                    if layer == 1:
                        xm = sm.tile([128, HC], F16, tag="xm")
                        nc_b.scalar.activation(out=xm[:], in_=outb[:],
                                               func=ACT.Relu, scale=-1.0)
                        nc_b.scalar.activation(out=em[:], in_=xm[:],
                                               func=ACT.Exp, scale=-1.0)
                        nc_b.vector.scalar_tensor_tensor(
                            out=elu[:], in0=em[:], scalar=-1.0, in1=rl[:],
                            op0=OP.add, op1=OP.add,
                        )
                    else:
                        nc_b.scalar.activation(out=em[:], in_=outb[:],
                                               func=ACT.Exp)
                        nc_b.vector.scalar_tensor_tensor(
                            out=elu[:], in0=em[:], scalar=1.0, in1=rl[:],
                            op0=OP.min, op1=OP.add,
                        )
                    if layer == 1:
                        et = sm.tile([128, 2, 128], F16, tag="et")
                        for half_i in range(2):
                            ptr = pt.tile([128, 128], F16, space="PSUM",
                                          tag="ptr")
                            nc_b.tensor.transpose(
                                out=ptr[:],
                                in_=elu[:, half_i * 128 : (half_i + 1) * 128],
                                identity=idt[:],
                            )
                            nc_b.scalar.copy(out=et[:, half_i, :],
                                             in_=ptr[:])
                        nc_b.sync.dma_start(
                            out=bass.AP(
                                tensor=elu1T[:, :].tensor,
                                offset=g * 128,
                                ap=[[NP // NC, 128], [128 * (NP // NC), 2],
                                    [1, 128]],
                            ),
                            in_=et[:],
                        )
                    else:
                        fsc = sm.tile([128, HC], F32, tag="xm")
                        nc_b.vector.scalar_tensor_tensor(
                            out=fsc[:], in0=elu[:], scalar=1.0, in1=fcws[:],
                            op0=OP.bypass, op1=OP.mult,
                            accum_out=zall[:, g : g + 1],
                        )
                    del state[g]

                stage_gather(0)
                if G > 1:
                    stage_gather(1)
                stage_prep(0)
                for g in range(G):
                    if g + 1 < G:
                        stage_prep(g + 1)
                    if g + 2 < G:
                        stage_gather(g + 2)
                    stage_mult(g)
                    stage_heavy(g)

            import os
            phases = os.environ.get("KM_PHASES", "h1,e1,cc,h2,e2").split(",")
            marks = {}

            def mark(label):
                marks[label] = len(nc_b.inst_map)

            nc_b._phase_marks = marks
            mark("setup_end")
            if "h1" in phases:
                h_phase(1)
            mark("h1_end")
            if "e1" in phases:
                edge_phase(1)
            mark("e1_end")
            if "cc" in phases:
                nc_b.gpsimd.collective_compute(
                    "AllGather",
                    OP.bypass,
                    replica_groups=[list(range(NC))],
                    ins=[elu1T[:, :].opt()],
                    outs=[gath[:, :].opt()],
                )
            mark("cc_end")
            if "h2" in phases:
                h_phase(2)
            mark("h2_end")
            if "e2" in phases:
                edge_phase(2)
            mark("e2_end")

            # final: y = sigmoid(z + fc_b), transposed out
            if "e2" not in phases:
                nc_b.vector.memset(zall[:], 0.0)
            zsig = cst.tile([128, G], F16)
            nc_b.scalar.activation(
                out=zsig[:], in_=zall[:], func=ACT.Sigmoid,
                bias=fcbs[:, 0:1], scale=1.0,
            )
            pz = ph.tile([G, 128], F16, space="PSUM", tag="pz")
            nc_b.tensor.transpose(out=pz[:], in_=zsig[:], identity=idt[:])
            yT = cst.tile([G, 128], F32)  # copy converts f16 psum -> f32
            nc_b.vector.tensor_copy(out=yT[:], in_=pz[:])
            nc_b.sync.dma_start(
                out=bass.AP(
                    tensor=y[:, :].tensor, offset=0, ap=[[128, G], [1, 128]]
                ),
                in_=yT[:],
            )
    nc_b.finalize()
    return nc_b


def make_block_diag(a):
    """a: [H, C] -> [H*C, H] block diagonal."""
    out = np.zeros((HC, HEADS), np.float32)
    for h in range(HEADS):
        out[h * CH : (h + 1) * CH, h] = a[h]
    return out


def _aug(W, a_src, a_dst):
    """[W | W@As_bd | W@Ad_bd | pad] as fp16, width PAY."""
    W = np.asarray(W, np.float32)
    aug = np.zeros((W.shape[0], PAY), np.float32)
    aug[:, 0:HC] = W
    aug[:, HC : HC + HEADS] = W @ make_block_diag(np.asarray(a_src, np.float32))
    aug[:, HC + HEADS : HC + 2 * HEADS] = W @ make_block_diag(
        np.asarray(a_dst, np.float32)
    )
    return aug.astype(np.float16)


def build_inputs(cfg: Cfg, layout, x, W1, a_src1, a_dst1, b1, W2, a_src2,
                 a_dst2, b2, fc_w, fc_b):
    NP = cfg.np_
    node_of_sp = layout["node_of_sp"]
    xs = np.zeros((NP, DIN), np.float32)
    valid = node_of_sp >= 0
    xs[valid] = np.asarray(x, np.float32)[node_of_sp[valid]]
    xT = np.ascontiguousarray(xs.T).astype(np.float16)

    dumrow = np.zeros((3, EWP), np.float16)
    dumrow[0:2, AS_OFF : AS_OFF + HEADS] = DUM_AS

    base = dict(
        xT=xT,
        W1AUG=_aug(W1, a_src1, a_dst1),
        W2AUG=_aug(W2, a_src2, a_dst2),
        bmt=np.ones((1, 128), np.float16),
        br1=np.concatenate(
            [np.asarray(b1, np.float16).reshape(1, HC),
             np.zeros((1, PAY - HC), np.float16)], axis=1),
        br2=np.concatenate(
            [np.asarray(b2, np.float16).reshape(1, HC),
             np.zeros((1, PAY - HC), np.float16)], axis=1),
        fcwt=np.broadcast_to(
            np.asarray(fc_w, np.float32).reshape(1, HC), (128, HC)
        ).copy(),
        # layer 2 produces elu+1; the -1 contribution is folded here:
        # z = sum((elu+1) * fcw) + (fc_b - sum(fcw))
        fcbt=np.full(
            (128, 1),
            np.float32(
                np.asarray(fc_b).reshape(-1)[0]
                - np.asarray(fc_w, np.float32).sum()
            ),
        ),
        identh=np.eye(128, dtype=np.float16),
        dumrow=dumrow,
    )
    in_maps = []
    for c in range(cfg.nc):
        m = dict(base)
        m["idx"] = layout["idx"][c]
        in_maps.append(m)
    return in_maps


def assemble_output(cfg: Cfg, layout, results):
    node_of_sp = layout["node_of_sp"]
    yfull = np.zeros((cfg.n_real, 1), np.float32)
    for c in range(cfg.nc):
        yc = results[c]["y"].reshape(-1)       # [PC] local order (g*128 + p)
        loc = np.arange(cfg.pc)
        sp = (loc // 128) * cfg.stratum + c * 128 + (loc % 128)
        nodes = node_of_sp[sp]
        ok = nodes >= 0
        yfull[nodes[ok], 0] = yc[ok]
    return yfull


def _absorb_device_wedge():
    """Run a trivial 8-core kernel; a crashed prior session can leave the
    NeuronCores in NRT_EXEC_UNIT_UNRECOVERABLE state for the next session,
    which a fresh trivial execution clears."""
    try:
        from concourse.bass_utils import run_bass_kernel_spmd

        nc_t = bacc.Bacc(None, num_devices=8)
        a = nc_t.dram_tensor("a", [128, 128], F32, kind="ExternalInput")
        o = nc_t.dram_tensor("o", [128, 128], F32, kind="ExternalOutput")
        with tile.TileContext(nc_t) as tc:
            with tc.tile_pool(name="sb", bufs=1) as sb:
                t = sb.tile([128, 128], F32)
                nc_t.sync.dma_start(out=t[:], in_=a[:, :])
                nc_t.sync.dma_start(out=o[:, :], in_=t[:])
        nc_t.finalize()
        run_bass_kernel_spmd(
            nc_t, [{"a": np.zeros((128, 128), np.float32)}] * 8,
            core_ids=list(range(8)),
        )
    except Exception:
        pass


def kernel(**inputs):
    from concourse.bass_utils import run_bass_kernel_spmd

    cfg = Cfg()
    layout = build_layout(inputs["edge_index"], cfg)
    in_maps = build_inputs(
        cfg, layout,
        inputs["x"], inputs["W1"], inputs["a_src1"], inputs["a_dst1"],
        inputs["b1"], inputs["W2"], inputs["a_src2"], inputs["a_dst2"],
        inputs["b2"], inputs["fc_w"], inputs["fc_b"],
    )
    nc_b = build_program(cfg, shared_out=True)
    last_err = None
    for attempt in range(3):
        try:
            res = run_bass_kernel_spmd(
                nc_b, in_maps, core_ids=list(range(cfg.nc))
            )
            return assemble_output(cfg, layout, res.results)
        except Exception as e:  # wedged device from a prior crashed session
            last_err = e
            _absorb_device_wedge()
    raise last_err


if __name__ == "__main__":
    pass


# revision 48
# speedup vs baseline: 1.0114x; 1.0036x over previous
"""Two-layer GAT (PyG GATConv-style) on 8 Trainium2 NeuronCores via Bass/Tile.

Strategy (dst-sharded, data-parallel, fp16 data path):
  - Nodes are assigned to (core, group) "strata": all 8 cores' group g hold
    nodes with similar in-degree statistics (snake-ordered by (cA, cB)), so a
    single SPMD program with uniform per-group slot counts serves all cores.
  - Each core redundantly computes the dense part h = x @ W (plus fused
    alpha_src/alpha_dst columns) for ALL nodes and writes a DRAM row table
    (fp16 row, 768B pitch = [h(256) | a_src(4) | a_dst(4) | pad]); the edge
    phase (gather h[src], per-dst softmax over incoming edges, weighted sum)
    runs only on the core's own 1/8 of destination nodes via dma_gather with
    host-precomputed int16 index lists (slot-major, per-dst-row layout).
  - int16 gather indices only reach 32767, so the table is split into an A
    range and a B range (split aligned to a stratum boundary); every dst row's
    slots are [A-slots | B-slots] with per-group uniform counts KA/KB. Slot 0
    of a node's own side is its self-loop, which supplies alpha_dst.
  - Edge math: e = leaky(as+ad) (no max-shift; dummy slots use as=-60000 so
    exp underflows to 0), p = exp(e), den = sum_k p, alpha = p/den fused into
    an Act-engine per-head expansion to [k, c] fp16, weighted products via a
    DVE 2x-mode tensor_tensor, then a binary-tree fp16 reduction over slots.
  - Between layers, each core's elu(out1)^T block is AllGathered (fp16) so
    layer 2's replicated dense phase can read every node's features.
  - Final sigmoid(fc) output is computed per-core and unpermuted on the host.
"""

import sys

sys.path.insert(0, "/opt/trn_rl_repo")

from dataclasses import dataclass, field

import numpy as np

import concourse.bass as bass
import concourse.bacc as bacc
import concourse.tile as tile
from concourse import mybir
from concourse.library_config import mlp as MLP_LIB

F32 = mybir.dt.float32
F16 = mybir.dt.float16
I16 = mybir.dt.int16
AX = mybir.AxisListType
OP = mybir.AluOpType
ACT = mybir.ActivationFunctionType

HEADS = 4
CH = 64
HC = HEADS * CH          # 256
DIN = 128
NEG_SLOPE = 0.2
EWP = 384                # table row pitch in fp16 elems (768 B, %256==0)
PAY = 272                # payload elems per row: h(256) | as(4) | ad(4) | pad8
AS_OFF = 256             # alpha_src column offset within a row
AD_OFF = 260             # alpha_dst column offset
DUM_AS = -60000.0        # dummy-slot alpha_src (finite; exp underflows to 0)


@dataclass
class Cfg:
    n_real: int = 50000
    nc: int = 8                  # cores
    g: int = 49                  # groups (strata) per core
    split_strat: int = 31        # strata in table range A
    ka: list = field(default_factory=list)   # per-group A slots
    kb: list = field(default_factory=list)   # per-group B slots

    @property
    def stratum(self):
        return self.nc * 128

    @property
    def np_(self):
        return self.g * self.stratum

    @property
    def pc(self):
        return self.g * 128

    @property
    def split_sp(self):
        return self.split_strat * self.stratum

    @property
    def nb(self):
        return self.np_ - self.split_sp

    @property
    def rows(self):
        return self.np_ + 3      # dumA + dumB + padself

    @property
    def row_b0(self):
        return self.split_sp + 1   # dummyB row

    @property
    def padself_idx(self):
        return self.nb + 1         # local B index of the pad-self row


def build_layout(edge_index, cfg: Cfg):
    """Host-side graph layout. Returns per-core index arrays + permutation."""
    n = cfg.n_real
    src = np.asarray(edge_index[0], dtype=np.int64)
    dst = np.asarray(edge_index[1], dtype=np.int64)
    src = np.concatenate([src, np.arange(n, dtype=np.int64)])
    dst = np.concatenate([dst, np.arange(n, dtype=np.int64)])
    deg = np.bincount(dst, minlength=n)

    split = cfg.split_sp
    order1 = np.argsort(-deg, kind="stable")
    a_nodes = order1[:split]
    b_nodes = order1[split:]
    is_a = np.zeros(n, bool)
    is_a[a_nodes] = True
    ca = np.bincount(dst[is_a[src]], minlength=n)
    cb = deg - ca

    def snake(nodes):
        out = []
        flip = False
        avals = ca[nodes]
        for v in np.sort(np.unique(avals))[::-1]:
            grp = nodes[avals == v]
            grp = grp[np.argsort(cb[grp], kind="stable")]
            if flip:
                grp = grp[::-1]
            flip = not flip
            out.append(grp)
        return np.concatenate(out) if out else nodes

    seq = np.concatenate([snake(a_nodes), snake(b_nodes)])
    # node_of_sp: storage position -> node (-1 = pad). pads go at the end.
    node_of_sp = np.full(cfg.np_, -1, np.int64)
    node_of_sp[: len(seq)] = seq
    sp_of = np.full(n, -1, np.int64)
    sp_of[seq] = np.arange(len(seq))

    # per-group slot counts
    ka = np.zeros(cfg.g, np.int64)
    kb = np.zeros(cfg.g, np.int64)
    for s in range(cfg.g):
        nodes = node_of_sp[s * cfg.stratum : (s + 1) * cfg.stratum]
        nodes = nodes[nodes >= 0]
        if len(nodes):
            ka[s] = ca[nodes].max()
            kb[s] = cb[nodes].max()
    ka = np.maximum(ka, 1)
    kb = np.maximum(kb, 1)
    # pads live in the last stratum; their pad-self slot is B slot 0
    cfg.ka = ka.tolist()
    cfg.kb = kb.tolist()

    # group edges by dst
    eorder = np.argsort(dst, kind="stable")
    s_sorted = src[eorder]
    starts = np.searchsorted(dst, np.arange(n + 1), sorter=eorder)

    sp_src = sp_of[s_sorted]           # storage pos of each edge's src
    src_is_a = sp_src < split
    idx_a_val = (sp_src + 1).astype(np.int64)            # A-range row index
    idx_b_val = (sp_src - split + 1).astype(np.int64)    # B-range local row

    def pack(flat):
        # idx element i lives at [i % 16, i // 16]; the two Q7 DGE cores read
        # partitions 0-15 and 16-31 respectively, so replicate into both.
        ln = len(flat)
        f = np.zeros((128, ln // 16), np.int16)
        ii = np.arange(ln)
        f[ii % 16, ii // 16] = flat
        f[16 + (ii % 16), ii // 16] = flat
        return f

    idx_cores = []
    for c in range(cfg.nc):
        flats = []
        for s in range(cfg.g):
            KA, KB = int(ka[s]), int(kb[s])
            ma = np.zeros((KA, 128), np.int64)   # [slot, partition]
            mb = np.zeros((KB, 128), np.int64)
            base_sp = s * cfg.stratum + c * 128
            for p in range(128):
                node = node_of_sp[base_sp + p]
                if node < 0:
                    # pad position: pad-self row keeps den positive
                    mb[0, p] = cfg.padself_idx
                    continue
                lo, hi = starts[node], starts[node + 1]
                aa = idx_a_val[lo:hi][src_is_a[lo:hi]]
                bb = idx_b_val[lo:hi][~src_is_a[lo:hi]]
                # self-loop first in its own side (supplies alpha_dst)
                own_sp = base_sp + p
                if own_sp < split:
                    self_idx = own_sp + 1
                    w = np.where(aa == self_idx)[0]
                    if len(w) == 0:
                        raise RuntimeError("self-loop missing in A list")
                    aa[[0, w[0]]] = aa[[w[0], 0]]
                else:
                    self_idx = own_sp - split + 1
                    w = np.where(bb == self_idx)[0]
                    if len(w) == 0:
                        raise RuntimeError("self-loop missing in B list")
                    bb[[0, w[0]]] = bb[[w[0], 0]]
                ma[: len(aa), p] = aa
                mb[: len(bb), p] = bb
            flats.append(
                np.concatenate([pack(ma.reshape(-1)), pack(mb.reshape(-1))],
                               axis=1)
            )
        idx_cores.append(np.concatenate(flats, axis=1))

    return dict(node_of_sp=node_of_sp, sp_of=sp_of, idx=idx_cores)


def build_program(cfg: Cfg, shared_out: bool = True):
    nc_b = bacc.Bacc(None, num_devices=cfg.nc)
    NP, G, NC = cfg.np_, cfg.g, cfg.nc
    NT = NP // 128                 # global tiles (stratum-major)
    SPLIT_T = cfg.split_sp // 128  # first B tile
    R = cfg.rows
    ROW_B0 = cfg.row_b0
    PC = cfg.pc
    sumK8 = 8 * int(np.sum(cfg.ka) + np.sum(cfg.kb))

    xT = nc_b.dram_tensor("xT", [DIN, NP], F16, kind="ExternalInput")
    W1AUG = nc_b.dram_tensor("W1AUG", [DIN, PAY], F16, kind="ExternalInput")
    W2AUG = nc_b.dram_tensor("W2AUG", [HC, PAY], F16, kind="ExternalInput")
    bmt = nc_b.dram_tensor("bmt", [1, 128], F16, kind="ExternalInput")
    br1 = nc_b.dram_tensor("br1", [1, PAY], F16, kind="ExternalInput")
    br2 = nc_b.dram_tensor("br2", [1, PAY], F16, kind="ExternalInput")
    fcwt = nc_b.dram_tensor("fcwt", [128, HC], F32, kind="ExternalInput")
    fcbt = nc_b.dram_tensor("fcbt", [128, 1], F32, kind="ExternalInput")
    identh = nc_b.dram_tensor("identh", [128, 128], F16, kind="ExternalInput")
    dumrow = nc_b.dram_tensor("dumrow", [3, EWP], F16, kind="ExternalInput")
    idx = nc_b.dram_tensor("idx", [128, sumK8], I16, kind="ExternalInput")
    y = nc_b.dram_tensor("y", [PC, 1], F32, kind="ExternalOutput")

    table1 = nc_b.dram_tensor("table1", [R, EWP], F16)
    table2 = nc_b.dram_tensor("table2", [R, EWP], F16)
    elu1T = nc_b.dram_tensor("elu1T", [HC, NP // NC], F16)
    gath = nc_b.dram_tensor(
        "gath", [NC * HC, NP // NC], F16,
        addr_space="Shared" if shared_out else "Local",
    )

    with tile.TileContext(nc_b) as tc:
        ctxmgrs = [
            tc.tile_pool(name="cst", bufs=1),
            tc.tile_pool(name="lw", bufs=2),
            tc.tile_pool(name="hp", bufs=2),
            tc.tile_pool(name="ge", bufs=3),
            tc.tile_pool(name="px", bufs=2),
            tc.tile_pool(name="th", bufs=1),
            tc.tile_pool(name="sm", bufs=2),
            tc.tile_pool(name="ix", bufs=2),
            tc.tile_pool(name="ph", bufs=1, space="PSUM"),
            tc.tile_pool(name="pt", bufs=2, space="PSUM"),
        ]
        import contextlib

        with contextlib.ExitStack() as st:
            cst, lw, hp, ge, px, th, sm, ix, ph, pt = [
                st.enter_context(m) for m in ctxmgrs
            ]
            nc_b.gpsimd.load_library(MLP_LIB)

            # ---- constants ----
            idt = cst.tile([128, 128], F16)
            nc_b.sync.dma_start(out=idt[:], in_=identh[:, :])
            bms = cst.tile([1, 128], F16)
            nc_b.sync.dma_start(out=bms[:], in_=bmt[:, :])
            br1s = cst.tile([1, PAY], F16)
            nc_b.sync.dma_start(out=br1s[:], in_=br1[:, :])
            br2s = cst.tile([1, PAY], F16)
            nc_b.sync.dma_start(out=br2s[:], in_=br2[:, :])
            fcws = cst.tile([128, HC], F32)
            nc_b.sync.dma_start(out=fcws[:], in_=fcwt[:, :])
            fcbs = cst.tile([128, 1], F32)
            nc_b.sync.dma_start(out=fcbs[:], in_=fcbt[:, :])
            w1s = cst.tile([128, PAY], F16)
            nc_b.sync.dma_start(out=w1s[:], in_=W1AUG[:, :])
            w2s = cst.tile([128, 2, PAY], F16)
            nc_b.sync.dma_start(out=w2s[:, 0, :], in_=W2AUG[0:128, :])
            nc_b.sync.dma_start(out=w2s[:, 1, :], in_=W2AUG[128:HC, :])
            zall = cst.tile([128, G], F32)

            # dummy + pad-self rows for both tables (DRAM -> DRAM)
            for tab in (table1, table2):
                nc_b.sync.dma_start(out=tab[0:1, :], in_=dumrow[0:1, :])
                nc_b.sync.dma_start(
                    out=tab[ROW_B0 : ROW_B0 + 1, :], in_=dumrow[1:2, :]
                )
                nc_b.sync.dma_start(out=tab[R - 1 : R, :], in_=dumrow[2:3, :])

            # ---- dense phase (replicated): h|alphas -> table ----
            def h_phase(layer):
                tab = table1 if layer == 1 else table2
                cp_rr = [0]

                def copy_psum(dst_ap, src_ap):
                    e = cp_rr[0] % 2
                    cp_rr[0] += 1
                    if e == 0:
                        nc_b.scalar.copy(out=dst_ap, in_=src_ap)
                    else:
                        nc_b.vector.tensor_copy(out=dst_ap, in_=src_ap)

                def write_rows(ht_slice, row0, nt):
                    # ht_slice[:, j, :] (j in 0..nt) -> rows row0 + j*128 + p
                    nc_b.sync.dma_start(
                        out=bass.AP(
                            tensor=tab[:, :].tensor,
                            offset=row0 * EWP,
                            ap=[[EWP, 128], [128 * EWP, nt], [1, PAY]],
                        ),
                        in_=ht_slice,
                    )

                if layer == 1:
                    TB = 12
                    t0 = 0
                    while t0 < NT:
                        nt = min(TB, NT - t0)
                        lh = lw.tile([128, TB * 128], F16, tag="gh")
                        nc_b.sync.dma_start(
                            out=lh[:, 0 : nt * 128],
                            in_=xT[:, t0 * 128 : (t0 + nt) * 128],
                        )
                        ht = hp.tile([128, TB, PAY], F16, tag="ht")
                        for j in range(nt):
                            pj = ph.tile([128, PAY], F32, space="PSUM",
                                         tag=f"pj{j % 5}")
                            nc_b.tensor.matmul(
                                out=pj[:], lhsT=lh[:, j * 128 : (j + 1) * 128],
                                rhs=w1s[:], start=True, stop=False,
                            )
                            nc_b.tensor.matmul(
                                out=pj[:], lhsT=bms[:], rhs=br1s[:],
                                start=False, stop=True,
                            )
                            copy_psum(ht[:, j, :], pj[:])
                        # contiguous row runs (split at the A/B boundary)
                        runs = []
                        for j in range(nt):
                            t = t0 + j
                            sh = 1 if t < SPLIT_T else 2
                            if runs and runs[-1][2] == sh:
                                runs[-1][1] += 1
                            else:
                                runs.append([j, 1, sh])
                        for j0, nj, sh in runs:
                            write_rows(ht[:, j0 : j0 + nj, :],
                                       (t0 + j0) * 128 + sh, nj)
                        t0 += nt
                else:
                    TB = 11
                    for c in range(NC):
                        s0 = 0
                        while s0 < G:
                            ns = min(TB, G - s0)
                            gh = lw.tile([128, 2, TB * 128], F16, tag="gh")
                            nc_b.sync.dma_start(
                                out=gh[:, :, 0 : ns * 128],
                                in_=bass.AP(
                                    tensor=gath[:, :].tensor,
                                    offset=(c * HC) * (NP // NC) + s0 * 128,
                                    ap=[[NP // NC, 128],
                                        [128 * (NP // NC), 2],
                                        [1, ns * 128]],
                                ),
                            )
                            ht = hp.tile([128, TB, PAY], F16, tag="ht")
                            for j in range(ns):
                                pj = ph.tile([128, PAY], F32, space="PSUM",
                                             tag=f"pj{j % 5}")
                                nc_b.tensor.matmul(
                                    out=pj[:],
                                    lhsT=gh[:, 0, j * 128 : (j + 1) * 128],
                                    rhs=w2s[:, 0, :], start=True, stop=False,
                                )
                                nc_b.tensor.matmul(
                                    out=pj[:],
                                    lhsT=gh[:, 1, j * 128 : (j + 1) * 128],
                                    rhs=w2s[:, 1, :], start=False, stop=False,
                                )
                                nc_b.tensor.matmul(
                                    out=pj[:], lhsT=bms[:], rhs=br2s[:],
                                    start=False, stop=True,
                                )
                                copy_psum(ht[:, j, :], pj[:])
                            runs = []
                            for j in range(ns):
                                sv = s0 + j
                                sh = 1 if sv < cfg.split_strat else 2
                                if runs and runs[-1][2] == sh:
                                    runs[-1][1] += 1
                                else:
                                    runs.append([j, 1, sh])
                            for j0, nj, sh in runs:
                                sv = s0 + j0
                                nc_b.sync.dma_start(
                                    out=bass.AP(
                                        tensor=tab[:, :].tensor,
                                        offset=(sv * cfg.stratum + c * 128 + sh)
                                        * EWP,
                                        ap=[[EWP, 128],
                                            [cfg.stratum * EWP, nj],
                                            [1, PAY]],
                                    ),
                                    in_=ht[:, j0 : j0 + nj, :],
                                )
                            s0 += ns

            # ---- edge phase ----
            # Software-pipelined: gathers run 2 groups ahead, softmax prep
            # (e/leaky/exp/den/recip/alpha-expansion) 1 group ahead of the
            # heavy weighted-sum work, so no engine head-of-line blocks.
            GCH = 8

            def edge_phase(layer):
                tab = table1 if layer == 1 else table2
                offs = []
                off = 0
                for g in range(G):
                    offs.append(off)
                    off += 8 * (cfg.ka[g] + cfg.kb[g])
                state = {}

                def stage_gather(g):
                    KA, KB = cfg.ka[g], cfg.kb[g]
                    K = KA + KB
                    ixg = ix.tile([128, 8 * K], I16, tag="ixg")
                    nc_b.sync.dma_start(
                        out=ixg[:], in_=idx[:, offs[g] : offs[g] + 8 * K]
                    )
                    gt = ge.tile([128, K * EWP], F16, tag="gt")
                    gta = gt[:]
                    for base, kn, ioff, in_ap in (
                        (0, KA, 0, tab[0:ROW_B0, :]),
                        (KA, KB, 8 * KA, tab[ROW_B0:R, :]),
                    ):
                        for c0 in range(0, kn, GCH):
                            cw = min(GCH, kn - c0)
                            nc_b.gpsimd.dma_gather(
                                out_ap=bass.AP(
                                    tensor=gta.tensor,
                                    offset=gta.offset + (base + c0) * EWP,
                                    ap=[gta.ap[0], [EWP, cw], [1, EWP]],
                                ),
                                in_ap=in_ap,
                                idxs_ap=ixg[:, ioff + 8 * c0 : ioff + 8 * (c0 + cw)],
                                num_idxs=128 * cw,
                                num_idxs_reg=128 * cw,
                                elem_size=EWP,
                            )
                    state[g] = dict(gt=gt)

                def stage_prep(g):
                    KA, KB = cfg.ka[g], cfg.kb[g]
                    K = KA + KB
                    gta = state[g]["gt"][:]
                    # alpha_dst from own self-loop slot (slot 0 of own side)
                    ad_off = AD_OFF if g < cfg.split_strat else KA * EWP + AD_OFF
                    # e[p, h*K+k] = as(slot k, h) + ad(h)   [Act x4,
                    # ad supplied as a per-partition bias column]
                    e = sm.tile([128, HEADS * K], F32, tag="e")
                    for h in range(HEADS):
                        nc_b.scalar.activation(
                            out=e[:, h * K : (h + 1) * K],
                            in_=bass.AP(
                                tensor=gta.tensor,
                                offset=gta.offset + AS_OFF + h,
                                ap=[gta.ap[0], [EWP, K]],
                            ),
                            func=ACT.Identity,
                            bias=bass.AP(
                                tensor=gta.tensor,
                                offset=gta.offset + ad_off + h,
                                ap=[gta.ap[0], [1, 1]],
                            ),
                        )
                    # leaky relu [DVE]
                    el = sm.tile([128, HEADS * K], F32, tag="el")
                    nc_b.vector.scalar_tensor_tensor(
                        out=el[:], in0=e[:], scalar=NEG_SLOPE, in1=e[:],
                        op0=OP.mult, op1=OP.max,
                    )
                    # p = exp(el) [Act]; den = sum_k p [DVE]; rden [DVE]
                    p = sm.tile([128, HEADS * K], F32, tag="p")
                    nc_b.scalar.activation(out=p[:], in_=el[:], func=ACT.Exp)
                    den = sm.tile([128, HEADS], F32, tag="den")
                    nc_b.vector.tensor_reduce(
                        out=den[:],
                        in_=bass.AP(
                            tensor=p[:].tensor, offset=p[:].offset,
                            ap=[p[:].ap[0], [K, HEADS], [1, K]],
                        ),
                        axis=AX.X, op=OP.add,
                    )
                    rden = sm.tile([128, HEADS], F32, tag="rden")
                    nc_b.vector.reciprocal(out=rden[:], in_=den[:])
                    # pexp[p, h, k, 0:2] = alpha = p * rden_h  (fp16) [Act]
                    # Only 2 copies per alpha: the multiply broadcasts over
                    # the middle (c_hi) dim; DVE 2x only requires the LAST
                    # dim packed.
                    pex = px.tile([128, HEADS, K, 2], F16, tag="pex")
                    for h in range(HEADS):
                        nc_b.scalar.activation(
                            out=bass.AP(
                                tensor=pex[:].tensor,
                                offset=pex[:].offset + h * K * 2,
                                ap=[pex[:].ap[0], [2, K], [1, 2]],
                            ),
                            in_=bass.AP(
                                tensor=p[:].tensor, offset=p[:].offset + h * K,
                                ap=[p[:].ap[0], [1, K], [0, 2]],
                            ),
                            func=ACT.Copy,
                            scale=rden[:, h : h + 1],
                        )
                    state[g]["pex"] = pex

                def stage_mult(g):
                    KA, KB = cfg.ka[g], cfg.kb[g]
                    K = KA + KB
                    gta = state[g]["gt"][:]
                    pex = state[g]["pex"]
                    # tht[p, h, k, c] = alpha * h_src  (fp16, DVE 2x);
                    # split per head so each starts as soon as that head's
                    # alpha expansion lands
                    tht = th.tile([128, HEADS * K * CH], F16, tag="tht")
                    for h in range(HEADS):
                        nc_b.vector.tensor_tensor(
                            out=bass.AP(
                                tensor=tht[:].tensor,
                                offset=tht[:].offset + h * K * CH,
                                ap=[tht[:].ap[0], [CH, K], [2, CH // 2],
                                    [1, 2]],
                            ),
                            in0=bass.AP(
                                tensor=gta.tensor, offset=gta.offset + h * CH,
                                ap=[gta.ap[0], [EWP, K], [2, CH // 2], [1, 2]],
                            ),
                            in1=bass.AP(
                                tensor=pex[:].tensor,
                                offset=pex[:].offset + h * K * 2,
                                ap=[pex[:].ap[0], [2, K], [0, CH // 2],
                                    [1, 2]],
                            ),
                            op=OP.mult,
                        )
                    state[g]["tht"] = tht

                def stage_heavy(g):
                    KA, KB = cfg.ka[g], cfg.kb[g]
                    K = KA + KB
                    pex = state[g]["pex"]
                    tht = state[g]["tht"]
                    # binary-tree reduce over k (fp16 DVE 2x adds); ping-pong
                    # tht <-> thB
                    thB = th.tile([128, HEADS * ((K + 1) // 2) * CH], F16,
                                  tag="thB")
                    opre = sm.tile([128, HC], F16, tag="opre")
                    cur, alt = (tht, K * CH), (thB, ((K + 1) // 2) * CH)
                    n = K
                    while n > 1:
                        (cur_t, HS), (alt_t, HSa) = cur, alt
                        ca_, aa_ = cur_t[:], alt_t[:]
                        if n == 2:
                            nc_b.vector.tensor_tensor(
                                out=bass.AP(
                                    tensor=opre[:].tensor, offset=opre[:].offset,
                                    ap=[opre[:].ap[0], [CH, HEADS], [1, CH]],
                                ),
                                in0=bass.AP(
                                    tensor=ca_.tensor, offset=ca_.offset,
                                    ap=[ca_.ap[0], [HS, HEADS], [1, CH]],
                                ),
                                in1=bass.AP(
                                    tensor=ca_.tensor, offset=ca_.offset + CH,
                                    ap=[ca_.ap[0], [HS, HEADS], [1, CH]],
                                ),
                                op=OP.add,
                            )
                            n = 1
                            continue
                        if n % 2 == 1:
                            # fold straggler (slot n-1) into slot 0 in place
                            nc_b.vector.tensor_tensor(
                                out=bass.AP(
                                    tensor=ca_.tensor, offset=ca_.offset,
                                    ap=[ca_.ap[0], [HS, HEADS], [1, CH]],
                                ),
                                in0=bass.AP(
                                    tensor=ca_.tensor, offset=ca_.offset,
                                    ap=[ca_.ap[0], [HS, HEADS], [1, CH]],
                                ),
                                in1=bass.AP(
                                    tensor=ca_.tensor,
                                    offset=ca_.offset + (n - 1) * CH,
                                    ap=[ca_.ap[0], [HS, HEADS], [1, CH]],
                                ),
                                op=OP.add,
                            )
                            n -= 1
                            continue
                        half = n // 2
                        nc_b.vector.tensor_tensor(
                            out=bass.AP(
                                tensor=aa_.tensor, offset=aa_.offset,
                                ap=[aa_.ap[0], [HSa, HEADS], [CH, half],
                                    [1, CH]],
                            ),
                            in0=bass.AP(
                                tensor=ca_.tensor, offset=ca_.offset,
                                ap=[ca_.ap[0], [HS, HEADS], [2 * CH, half],
                                    [1, CH]],
                            ),
                            in1=bass.AP(
                                tensor=ca_.tensor, offset=ca_.offset + CH,
                                ap=[ca_.ap[0], [HS, HEADS], [2 * CH, half],
                                    [1, CH]],
                            ),
                            op=OP.add,
                        )
                        n = half
                        cur, alt = alt, cur

                    # bias already folded into the table rows (rank-1
                    # matmul in the dense phase; softmax weights sum to 1)
                    outb = opre
                    # elu = relu(x) + exp(min(x,0)) - 1.  Layer 1 computes
                    # it exactly; layer 2 computes elu+1 = min(exp(x),1) +
                    # relu(x) (exp overflows for x>~88? no: clamp via min
                    # AFTER exp is exact since exp(x)>=1 iff x>=0) and the -1
                    # is folded into the host-adjusted fc bias.
                    rl = sm.tile([128, HC], F16, tag="rl")
                    nc_b.scalar.activation(out=rl[:], in_=outb[:], func=ACT.Relu)
                    em = sm.tile([128, HC], F16, tag="em")
                    elu = sm.tile([128, HC], F16, tag="elu")
                    if layer == 1:
                        xm = sm.tile([128, HC], F16, tag="xm")
                        nc_b.scalar.activation(out=xm[:], in_=outb[:],
                                               func=ACT.Relu, scale=-1.0)
                        nc_b.scalar.activation(out=em[:], in_=xm[:],
                                               func=ACT.Exp, scale=-1.0)
                        em1 = sm.tile([128, HC], F16, tag="em1")
                        nc_b.scalar.activation(out=em1[:], in_=em[:],
                                               func=ACT.Copy, bias=-1.0)
                        nc_b.vector.tensor_tensor(
                            out=elu[:], in0=em1[:], in1=rl[:], op=OP.add,
                        )
                    else:
                        nc_b.scalar.activation(out=em[:], in_=outb[:],
                                               func=ACT.Exp)
                        nc_b.vector.scalar_tensor_tensor(
                            out=elu[:], in0=em[:], scalar=1.0, in1=rl[:],
                            op0=OP.min, op1=OP.add,
                        )
                    if layer == 1:
                        et = sm.tile([128, 2, 128], F16, tag="et")
                        for half_i in range(2):
                            ptr = pt.tile([128, 128], F16, space="PSUM",
                                          tag="ptr")
                            nc_b.tensor.transpose(
                                out=ptr[:],
                                in_=elu[:, half_i * 128 : (half_i + 1) * 128],
                                identity=idt[:],
                            )
                            nc_b.scalar.copy(out=et[:, half_i, :],
                                             in_=ptr[:])
                        nc_b.sync.dma_start(
                            out=bass.AP(
                                tensor=elu1T[:, :].tensor,
                                offset=g * 128,
                                ap=[[NP // NC, 128], [128 * (NP // NC), 2],
                                    [1, 128]],
                            ),
                            in_=et[:],
                        )
                    else:
                        fsc = sm.tile([128, HC], F32, tag="xm")
                        nc_b.vector.scalar_tensor_tensor(
                            out=fsc[:], in0=elu[:], scalar=1.0, in1=fcws[:],
                            op0=OP.bypass, op1=OP.mult,
                            accum_out=zall[:, g : g + 1],
                        )
                    del state[g]

                stage_gather(0)
                if G > 1:
                    stage_gather(1)
                stage_prep(0)
                for g in range(G):
                    if g + 1 < G:
                        stage_prep(g + 1)
                    if g + 2 < G:
                        stage_gather(g + 2)
                    stage_mult(g)
                    stage_heavy(g)

            import os
            phases = os.environ.get("KM_PHASES", "h1,e1,cc,h2,e2").split(",")
            marks = {}

            def mark(label):
                marks[label] = len(nc_b.inst_map)

            nc_b._phase_marks = marks
            mark("setup_end")
            if "h1" in phases:
                h_phase(1)
            mark("h1_end")
            if "e1" in phases:
                edge_phase(1)
            mark("e1_end")
            if "cc" in phases:
                nc_b.gpsimd.collective_compute(
                    "AllGather",
                    OP.bypass,
                    replica_groups=[list(range(NC))],
                    ins=[elu1T[:, :].opt()],
                    outs=[gath[:, :].opt()],
                )
            mark("cc_end")
            if "h2" in phases:
                h_phase(2)
            mark("h2_end")
            if "e2" in phases:
                edge_phase(2)
            mark("e2_end")

            # final: y = sigmoid(z + fc_b), transposed out
            if "e2" not in phases:
                nc_b.vector.memset(zall[:], 0.0)
            zsig = cst.tile([128, G], F16)
            nc_b.scalar.activation(
                out=zsig[:], in_=zall[:], func=ACT.Sigmoid,
                bias=fcbs[:, 0:1], scale=1.0,
            )
            pz = ph.tile([G, 128], F16, space="PSUM", tag="pz")
            nc_b.tensor.transpose(out=pz[:], in_=zsig[:], identity=idt[:])
            yT = cst.tile([G, 128], F32)  # copy converts f16 psum -> f32
            nc_b.vector.tensor_copy(out=yT[:], in_=pz[:])
            nc_b.sync.dma_start(
                out=bass.AP(
                    tensor=y[:, :].tensor, offset=0, ap=[[128, G], [1, 128]]
                ),
                in_=yT[:],
            )
    nc_b.finalize()
    return nc_b


def make_block_diag(a):
    """a: [H, C] -> [H*C, H] block diagonal."""
    out = np.zeros((HC, HEADS), np.float32)
    for h in range(HEADS):
        out[h * CH : (h + 1) * CH, h] = a[h]
    return out


def _aug(W, a_src, a_dst):
    """[W | W@As_bd | W@Ad_bd | pad] as fp16, width PAY."""
    W = np.asarray(W, np.float32)
    aug = np.zeros((W.shape[0], PAY), np.float32)
    aug[:, 0:HC] = W
    aug[:, HC : HC + HEADS] = W @ make_block_diag(np.asarray(a_src, np.float32))
    aug[:, HC + HEADS : HC + 2 * HEADS] = W @ make_block_diag(
        np.asarray(a_dst, np.float32)
    )
    return aug.astype(np.float16)


def build_inputs(cfg: Cfg, layout, x, W1, a_src1, a_dst1, b1, W2, a_src2,
                 a_dst2, b2, fc_w, fc_b):
    NP = cfg.np_
    node_of_sp = layout["node_of_sp"]
    xs = np.zeros((NP, DIN), np.float32)
    valid = node_of_sp >= 0
    xs[valid] = np.asarray(x, np.float32)[node_of_sp[valid]]
    xT = np.ascontiguousarray(xs.T).astype(np.float16)

    dumrow = np.zeros((3, EWP), np.float16)
    dumrow[0:2, AS_OFF : AS_OFF + HEADS] = DUM_AS

    base = dict(
        xT=xT,
        W1AUG=_aug(W1, a_src1, a_dst1),
        W2AUG=_aug(W2, a_src2, a_dst2),
        bmt=np.ones((1, 128), np.float16),
        br1=np.concatenate(
            [np.asarray(b1, np.float16).reshape(1, HC),
             np.zeros((1, PAY - HC), np.float16)], axis=1),
        br2=np.concatenate(
            [np.asarray(b2, np.float16).reshape(1, HC),
             np.zeros((1, PAY - HC), np.float16)], axis=1),
        fcwt=np.broadcast_to(
            np.asarray(fc_w, np.float32).reshape(1, HC), (128, HC)
        ).copy(),
        # layer 2 produces elu+1; the -1 contribution is folded here:
        # z = sum((elu+1) * fcw) + (fc_b - sum(fcw))
        fcbt=np.full(
            (128, 1),
            np.float32(
                np.asarray(fc_b).reshape(-1)[0]
                - np.asarray(fc_w, np.float32).sum()
            ),
        ),
        identh=np.eye(128, dtype=np.float16),
        dumrow=dumrow,
    )
    in_maps = []
    for c in range(cfg.nc):
        m = dict(base)
        m["idx"] = layout["idx"][c]
        in_maps.append(m)
    return in_maps


def assemble_output(cfg: Cfg, layout, results):
    node_of_sp = layout["node_of_sp"]
    yfull = np.zeros((cfg.n_real, 1), np.float32)
    for c in range(cfg.nc):
        yc = results[c]["y"].reshape(-1)       # [PC] local order (g*128 + p)
        loc = np.arange(cfg.pc)
        sp = (loc // 128) * cfg.stratum + c * 128 + (loc % 128)
        nodes = node_of_sp[sp]
        ok = nodes >= 0
        yfull[nodes[ok], 0] = yc[ok]
    return yfull


def _absorb_device_wedge():
    """Run a trivial 8-core kernel; a crashed prior session can leave the
    NeuronCores in NRT_EXEC_UNIT_UNRECOVERABLE state for the next session,
    which a fresh trivial execution clears."""
    try:
        from concourse.bass_utils import run_bass_kernel_spmd

        nc_t = bacc.Bacc(None, num_devices=8)
        a = nc_t.dram_tensor("a", [128, 128], F32, kind="ExternalInput")
        o = nc_t.dram_tensor("o", [128, 128], F32, kind="ExternalOutput")
        with tile.TileContext(nc_t) as tc:
            with tc.tile_pool(name="sb", bufs=1) as sb:
                t = sb.tile([128, 128], F32)
                nc_t.sync.dma_start(out=t[:], in_=a[:, :])
                nc_t.sync.dma_start(out=o[:, :], in_=t[:])
        nc_t.finalize()
        run_bass_kernel_spmd(
            nc_t, [{"a": np.zeros((128, 128), np.float32)}] * 8,
            core_ids=list(range(8)),
        )
    except Exception:
        pass


def kernel(**inputs):
    from concourse.bass_utils import run_bass_kernel_spmd

    cfg = Cfg()
    layout = build_layout(inputs["edge_index"], cfg)
    in_maps = build_inputs(
        cfg, layout,
        inputs["x"], inputs["W1"], inputs["a_src1"], inputs["a_dst1"],
        inputs["b1"], inputs["W2"], inputs["a_src2"], inputs["a_dst2"],
        inputs["b2"], inputs["fc_w"], inputs["fc_b"],
    )
    nc_b = build_program(cfg, shared_out=True)
    last_err = None
    for attempt in range(3):
        try:
            res = run_bass_kernel_spmd(
                nc_b, in_maps, core_ids=list(range(cfg.nc))
            )
            return assemble_output(cfg, layout, res.results)
        except Exception as e:  # wedged device from a prior crashed session
            last_err = e
            _absorb_device_wedge()
    raise last_err


if __name__ == "__main__":
    pass


# revision 49
# speedup vs baseline: 1.0180x; 1.0065x over previous
"""Two-layer GAT (PyG GATConv-style) on 8 Trainium2 NeuronCores via Bass/Tile.

Strategy (dst-sharded, data-parallel, fp16 data path):
  - Nodes are assigned to (core, group) "strata": all 8 cores' group g hold
    nodes with similar in-degree statistics (snake-ordered by (cA, cB)), so a
    single SPMD program with uniform per-group slot counts serves all cores.
  - Each core redundantly computes the dense part h = x @ W (plus fused
    alpha_src/alpha_dst columns) for ALL nodes and writes a DRAM row table
    (fp16 row, 768B pitch = [h(256) | a_src(4) | a_dst(4) | pad]); the edge
    phase (gather h[src], per-dst softmax over incoming edges, weighted sum)
    runs only on the core's own 1/8 of destination nodes via dma_gather with
    host-precomputed int16 index lists (slot-major, per-dst-row layout).
  - int16 gather indices only reach 32767, so the table is split into an A
    range and a B range (split aligned to a stratum boundary); every dst row's
    slots are [A-slots | B-slots] with per-group uniform counts KA/KB. Slot 0
    of a node's own side is its self-loop, which supplies alpha_dst.
  - Edge math: e = leaky(as+ad) (no max-shift; dummy slots use as=-60000 so
    exp underflows to 0), p = exp(e), den = sum_k p, alpha = p/den fused into
    an Act-engine per-head expansion to [k, c] fp16, weighted products via a
    DVE 2x-mode tensor_tensor, then a binary-tree fp16 reduction over slots.
  - Between layers, each core's elu(out1)^T block is AllGathered (fp16) so
    layer 2's replicated dense phase can read every node's features.
  - Final sigmoid(fc) output is computed per-core and unpermuted on the host.
"""

import sys

sys.path.insert(0, "/opt/trn_rl_repo")

from dataclasses import dataclass, field

import numpy as np

import concourse.bass as bass
import concourse.bacc as bacc
import concourse.tile as tile
from concourse import mybir
from concourse.library_config import mlp as MLP_LIB

F32 = mybir.dt.float32
F16 = mybir.dt.float16
I16 = mybir.dt.int16
AX = mybir.AxisListType
OP = mybir.AluOpType
ACT = mybir.ActivationFunctionType

HEADS = 4
CH = 64
HC = HEADS * CH          # 256
DIN = 128
NEG_SLOPE = 0.2
EWP = 384                # table row pitch in fp16 elems (768 B, %256==0)
PAY = 272                # payload elems per row: h(256) | as(4) | ad(4) | pad8
AS_OFF = 256             # alpha_src column offset within a row
AD_OFF = 260             # alpha_dst column offset
DUM_AS = -60000.0        # dummy-slot alpha_src (finite; exp underflows to 0)


@dataclass
class Cfg:
    n_real: int = 50000
    nc: int = 8                  # cores
    g: int = 49                  # groups (strata) per core
    split_strat: int = 31        # strata in table range A
    ka: list = field(default_factory=list)   # per-group A slots
    kb: list = field(default_factory=list)   # per-group B slots

    @property
    def stratum(self):
        return self.nc * 128

    @property
    def np_(self):
        return self.g * self.stratum

    @property
    def pc(self):
        return self.g * 128

    @property
    def split_sp(self):
        return self.split_strat * self.stratum

    @property
    def nb(self):
        return self.np_ - self.split_sp

    @property
    def rows(self):
        return self.np_ + 3      # dumA + dumB + padself

    @property
    def row_b0(self):
        return self.split_sp + 1   # dummyB row

    @property
    def padself_idx(self):
        return self.nb + 1         # local B index of the pad-self row


def build_layout(edge_index, cfg: Cfg):
    """Host-side graph layout. Returns per-core index arrays + permutation."""
    n = cfg.n_real
    src = np.asarray(edge_index[0], dtype=np.int64)
    dst = np.asarray(edge_index[1], dtype=np.int64)
    src = np.concatenate([src, np.arange(n, dtype=np.int64)])
    dst = np.concatenate([dst, np.arange(n, dtype=np.int64)])
    deg = np.bincount(dst, minlength=n)

    split = cfg.split_sp
    order1 = np.argsort(-deg, kind="stable")
    a_nodes = order1[:split]
    b_nodes = order1[split:]
    is_a = np.zeros(n, bool)
    is_a[a_nodes] = True
    ca = np.bincount(dst[is_a[src]], minlength=n)
    cb = deg - ca

    def snake(nodes):
        out = []
        flip = False
        avals = ca[nodes]
        for v in np.sort(np.unique(avals))[::-1]:
            grp = nodes[avals == v]
            grp = grp[np.argsort(cb[grp], kind="stable")]
            if flip:
                grp = grp[::-1]
            flip = not flip
            out.append(grp)
        return np.concatenate(out) if out else nodes

    seq = np.concatenate([snake(a_nodes), snake(b_nodes)])
    # node_of_sp: storage position -> node (-1 = pad). pads go at the end.
    node_of_sp = np.full(cfg.np_, -1, np.int64)
    node_of_sp[: len(seq)] = seq
    sp_of = np.full(n, -1, np.int64)
    sp_of[seq] = np.arange(len(seq))

    # per-group slot counts
    ka = np.zeros(cfg.g, np.int64)
    kb = np.zeros(cfg.g, np.int64)
    for s in range(cfg.g):
        nodes = node_of_sp[s * cfg.stratum : (s + 1) * cfg.stratum]
        nodes = nodes[nodes >= 0]
        if len(nodes):
            ka[s] = ca[nodes].max()
            kb[s] = cb[nodes].max()
    ka = np.maximum(ka, 1)
    kb = np.maximum(kb, 1)
    # pads live in the last stratum; their pad-self slot is B slot 0
    cfg.ka = ka.tolist()
    cfg.kb = kb.tolist()

    # group edges by dst
    eorder = np.argsort(dst, kind="stable")
    s_sorted = src[eorder]
    starts = np.searchsorted(dst, np.arange(n + 1), sorter=eorder)

    sp_src = sp_of[s_sorted]           # storage pos of each edge's src
    src_is_a = sp_src < split
    idx_a_val = (sp_src + 1).astype(np.int64)            # A-range row index
    idx_b_val = (sp_src - split + 1).astype(np.int64)    # B-range local row

    def pack(flat):
        # idx element i lives at [i % 16, i // 16]; the two Q7 DGE cores read
        # partitions 0-15 and 16-31 respectively, so replicate into both.
        ln = len(flat)
        f = np.zeros((128, ln // 16), np.int16)
        ii = np.arange(ln)
        f[ii % 16, ii // 16] = flat
        f[16 + (ii % 16), ii // 16] = flat
        return f

    idx_cores = []
    for c in range(cfg.nc):
        flats = []
        for s in range(cfg.g):
            KA, KB = int(ka[s]), int(kb[s])
            ma = np.zeros((KA, 128), np.int64)   # [slot, partition]
            mb = np.zeros((KB, 128), np.int64)
            base_sp = s * cfg.stratum + c * 128
            for p in range(128):
                node = node_of_sp[base_sp + p]
                if node < 0:
                    # pad position: pad-self row keeps den positive
                    mb[0, p] = cfg.padself_idx
                    continue
                lo, hi = starts[node], starts[node + 1]
                aa = idx_a_val[lo:hi][src_is_a[lo:hi]]
                bb = idx_b_val[lo:hi][~src_is_a[lo:hi]]
                # self-loop first in its own side (supplies alpha_dst)
                own_sp = base_sp + p
                if own_sp < split:
                    self_idx = own_sp + 1
                    w = np.where(aa == self_idx)[0]
                    if len(w) == 0:
                        raise RuntimeError("self-loop missing in A list")
                    aa[[0, w[0]]] = aa[[w[0], 0]]
                else:
                    self_idx = own_sp - split + 1
                    w = np.where(bb == self_idx)[0]
                    if len(w) == 0:
                        raise RuntimeError("self-loop missing in B list")
                    bb[[0, w[0]]] = bb[[w[0], 0]]
                ma[: len(aa), p] = aa
                mb[: len(bb), p] = bb
            flats.append(
                np.concatenate([pack(ma.reshape(-1)), pack(mb.reshape(-1))],
                               axis=1)
            )
        idx_cores.append(np.concatenate(flats, axis=1))

    return dict(node_of_sp=node_of_sp, sp_of=sp_of, idx=idx_cores)


def build_program(cfg: Cfg, shared_out: bool = True):
    nc_b = bacc.Bacc(None, num_devices=cfg.nc)
    NP, G, NC = cfg.np_, cfg.g, cfg.nc
    NT = NP // 128                 # global tiles (stratum-major)
    SPLIT_T = cfg.split_sp // 128  # first B tile
    R = cfg.rows
    ROW_B0 = cfg.row_b0
    PC = cfg.pc
    sumK8 = 8 * int(np.sum(cfg.ka) + np.sum(cfg.kb))

    xT = nc_b.dram_tensor("xT", [DIN, NP], F16, kind="ExternalInput")
    W1AUG = nc_b.dram_tensor("W1AUG", [DIN, PAY], F16, kind="ExternalInput")
    W2AUG = nc_b.dram_tensor("W2AUG", [HC, PAY], F16, kind="ExternalInput")
    bmt = nc_b.dram_tensor("bmt", [1, 128], F16, kind="ExternalInput")
    br1 = nc_b.dram_tensor("br1", [1, PAY], F16, kind="ExternalInput")
    br2 = nc_b.dram_tensor("br2", [1, PAY], F16, kind="ExternalInput")
    fcwt = nc_b.dram_tensor("fcwt", [128, HC], F32, kind="ExternalInput")
    fcbt = nc_b.dram_tensor("fcbt", [128, 1], F32, kind="ExternalInput")
    identh = nc_b.dram_tensor("identh", [128, 128], F16, kind="ExternalInput")
    dumrow = nc_b.dram_tensor("dumrow", [3, EWP], F16, kind="ExternalInput")
    idx = nc_b.dram_tensor("idx", [128, sumK8], I16, kind="ExternalInput")
    y = nc_b.dram_tensor("y", [PC, 1], F32, kind="ExternalOutput")

    table1 = nc_b.dram_tensor("table1", [R, EWP], F16)
    table2 = nc_b.dram_tensor("table2", [R, EWP], F16)
    elu1T = nc_b.dram_tensor("elu1T", [HC, NP // NC], F16)
    gath = nc_b.dram_tensor(
        "gath", [NC * HC, NP // NC], F16,
        addr_space="Shared" if shared_out else "Local",
    )

    with tile.TileContext(nc_b) as tc:
        ctxmgrs = [
            tc.tile_pool(name="cst", bufs=1),
            tc.tile_pool(name="lw", bufs=2),
            tc.tile_pool(name="hp", bufs=2),
            tc.tile_pool(name="ge", bufs=3),
            tc.tile_pool(name="px", bufs=2),
            tc.tile_pool(name="th", bufs=1),
            tc.tile_pool(name="sm", bufs=2),
            tc.tile_pool(name="ix", bufs=2),
            tc.tile_pool(name="ph", bufs=1, space="PSUM"),
            tc.tile_pool(name="pt", bufs=2, space="PSUM"),
        ]
        import contextlib

        with contextlib.ExitStack() as st:
            cst, lw, hp, ge, px, th, sm, ix, ph, pt = [
                st.enter_context(m) for m in ctxmgrs
            ]
            nc_b.gpsimd.load_library(MLP_LIB)

            # ---- constants ----
            idt = cst.tile([128, 128], F16)
            nc_b.sync.dma_start(out=idt[:], in_=identh[:, :])
            bms = cst.tile([1, 128], F16)
            nc_b.sync.dma_start(out=bms[:], in_=bmt[:, :])
            br1s = cst.tile([1, PAY], F16)
            nc_b.sync.dma_start(out=br1s[:], in_=br1[:, :])
            br2s = cst.tile([1, PAY], F16)
            nc_b.sync.dma_start(out=br2s[:], in_=br2[:, :])
            fcws = cst.tile([128, HC], F32)
            nc_b.sync.dma_start(out=fcws[:], in_=fcwt[:, :])
            fcbs = cst.tile([128, 1], F32)
            nc_b.sync.dma_start(out=fcbs[:], in_=fcbt[:, :])
            w1s = cst.tile([128, PAY], F16)
            nc_b.sync.dma_start(out=w1s[:], in_=W1AUG[:, :])
            w2s = cst.tile([128, 2, PAY], F16)
            nc_b.sync.dma_start(out=w2s[:, 0, :], in_=W2AUG[0:128, :])
            nc_b.sync.dma_start(out=w2s[:, 1, :], in_=W2AUG[128:HC, :])
            zall = cst.tile([128, G], F32)

            # dummy + pad-self rows for both tables (DRAM -> DRAM)
            for tab in (table1, table2):
                nc_b.sync.dma_start(out=tab[0:1, :], in_=dumrow[0:1, :])
                nc_b.sync.dma_start(
                    out=tab[ROW_B0 : ROW_B0 + 1, :], in_=dumrow[1:2, :]
                )
                nc_b.sync.dma_start(out=tab[R - 1 : R, :], in_=dumrow[2:3, :])

            # ---- dense phase (replicated): h|alphas -> table ----
            def h_phase(layer):
                tab = table1 if layer == 1 else table2
                cp_rr = [0]

                def copy_psum(dst_ap, src_ap):
                    e = cp_rr[0] % 2
                    cp_rr[0] += 1
                    if e == 0:
                        nc_b.scalar.copy(out=dst_ap, in_=src_ap)
                    else:
                        nc_b.vector.tensor_copy(out=dst_ap, in_=src_ap)

                def write_rows(ht_slice, row0, nt):
                    # ht_slice[:, j, :] (j in 0..nt) -> rows row0 + j*128 + p
                    nc_b.sync.dma_start(
                        out=bass.AP(
                            tensor=tab[:, :].tensor,
                            offset=row0 * EWP,
                            ap=[[EWP, 128], [128 * EWP, nt], [1, PAY]],
                        ),
                        in_=ht_slice,
                    )

                if layer == 1:
                    TB = 12
                    t0 = 0
                    while t0 < NT:
                        nt = min(TB, NT - t0)
                        lh = lw.tile([128, TB * 128], F16, tag="gh")
                        nc_b.sync.dma_start(
                            out=lh[:, 0 : nt * 128],
                            in_=xT[:, t0 * 128 : (t0 + nt) * 128],
                        )
                        ht = hp.tile([128, TB, PAY], F16, tag="ht")
                        for j in range(nt):
                            pj = ph.tile([128, PAY], F32, space="PSUM",
                                         tag=f"pj{j % 5}")
                            nc_b.tensor.matmul(
                                out=pj[:], lhsT=lh[:, j * 128 : (j + 1) * 128],
                                rhs=w1s[:], start=True, stop=False,
                            )
                            nc_b.tensor.matmul(
                                out=pj[:], lhsT=bms[:], rhs=br1s[:],
                                start=False, stop=True,
                            )
                            copy_psum(ht[:, j, :], pj[:])
                        # contiguous row runs (split at the A/B boundary)
                        runs = []
                        for j in range(nt):
                            t = t0 + j
                            sh = 1 if t < SPLIT_T else 2
                            if runs and runs[-1][2] == sh:
                                runs[-1][1] += 1
                            else:
                                runs.append([j, 1, sh])
                        for j0, nj, sh in runs:
                            write_rows(ht[:, j0 : j0 + nj, :],
                                       (t0 + j0) * 128 + sh, nj)
                        t0 += nt
                else:
                    TB = 11
                    for c in range(NC):
                        s0 = 0
                        while s0 < G:
                            ns = min(TB, G - s0)
                            gh = lw.tile([128, 2, TB * 128], F16, tag="gh")
                            nc_b.sync.dma_start(
                                out=gh[:, :, 0 : ns * 128],
                                in_=bass.AP(
                                    tensor=gath[:, :].tensor,
                                    offset=(c * HC) * (NP // NC) + s0 * 128,
                                    ap=[[NP // NC, 128],
                                        [128 * (NP // NC), 2],
                                        [1, ns * 128]],
                                ),
                            )
                            ht = hp.tile([128, TB, PAY], F16, tag="ht")
                            for j in range(ns):
                                pj = ph.tile([128, PAY], F32, space="PSUM",
                                             tag=f"pj{j % 5}")
                                nc_b.tensor.matmul(
                                    out=pj[:],
                                    lhsT=gh[:, 0, j * 128 : (j + 1) * 128],
                                    rhs=w2s[:, 0, :], start=True, stop=False,
                                )
                                nc_b.tensor.matmul(
                                    out=pj[:],
                                    lhsT=gh[:, 1, j * 128 : (j + 1) * 128],
                                    rhs=w2s[:, 1, :], start=False, stop=False,
                                )
                                nc_b.tensor.matmul(
                                    out=pj[:], lhsT=bms[:], rhs=br2s[:],
                                    start=False, stop=True,
                                )
                                copy_psum(ht[:, j, :], pj[:])
                            runs = []
                            for j in range(ns):
                                sv = s0 + j
                                sh = 1 if sv < cfg.split_strat else 2
                                if runs and runs[-1][2] == sh:
                                    runs[-1][1] += 1
                                else:
                                    runs.append([j, 1, sh])
                            for j0, nj, sh in runs:
                                sv = s0 + j0
                                nc_b.sync.dma_start(
                                    out=bass.AP(
                                        tensor=tab[:, :].tensor,
                                        offset=(sv * cfg.stratum + c * 128 + sh)
                                        * EWP,
                                        ap=[[EWP, 128],
                                            [cfg.stratum * EWP, nj],
                                            [1, PAY]],
                                    ),
                                    in_=ht[:, j0 : j0 + nj, :],
                                )
                            s0 += ns

            # ---- edge phase ----
            # Software-pipelined: gathers run 2 groups ahead, softmax prep
            # (e/leaky/exp/den/recip/alpha-expansion) 1 group ahead of the
            # heavy weighted-sum work, so no engine head-of-line blocks.
            GCH = 8

            def edge_phase(layer):
                tab = table1 if layer == 1 else table2
                offs = []
                off = 0
                for g in range(G):
                    offs.append(off)
                    off += 8 * (cfg.ka[g] + cfg.kb[g])
                state = {}

                def stage_gather(g):
                    KA, KB = cfg.ka[g], cfg.kb[g]
                    K = KA + KB
                    ixg = ix.tile([128, 8 * K], I16, tag="ixg")
                    nc_b.sync.dma_start(
                        out=ixg[:], in_=idx[:, offs[g] : offs[g] + 8 * K]
                    )
                    gt = ge.tile([128, K * EWP], F16, tag="gt")
                    gta = gt[:]
                    for base, kn, ioff, in_ap in (
                        (0, KA, 0, tab[0:ROW_B0, :]),
                        (KA, KB, 8 * KA, tab[ROW_B0:R, :]),
                    ):
                        for c0 in range(0, kn, GCH):
                            cw = min(GCH, kn - c0)
                            nc_b.gpsimd.dma_gather(
                                out_ap=bass.AP(
                                    tensor=gta.tensor,
                                    offset=gta.offset + (base + c0) * EWP,
                                    ap=[gta.ap[0], [EWP, cw], [1, EWP]],
                                ),
                                in_ap=in_ap,
                                idxs_ap=ixg[:, ioff + 8 * c0 : ioff + 8 * (c0 + cw)],
                                num_idxs=128 * cw,
                                num_idxs_reg=128 * cw,
                                elem_size=EWP,
                            )
                    state[g] = dict(gt=gt)

                def stage_prep(g):
                    KA, KB = cfg.ka[g], cfg.kb[g]
                    K = KA + KB
                    gta = state[g]["gt"][:]
                    # alpha_dst from own self-loop slot (slot 0 of own side)
                    ad_off = AD_OFF if g < cfg.split_strat else KA * EWP + AD_OFF
                    # e[p, h*K+k] = as(slot k, h) + ad(h)   [Act x4,
                    # ad supplied as a per-partition bias column]
                    e = sm.tile([128, HEADS * K], F32, tag="e")
                    for h in range(HEADS):
                        nc_b.scalar.activation(
                            out=e[:, h * K : (h + 1) * K],
                            in_=bass.AP(
                                tensor=gta.tensor,
                                offset=gta.offset + AS_OFF + h,
                                ap=[gta.ap[0], [EWP, K]],
                            ),
                            func=ACT.Identity,
                            bias=bass.AP(
                                tensor=gta.tensor,
                                offset=gta.offset + ad_off + h,
                                ap=[gta.ap[0], [1, 1]],
                            ),
                        )
                    # leaky relu [DVE]
                    el = sm.tile([128, HEADS * K], F32, tag="el")
                    nc_b.vector.scalar_tensor_tensor(
                        out=el[:], in0=e[:], scalar=NEG_SLOPE, in1=e[:],
                        op0=OP.mult, op1=OP.max,
                    )
                    # p = exp(el) [Act]; den = sum_k p [DVE]; rden [DVE]
                    p = sm.tile([128, HEADS * K], F32, tag="p")
                    nc_b.scalar.activation(out=p[:], in_=el[:], func=ACT.Exp)
                    den = sm.tile([128, HEADS], F32, tag="den")
                    nc_b.vector.tensor_reduce(
                        out=den[:],
                        in_=bass.AP(
                            tensor=p[:].tensor, offset=p[:].offset,
                            ap=[p[:].ap[0], [K, HEADS], [1, K]],
                        ),
                        axis=AX.X, op=OP.add,
                    )
                    rden = sm.tile([128, HEADS], F32, tag="rden")
                    nc_b.vector.reciprocal(out=rden[:], in_=den[:])
                    # pexp[p, h, k, 0:2] = alpha = p * rden_h  (fp16) [Act]
                    # Only 2 copies per alpha: the multiply broadcasts over
                    # the middle (c_hi) dim; DVE 2x only requires the LAST
                    # dim packed.
                    pex = px.tile([128, HEADS, K, 2], F16, tag="pex")
                    for h in range(HEADS):
                        nc_b.scalar.activation(
                            out=bass.AP(
                                tensor=pex[:].tensor,
                                offset=pex[:].offset + h * K * 2,
                                ap=[pex[:].ap[0], [2, K], [1, 2]],
                            ),
                            in_=bass.AP(
                                tensor=p[:].tensor, offset=p[:].offset + h * K,
                                ap=[p[:].ap[0], [1, K], [0, 2]],
                            ),
                            func=ACT.Copy,
                            scale=rden[:, h : h + 1],
                        )
                    state[g]["pex"] = pex

                def stage_mult(g):
                    KA, KB = cfg.ka[g], cfg.kb[g]
                    K = KA + KB
                    gta = state[g]["gt"][:]
                    pex = state[g]["pex"]
                    # tht[p, h, k, c] = alpha * h_src  (fp16, DVE 2x);
                    # split per head so each starts as soon as that head's
                    # alpha expansion lands
                    tht = th.tile([128, HEADS * K * CH], F16, tag="tht")
                    for h in range(HEADS):
                        nc_b.vector.tensor_tensor(
                            out=bass.AP(
                                tensor=tht[:].tensor,
                                offset=tht[:].offset + h * K * CH,
                                ap=[tht[:].ap[0], [CH, K], [2, CH // 2],
                                    [1, 2]],
                            ),
                            in0=bass.AP(
                                tensor=gta.tensor, offset=gta.offset + h * CH,
                                ap=[gta.ap[0], [EWP, K], [2, CH // 2], [1, 2]],
                            ),
                            in1=bass.AP(
                                tensor=pex[:].tensor,
                                offset=pex[:].offset + h * K * 2,
                                ap=[pex[:].ap[0], [2, K], [0, CH // 2],
                                    [1, 2]],
                            ),
                            op=OP.mult,
                        )
                    state[g]["tht"] = tht

                def stage_heavy(g):
                    KA, KB = cfg.ka[g], cfg.kb[g]
                    K = KA + KB
                    pex = state[g]["pex"]
                    tht = state[g]["tht"]
                    # binary-tree reduce over k (fp16 DVE 2x adds); ping-pong
                    # tht <-> thB
                    thB = th.tile([128, HEADS * ((K + 1) // 2) * CH], F16,
                                  tag="thB")
                    opre = sm.tile([128, HC], F16, tag="opre")
                    cur, alt = (tht, K * CH), (thB, ((K + 1) // 2) * CH)
                    n = K
                    while n > 1:
                        (cur_t, HS), (alt_t, HSa) = cur, alt
                        ca_, aa_ = cur_t[:], alt_t[:]
                        if n == 2:
                            nc_b.vector.tensor_tensor(
                                out=bass.AP(
                                    tensor=opre[:].tensor, offset=opre[:].offset,
                                    ap=[opre[:].ap[0], [CH, HEADS], [1, CH]],
                                ),
                                in0=bass.AP(
                                    tensor=ca_.tensor, offset=ca_.offset,
                                    ap=[ca_.ap[0], [HS, HEADS], [1, CH]],
                                ),
                                in1=bass.AP(
                                    tensor=ca_.tensor, offset=ca_.offset + CH,
                                    ap=[ca_.ap[0], [HS, HEADS], [1, CH]],
                                ),
                                op=OP.add,
                            )
                            n = 1
                            continue
                        if n % 2 == 1:
                            # fold straggler (slot n-1) into slot 0 in place
                            nc_b.vector.tensor_tensor(
                                out=bass.AP(
                                    tensor=ca_.tensor, offset=ca_.offset,
                                    ap=[ca_.ap[0], [HS, HEADS], [1, CH]],
                                ),
                                in0=bass.AP(
                                    tensor=ca_.tensor, offset=ca_.offset,
                                    ap=[ca_.ap[0], [HS, HEADS], [1, CH]],
                                ),
                                in1=bass.AP(
                                    tensor=ca_.tensor,
                                    offset=ca_.offset + (n - 1) * CH,
                                    ap=[ca_.ap[0], [HS, HEADS], [1, CH]],
                                ),
                                op=OP.add,
                            )
                            n -= 1
                            continue
                        half = n // 2
                        nc_b.vector.tensor_tensor(
                            out=bass.AP(
                                tensor=aa_.tensor, offset=aa_.offset,
                                ap=[aa_.ap[0], [HSa, HEADS], [CH, half],
                                    [1, CH]],
                            ),
                            in0=bass.AP(
                                tensor=ca_.tensor, offset=ca_.offset,
                                ap=[ca_.ap[0], [HS, HEADS], [2 * CH, half],
                                    [1, CH]],
                            ),
                            in1=bass.AP(
                                tensor=ca_.tensor, offset=ca_.offset + CH,
                                ap=[ca_.ap[0], [HS, HEADS], [2 * CH, half],
                                    [1, CH]],
                            ),
                            op=OP.add,
                        )
                        n = half
                        cur, alt = alt, cur

                    # bias already folded into the table rows (rank-1
                    # matmul in the dense phase; softmax weights sum to 1)
                    outb = opre
                    # elu = relu(x) + exp(min(x,0)) - 1.  Layer 1 computes
                    # it exactly; layer 2 computes elu+1 = min(exp(x),1) +
                    # relu(x) (exp overflows for x>~88? no: clamp via min
                    # AFTER exp is exact since exp(x)>=1 iff x>=0) and the -1
                    # is folded into the host-adjusted fc bias.
                    rl = sm.tile([128, HC], F16, tag="rl")
                    nc_b.scalar.activation(out=rl[:], in_=outb[:], func=ACT.Relu)
                    em = sm.tile([128, HC], F16, tag="em")
                    elu = sm.tile([128, HC], F16, tag="elu")
                    if layer == 1:
                        xm = sm.tile([128, HC], F16, tag="xm")
                        nc_b.scalar.activation(out=xm[:], in_=outb[:],
                                               func=ACT.Relu, scale=-1.0)
                        nc_b.scalar.activation(out=em[:], in_=xm[:],
                                               func=ACT.Exp, scale=-1.0)
                        em1 = sm.tile([128, HC], F16, tag="em1")
                        nc_b.scalar.activation(out=em1[:], in_=em[:],
                                               func=ACT.Copy, bias=-1.0)
                        nc_b.vector.tensor_tensor(
                            out=elu[:], in0=em1[:], in1=rl[:], op=OP.add,
                        )
                    else:
                        # elu = min(em,1) + rl - 1 = rl - Relu(1 - em)
                        nc_b.scalar.activation(out=em[:], in_=outb[:],
                                               func=ACT.Exp)
                        t1 = sm.tile([128, HC], F16, tag="em1")
                        nc_b.scalar.activation(out=t1[:], in_=em[:],
                                               func=ACT.Relu, scale=-1.0,
                                               bias=1.0)
                        nc_b.vector.tensor_tensor(
                            out=elu[:], in0=rl[:], in1=t1[:],
                            op=OP.subtract,
                        )
                    if layer == 1:
                        et = sm.tile([128, 2, 128], F16, tag="et")
                        for half_i in range(2):
                            ptr = pt.tile([128, 128], F16, space="PSUM",
                                          tag="ptr")
                            nc_b.tensor.transpose(
                                out=ptr[:],
                                in_=elu[:, half_i * 128 : (half_i + 1) * 128],
                                identity=idt[:],
                            )
                            nc_b.scalar.copy(out=et[:, half_i, :],
                                             in_=ptr[:])
                        nc_b.sync.dma_start(
                            out=bass.AP(
                                tensor=elu1T[:, :].tensor,
                                offset=g * 128,
                                ap=[[NP // NC, 128], [128 * (NP // NC), 2],
                                    [1, 128]],
                            ),
                            in_=et[:],
                        )
                    else:
                        fsc = sm.tile([128, HC], F32, tag="xm")
                        nc_b.vector.scalar_tensor_tensor(
                            out=fsc[:], in0=elu[:], scalar=1.0, in1=fcws[:],
                            op0=OP.bypass, op1=OP.mult,
                            accum_out=zall[:, g : g + 1],
                        )
                    del state[g]

                stage_gather(0)
                if G > 1:
                    stage_gather(1)
                stage_prep(0)
                for g in range(G):
                    if g + 1 < G:
                        stage_prep(g + 1)
                    if g + 2 < G:
                        stage_gather(g + 2)
                    stage_mult(g)
                    stage_heavy(g)

            import os
            phases = os.environ.get("KM_PHASES", "h1,e1,cc,h2,e2").split(",")
            marks = {}

            def mark(label):
                marks[label] = len(nc_b.inst_map)

            nc_b._phase_marks = marks
            mark("setup_end")
            if "h1" in phases:
                h_phase(1)
            mark("h1_end")
            if "e1" in phases:
                edge_phase(1)
            mark("e1_end")
            if "cc" in phases:
                nc_b.gpsimd.collective_compute(
                    "AllGather",
                    OP.bypass,
                    replica_groups=[list(range(NC))],
                    ins=[elu1T[:, :].opt()],
                    outs=[gath[:, :].opt()],
                )
            mark("cc_end")
            if "h2" in phases:
                h_phase(2)
            mark("h2_end")
            if "e2" in phases:
                edge_phase(2)
            mark("e2_end")

            # final: y = sigmoid(z + fc_b), transposed out
            if "e2" not in phases:
                nc_b.vector.memset(zall[:], 0.0)
            zsig = cst.tile([128, G], F16)
            nc_b.scalar.activation(
                out=zsig[:], in_=zall[:], func=ACT.Sigmoid,
                bias=fcbs[:, 0:1], scale=1.0,
            )
            pz = ph.tile([G, 128], F16, space="PSUM", tag="pz")
            nc_b.tensor.transpose(out=pz[:], in_=zsig[:], identity=idt[:])
            yT = cst.tile([G, 128], F32)  # copy converts f16 psum -> f32
            nc_b.vector.tensor_copy(out=yT[:], in_=pz[:])
            nc_b.sync.dma_start(
                out=bass.AP(
                    tensor=y[:, :].tensor, offset=0, ap=[[128, G], [1, 128]]
                ),
                in_=yT[:],
            )
    nc_b.finalize()
    return nc_b


def make_block_diag(a):
    """a: [H, C] -> [H*C, H] block diagonal."""
    out = np.zeros((HC, HEADS), np.float32)
    for h in range(HEADS):
        out[h * CH : (h + 1) * CH, h] = a[h]
    return out


def _aug(W, a_src, a_dst):
    """[W | W@As_bd | W@Ad_bd | pad] as fp16, width PAY."""
    W = np.asarray(W, np.float32)
    aug = np.zeros((W.shape[0], PAY), np.float32)
    aug[:, 0:HC] = W
    aug[:, HC : HC + HEADS] = W @ make_block_diag(np.asarray(a_src, np.float32))
    aug[:, HC + HEADS : HC + 2 * HEADS] = W @ make_block_diag(
        np.asarray(a_dst, np.float32)
    )
    return aug.astype(np.float16)


def build_inputs(cfg: Cfg, layout, x, W1, a_src1, a_dst1, b1, W2, a_src2,
                 a_dst2, b2, fc_w, fc_b):
    NP = cfg.np_
    node_of_sp = layout["node_of_sp"]
    xs = np.zeros((NP, DIN), np.float32)
    valid = node_of_sp >= 0
    xs[valid] = np.asarray(x, np.float32)[node_of_sp[valid]]
    xT = np.ascontiguousarray(xs.T).astype(np.float16)

    dumrow = np.zeros((3, EWP), np.float16)
    dumrow[0:2, AS_OFF : AS_OFF + HEADS] = DUM_AS

    base = dict(
        xT=xT,
        W1AUG=_aug(W1, a_src1, a_dst1),
        W2AUG=_aug(W2, a_src2, a_dst2),
        bmt=np.ones((1, 128), np.float16),
        br1=np.concatenate(
            [np.asarray(b1, np.float16).reshape(1, HC),
             np.zeros((1, PAY - HC), np.float16)], axis=1),
        br2=np.concatenate(
            [np.asarray(b2, np.float16).reshape(1, HC),
             np.zeros((1, PAY - HC), np.float16)], axis=1),
        fcwt=np.broadcast_to(
            np.asarray(fc_w, np.float32).reshape(1, HC), (128, HC)
        ).copy(),
        fcbt=np.full((128, 1), np.float32(np.asarray(fc_b).reshape(-1)[0])),
        identh=np.eye(128, dtype=np.float16),
        dumrow=dumrow,
    )
    in_maps = []
    for c in range(cfg.nc):
        m = dict(base)
        m["idx"] = layout["idx"][c]
        in_maps.append(m)
    return in_maps


def assemble_output(cfg: Cfg, layout, results):
    node_of_sp = layout["node_of_sp"]
    yfull = np.zeros((cfg.n_real, 1), np.float32)
    for c in range(cfg.nc):
        yc = results[c]["y"].reshape(-1)       # [PC] local order (g*128 + p)
        loc = np.arange(cfg.pc)
        sp = (loc // 128) * cfg.stratum + c * 128 + (loc % 128)
        nodes = node_of_sp[sp]
        ok = nodes >= 0
        yfull[nodes[ok], 0] = yc[ok]
    return yfull


def _absorb_device_wedge():
    """Run a trivial 8-core kernel; a crashed prior session can leave the
    NeuronCores in NRT_EXEC_UNIT_UNRECOVERABLE state for the next session,
    which a fresh trivial execution clears."""
    try:
        from concourse.bass_utils import run_bass_kernel_spmd

        nc_t = bacc.Bacc(None, num_devices=8)
        a = nc_t.dram_tensor("a", [128, 128], F32, kind="ExternalInput")
        o = nc_t.dram_tensor("o", [128, 128], F32, kind="ExternalOutput")
        with tile.TileContext(nc_t) as tc:
            with tc.tile_pool(name="sb", bufs=1) as sb:
                t = sb.tile([128, 128], F32)
                nc_t.sync.dma_start(out=t[:], in_=a[:, :])
                nc_t.sync.dma_start(out=o[:, :], in_=t[:])
        nc_t.finalize()
        run_bass_kernel_spmd(
            nc_t, [{"a": np.zeros((128, 128), np.float32)}] * 8,
            core_ids=list(range(8)),
        )
    except Exception:
        pass


def kernel(**inputs):
    from concourse.bass_utils import run_bass_kernel_spmd

    cfg = Cfg()
    layout = build_layout(inputs["edge_index"], cfg)
    in_maps = build_inputs(
        cfg, layout,
        inputs["x"], inputs["W1"], inputs["a_src1"], inputs["a_dst1"],
        inputs["b1"], inputs["W2"], inputs["a_src2"], inputs["a_dst2"],
        inputs["b2"], inputs["fc_w"], inputs["fc_b"],
    )
    nc_b = build_program(cfg, shared_out=True)
    last_err = None
    for attempt in range(3):
        try:
            res = run_bass_kernel_spmd(
                nc_b, in_maps, core_ids=list(range(cfg.nc))
            )
            return assemble_output(cfg, layout, res.results)
        except Exception as e:  # wedged device from a prior crashed session
            last_err = e
            _absorb_device_wedge()
    raise last_err


if __name__ == "__main__":
    pass


# revision 50
# speedup vs baseline: 1.0183x; 1.0004x over previous
"""Two-layer GAT (PyG GATConv-style) on 8 Trainium2 NeuronCores via Bass/Tile.

Strategy (dst-sharded, data-parallel, fp16 data path):
  - Nodes are assigned to (core, group) "strata": all 8 cores' group g hold
    nodes with similar in-degree statistics (snake-ordered by (cA, cB)), so a
    single SPMD program with uniform per-group slot counts serves all cores.
  - Each core redundantly computes the dense part h = x @ W (plus fused
    alpha_src/alpha_dst columns) for ALL nodes and writes a DRAM row table
    (fp16 row, 768B pitch = [h(256) | a_src(4) | a_dst(4) | pad]); the edge
    phase (gather h[src], per-dst softmax over incoming edges, weighted sum)
    runs only on the core's own 1/8 of destination nodes via dma_gather with
    host-precomputed int16 index lists (slot-major, per-dst-row layout).
  - int16 gather indices only reach 32767, so the table is split into an A
    range and a B range (split aligned to a stratum boundary); every dst row's
    slots are [A-slots | B-slots] with per-group uniform counts KA/KB. Slot 0
    of a node's own side is its self-loop, which supplies alpha_dst.
  - Edge math: e = leaky(as+ad) (no max-shift; dummy slots use as=-60000 so
    exp underflows to 0), p = exp(e), den = sum_k p, alpha = p/den fused into
    an Act-engine per-head expansion to [k, c] fp16, weighted products via a
    DVE 2x-mode tensor_tensor, then a binary-tree fp16 reduction over slots.
  - Between layers, each core's elu(out1)^T block is AllGathered (fp16) so
    layer 2's replicated dense phase can read every node's features.
  - Final sigmoid(fc) output is computed per-core and unpermuted on the host.
"""

import sys

sys.path.insert(0, "/opt/trn_rl_repo")

from dataclasses import dataclass, field

import numpy as np

import concourse.bass as bass
import concourse.bacc as bacc
import concourse.tile as tile
from concourse import mybir
from concourse.library_config import mlp as MLP_LIB

F32 = mybir.dt.float32
F16 = mybir.dt.float16
I16 = mybir.dt.int16
AX = mybir.AxisListType
OP = mybir.AluOpType
ACT = mybir.ActivationFunctionType

HEADS = 4
CH = 64
HC = HEADS * CH          # 256
DIN = 128
NEG_SLOPE = 0.2
EWP = 384                # table row pitch in fp16 elems (768 B, %256==0)
PAY = 272                # payload elems per row: h(256) | as(4) | ad(4) | pad8
AS_OFF = 256             # alpha_src column offset within a row
AD_OFF = 260             # alpha_dst column offset
DUM_AS = -60000.0        # dummy-slot alpha_src (finite; exp underflows to 0)


@dataclass
class Cfg:
    n_real: int = 50000
    nc: int = 8                  # cores
    g: int = 49                  # groups (strata) per core
    split_strat: int = 31        # strata in table range A
    ka: list = field(default_factory=list)   # per-group A slots
    kb: list = field(default_factory=list)   # per-group B slots

    @property
    def stratum(self):
        return self.nc * 128

    @property
    def np_(self):
        return self.g * self.stratum

    @property
    def pc(self):
        return self.g * 128

    @property
    def split_sp(self):
        return self.split_strat * self.stratum

    @property
    def nb(self):
        return self.np_ - self.split_sp

    @property
    def rows(self):
        return self.np_ + 3      # dumA + dumB + padself

    @property
    def row_b0(self):
        return self.split_sp + 1   # dummyB row

    @property
    def padself_idx(self):
        return self.nb + 1         # local B index of the pad-self row


def build_layout(edge_index, cfg: Cfg):
    """Host-side graph layout. Returns per-core index arrays + permutation."""
    n = cfg.n_real
    src = np.asarray(edge_index[0], dtype=np.int64)
    dst = np.asarray(edge_index[1], dtype=np.int64)
    src = np.concatenate([src, np.arange(n, dtype=np.int64)])
    dst = np.concatenate([dst, np.arange(n, dtype=np.int64)])
    deg = np.bincount(dst, minlength=n)

    split = cfg.split_sp
    order1 = np.argsort(-deg, kind="stable")
    a_nodes = order1[:split]
    b_nodes = order1[split:]
    is_a = np.zeros(n, bool)
    is_a[a_nodes] = True
    ca = np.bincount(dst[is_a[src]], minlength=n)
    cb = deg - ca

    def snake(nodes):
        out = []
        flip = False
        avals = ca[nodes]
        for v in np.sort(np.unique(avals))[::-1]:
            grp = nodes[avals == v]
            grp = grp[np.argsort(cb[grp], kind="stable")]
            if flip:
                grp = grp[::-1]
            flip = not flip
            out.append(grp)
        return np.concatenate(out) if out else nodes

    seq = np.concatenate([snake(a_nodes), snake(b_nodes)])
    # node_of_sp: storage position -> node (-1 = pad). pads go at the end.
    node_of_sp = np.full(cfg.np_, -1, np.int64)
    node_of_sp[: len(seq)] = seq
    sp_of = np.full(n, -1, np.int64)
    sp_of[seq] = np.arange(len(seq))

    # per-group slot counts
    ka = np.zeros(cfg.g, np.int64)
    kb = np.zeros(cfg.g, np.int64)
    for s in range(cfg.g):
        nodes = node_of_sp[s * cfg.stratum : (s + 1) * cfg.stratum]
        nodes = nodes[nodes >= 0]
        if len(nodes):
            ka[s] = ca[nodes].max()
            kb[s] = cb[nodes].max()
    ka = np.maximum(ka, 1)
    kb = np.maximum(kb, 1)
    # pads live in the last stratum; their pad-self slot is B slot 0
    cfg.ka = ka.tolist()
    cfg.kb = kb.tolist()

    # group edges by dst
    eorder = np.argsort(dst, kind="stable")
    s_sorted = src[eorder]
    starts = np.searchsorted(dst, np.arange(n + 1), sorter=eorder)

    sp_src = sp_of[s_sorted]           # storage pos of each edge's src
    src_is_a = sp_src < split
    idx_a_val = (sp_src + 1).astype(np.int64)            # A-range row index
    idx_b_val = (sp_src - split + 1).astype(np.int64)    # B-range local row

    def pack(flat):
        # idx element i lives at [i % 16, i // 16]; the two Q7 DGE cores read
        # partitions 0-15 and 16-31 respectively, so replicate into both.
        ln = len(flat)
        f = np.zeros((128, ln // 16), np.int16)
        ii = np.arange(ln)
        f[ii % 16, ii // 16] = flat
        f[16 + (ii % 16), ii // 16] = flat
        return f

    idx_cores = []
    for c in range(cfg.nc):
        flats = []
        for s in range(cfg.g):
            KA, KB = int(ka[s]), int(kb[s])
            ma = np.zeros((KA, 128), np.int64)   # [slot, partition]
            mb = np.zeros((KB, 128), np.int64)
            base_sp = s * cfg.stratum + c * 128
            for p in range(128):
                node = node_of_sp[base_sp + p]
                if node < 0:
                    # pad position: pad-self row keeps den positive
                    mb[0, p] = cfg.padself_idx
                    continue
                lo, hi = starts[node], starts[node + 1]
                aa = idx_a_val[lo:hi][src_is_a[lo:hi]]
                bb = idx_b_val[lo:hi][~src_is_a[lo:hi]]
                # self-loop first in its own side (supplies alpha_dst)
                own_sp = base_sp + p
                if own_sp < split:
                    self_idx = own_sp + 1
                    w = np.where(aa == self_idx)[0]
                    if len(w) == 0:
                        raise RuntimeError("self-loop missing in A list")
                    aa[[0, w[0]]] = aa[[w[0], 0]]
                else:
                    self_idx = own_sp - split + 1
                    w = np.where(bb == self_idx)[0]
                    if len(w) == 0:
                        raise RuntimeError("self-loop missing in B list")
                    bb[[0, w[0]]] = bb[[w[0], 0]]
                ma[: len(aa), p] = aa
                mb[: len(bb), p] = bb
            flats.append(
                np.concatenate([pack(ma.reshape(-1)), pack(mb.reshape(-1))],
                               axis=1)
            )
        idx_cores.append(np.concatenate(flats, axis=1))

    return dict(node_of_sp=node_of_sp, sp_of=sp_of, idx=idx_cores)


def build_program(cfg: Cfg, shared_out: bool = True):
    nc_b = bacc.Bacc(None, num_devices=cfg.nc)
    NP, G, NC = cfg.np_, cfg.g, cfg.nc
    NT = NP // 128                 # global tiles (stratum-major)
    SPLIT_T = cfg.split_sp // 128  # first B tile
    R = cfg.rows
    ROW_B0 = cfg.row_b0
    PC = cfg.pc
    sumK8 = 8 * int(np.sum(cfg.ka) + np.sum(cfg.kb))

    xT = nc_b.dram_tensor("xT", [DIN, NP], F16, kind="ExternalInput")
    W1AUG = nc_b.dram_tensor("W1AUG", [DIN, PAY], F16, kind="ExternalInput")
    W2AUG = nc_b.dram_tensor("W2AUG", [HC, PAY], F16, kind="ExternalInput")
    bmt = nc_b.dram_tensor("bmt", [1, 128], F16, kind="ExternalInput")
    br1 = nc_b.dram_tensor("br1", [1, PAY], F16, kind="ExternalInput")
    br2 = nc_b.dram_tensor("br2", [1, PAY], F16, kind="ExternalInput")
    fcwt = nc_b.dram_tensor("fcwt", [128, HC], F32, kind="ExternalInput")
    fcbt = nc_b.dram_tensor("fcbt", [128, 1], F32, kind="ExternalInput")
    identh = nc_b.dram_tensor("identh", [128, 128], F16, kind="ExternalInput")
    dumrow = nc_b.dram_tensor("dumrow", [3, EWP], F16, kind="ExternalInput")
    idx = nc_b.dram_tensor("idx", [128, sumK8], I16, kind="ExternalInput")
    y = nc_b.dram_tensor("y", [PC, 1], F32, kind="ExternalOutput")

    table1 = nc_b.dram_tensor("table1", [R, EWP], F16)
    table2 = nc_b.dram_tensor("table2", [R, EWP], F16)
    elu1T = nc_b.dram_tensor("elu1T", [HC, NP // NC], F16)
    gath = nc_b.dram_tensor(
        "gath", [NC * HC, NP // NC], F16,
        addr_space="Shared" if shared_out else "Local",
    )

    with tile.TileContext(nc_b) as tc:
        ctxmgrs = [
            tc.tile_pool(name="cst", bufs=1),
            tc.tile_pool(name="lw", bufs=2),
            tc.tile_pool(name="hp", bufs=2),
            tc.tile_pool(name="ge", bufs=3),
            tc.tile_pool(name="px", bufs=2),
            tc.tile_pool(name="th", bufs=1),
            tc.tile_pool(name="sm", bufs=2),
            tc.tile_pool(name="ix", bufs=2),
            tc.tile_pool(name="ph", bufs=1, space="PSUM"),
            tc.tile_pool(name="pt", bufs=2, space="PSUM"),
        ]
        import contextlib

        with contextlib.ExitStack() as st:
            cst, lw, hp, ge, px, th, sm, ix, ph, pt = [
                st.enter_context(m) for m in ctxmgrs
            ]
            nc_b.gpsimd.load_library(MLP_LIB)

            # ---- constants ----
            idt = cst.tile([128, 128], F16)
            nc_b.sync.dma_start(out=idt[:], in_=identh[:, :])
            bms = cst.tile([1, 128], F16)
            nc_b.sync.dma_start(out=bms[:], in_=bmt[:, :])
            br1s = cst.tile([1, PAY], F16)
            nc_b.sync.dma_start(out=br1s[:], in_=br1[:, :])
            br2s = cst.tile([1, PAY], F16)
            nc_b.sync.dma_start(out=br2s[:], in_=br2[:, :])
            fcws = cst.tile([128, HC], F32)
            nc_b.sync.dma_start(out=fcws[:], in_=fcwt[:, :])
            fcbs = cst.tile([128, 1], F32)
            nc_b.sync.dma_start(out=fcbs[:], in_=fcbt[:, :])
            w1s = cst.tile([128, PAY], F16)
            nc_b.sync.dma_start(out=w1s[:], in_=W1AUG[:, :])
            w2s = cst.tile([128, 2, PAY], F16)
            nc_b.sync.dma_start(out=w2s[:, 0, :], in_=W2AUG[0:128, :])
            nc_b.sync.dma_start(out=w2s[:, 1, :], in_=W2AUG[128:HC, :])
            zall = cst.tile([128, G], F32)

            # dummy + pad-self rows for both tables (DRAM -> DRAM)
            for tab in (table1, table2):
                nc_b.sync.dma_start(out=tab[0:1, :], in_=dumrow[0:1, :])
                nc_b.sync.dma_start(
                    out=tab[ROW_B0 : ROW_B0 + 1, :], in_=dumrow[1:2, :]
                )
                nc_b.sync.dma_start(out=tab[R - 1 : R, :], in_=dumrow[2:3, :])

            # ---- dense phase (replicated): h|alphas -> table ----
            def h_phase(layer):
                tab = table1 if layer == 1 else table2
                cp_rr = [0]

                def copy_psum(dst_ap, src_ap):
                    e = cp_rr[0] % 2
                    cp_rr[0] += 1
                    if e == 0:
                        nc_b.scalar.copy(out=dst_ap, in_=src_ap)
                    else:
                        nc_b.vector.tensor_copy(out=dst_ap, in_=src_ap)

                def write_rows(ht_slice, row0, nt):
                    # ht_slice[:, j, :] (j in 0..nt) -> rows row0 + j*128 + p
                    nc_b.sync.dma_start(
                        out=bass.AP(
                            tensor=tab[:, :].tensor,
                            offset=row0 * EWP,
                            ap=[[EWP, 128], [128 * EWP, nt], [1, PAY]],
                        ),
                        in_=ht_slice,
                    )

                if layer == 1:
                    TB = 12
                    t0 = 0
                    while t0 < NT:
                        nt = min(TB, NT - t0)
                        lh = lw.tile([128, TB * 128], F16, tag="gh")
                        nc_b.sync.dma_start(
                            out=lh[:, 0 : nt * 128],
                            in_=xT[:, t0 * 128 : (t0 + nt) * 128],
                        )
                        ht = hp.tile([128, TB, PAY], F16, tag="ht")
                        for j in range(nt):
                            pj = ph.tile([128, PAY], F32, space="PSUM",
                                         tag=f"pj{j % 5}")
                            nc_b.tensor.matmul(
                                out=pj[:], lhsT=lh[:, j * 128 : (j + 1) * 128],
                                rhs=w1s[:], start=True, stop=False,
                            )
                            nc_b.tensor.matmul(
                                out=pj[:], lhsT=bms[:], rhs=br1s[:],
                                start=False, stop=True,
                            )
                            copy_psum(ht[:, j, :], pj[:])
                        # contiguous row runs (split at the A/B boundary)
                        runs = []
                        for j in range(nt):
                            t = t0 + j
                            sh = 1 if t < SPLIT_T else 2
                            if runs and runs[-1][2] == sh:
                                runs[-1][1] += 1
                            else:
                                runs.append([j, 1, sh])
                        for j0, nj, sh in runs:
                            write_rows(ht[:, j0 : j0 + nj, :],
                                       (t0 + j0) * 128 + sh, nj)
                        t0 += nt
                else:
                    TB = 11
                    for c in range(NC):
                        s0 = 0
                        while s0 < G:
                            ns = min(TB, G - s0)
                            gh = lw.tile([128, 2, TB * 128], F16, tag="gh")
                            nc_b.sync.dma_start(
                                out=gh[:, :, 0 : ns * 128],
                                in_=bass.AP(
                                    tensor=gath[:, :].tensor,
                                    offset=(c * HC) * (NP // NC) + s0 * 128,
                                    ap=[[NP // NC, 128],
                                        [128 * (NP // NC), 2],
                                        [1, ns * 128]],
                                ),
                            )
                            ht = hp.tile([128, TB, PAY], F16, tag="ht")
                            for j in range(ns):
                                pj = ph.tile([128, PAY], F32, space="PSUM",
                                             tag=f"pj{j % 5}")
                                nc_b.tensor.matmul(
                                    out=pj[:],
                                    lhsT=gh[:, 0, j * 128 : (j + 1) * 128],
                                    rhs=w2s[:, 0, :], start=True, stop=False,
                                )
                                nc_b.tensor.matmul(
                                    out=pj[:],
                                    lhsT=gh[:, 1, j * 128 : (j + 1) * 128],
                                    rhs=w2s[:, 1, :], start=False, stop=False,
                                )
                                nc_b.tensor.matmul(
                                    out=pj[:], lhsT=bms[:], rhs=br2s[:],
                                    start=False, stop=True,
                                )
                                copy_psum(ht[:, j, :], pj[:])
                            runs = []
                            for j in range(ns):
                                sv = s0 + j
                                sh = 1 if sv < cfg.split_strat else 2
                                if runs and runs[-1][2] == sh:
                                    runs[-1][1] += 1
                                else:
                                    runs.append([j, 1, sh])
                            for j0, nj, sh in runs:
                                sv = s0 + j0
                                nc_b.sync.dma_start(
                                    out=bass.AP(
                                        tensor=tab[:, :].tensor,
                                        offset=(sv * cfg.stratum + c * 128 + sh)
                                        * EWP,
                                        ap=[[EWP, 128],
                                            [cfg.stratum * EWP, nj],
                                            [1, PAY]],
                                    ),
                                    in_=ht[:, j0 : j0 + nj, :],
                                )
                            s0 += ns

            # ---- edge phase ----
            # Software-pipelined: gathers run 2 groups ahead, softmax prep
            # (e/leaky/exp/den/recip/alpha-expansion) 1 group ahead of the
            # heavy weighted-sum work, so no engine head-of-line blocks.
            GCH = 8

            def edge_phase(layer):
                tab = table1 if layer == 1 else table2
                offs = []
                off = 0
                for g in range(G):
                    offs.append(off)
                    off += 8 * (cfg.ka[g] + cfg.kb[g])
                state = {}

                def stage_gather(g):
                    KA, KB = cfg.ka[g], cfg.kb[g]
                    K = KA + KB
                    ixg = ix.tile([128, 8 * K], I16, tag="ixg")
                    nc_b.sync.dma_start(
                        out=ixg[:], in_=idx[:, offs[g] : offs[g] + 8 * K]
                    )
                    gt = ge.tile([128, K * EWP], F16, tag="gt")
                    gta = gt[:]
                    for base, kn, ioff, in_ap in (
                        (0, KA, 0, tab[0:ROW_B0, :]),
                        (KA, KB, 8 * KA, tab[ROW_B0:R, :]),
                    ):
                        for c0 in range(0, kn, GCH):
                            cw = min(GCH, kn - c0)
                            nc_b.gpsimd.dma_gather(
                                out_ap=bass.AP(
                                    tensor=gta.tensor,
                                    offset=gta.offset + (base + c0) * EWP,
                                    ap=[gta.ap[0], [EWP, cw], [1, EWP]],
                                ),
                                in_ap=in_ap,
                                idxs_ap=ixg[:, ioff + 8 * c0 : ioff + 8 * (c0 + cw)],
                                num_idxs=128 * cw,
                                num_idxs_reg=128 * cw,
                                elem_size=EWP,
                            )
                    state[g] = dict(gt=gt)

                def stage_prep(g):
                    KA, KB = cfg.ka[g], cfg.kb[g]
                    K = KA + KB
                    gta = state[g]["gt"][:]
                    # alpha_dst from own self-loop slot (slot 0 of own side)
                    ad_off = AD_OFF if g < cfg.split_strat else KA * EWP + AD_OFF
                    # e[p, h*K+k] = as(slot k, h) + ad(h)   [Act x4,
                    # ad supplied as a per-partition bias column]
                    e = sm.tile([128, HEADS * K], F32, tag="e")
                    for h in range(HEADS):
                        nc_b.scalar.activation(
                            out=e[:, h * K : (h + 1) * K],
                            in_=bass.AP(
                                tensor=gta.tensor,
                                offset=gta.offset + AS_OFF + h,
                                ap=[gta.ap[0], [EWP, K]],
                            ),
                            func=ACT.Identity,
                            bias=bass.AP(
                                tensor=gta.tensor,
                                offset=gta.offset + ad_off + h,
                                ap=[gta.ap[0], [1, 1]],
                            ),
                        )
                    # leaky relu [DVE]
                    el = sm.tile([128, HEADS * K], F32, tag="el")
                    nc_b.vector.scalar_tensor_tensor(
                        out=el[:], in0=e[:], scalar=NEG_SLOPE, in1=e[:],
                        op0=OP.mult, op1=OP.max,
                    )
                    # p = exp(el) [Act]; den = sum_k p [DVE]; rden [DVE]
                    p = sm.tile([128, HEADS * K], F32, tag="p")
                    nc_b.scalar.activation(out=p[:], in_=el[:], func=ACT.Exp)
                    den = sm.tile([128, HEADS], F32, tag="den")
                    nc_b.vector.tensor_reduce(
                        out=den[:],
                        in_=bass.AP(
                            tensor=p[:].tensor, offset=p[:].offset,
                            ap=[p[:].ap[0], [K, HEADS], [1, K]],
                        ),
                        axis=AX.X, op=OP.add,
                    )
                    rden = sm.tile([128, HEADS], F32, tag="rden")
                    nc_b.vector.reciprocal(out=rden[:], in_=den[:])
                    # pexp[p, h, k, 0:2] = alpha = p * rden_h  (fp16) [Act]
                    # Only 2 copies per alpha: the multiply broadcasts over
                    # the middle (c_hi) dim; DVE 2x only requires the LAST
                    # dim packed.
                    pex = px.tile([128, HEADS, K, 2], F16, tag="pex")
                    for h in range(HEADS):
                        nc_b.scalar.activation(
                            out=bass.AP(
                                tensor=pex[:].tensor,
                                offset=pex[:].offset + h * K * 2,
                                ap=[pex[:].ap[0], [2, K], [1, 2]],
                            ),
                            in_=bass.AP(
                                tensor=p[:].tensor, offset=p[:].offset + h * K,
                                ap=[p[:].ap[0], [1, K], [0, 2]],
                            ),
                            func=ACT.Copy,
                            scale=rden[:, h : h + 1],
                        )
                    state[g]["pex"] = pex

                def stage_mult(g):
                    KA, KB = cfg.ka[g], cfg.kb[g]
                    K = KA + KB
                    gta = state[g]["gt"][:]
                    pex = state[g]["pex"]
                    # tht[p, h, k, c] = alpha * h_src  (fp16, DVE 2x);
                    # split per head so each starts as soon as that head's
                    # alpha expansion lands
                    tht = th.tile([128, HEADS * K * CH], F16, tag="tht")
                    for h in range(HEADS):
                        nc_b.vector.tensor_tensor(
                            out=bass.AP(
                                tensor=tht[:].tensor,
                                offset=tht[:].offset + h * K * CH,
                                ap=[tht[:].ap[0], [CH, K], [2, CH // 2],
                                    [1, 2]],
                            ),
                            in0=bass.AP(
                                tensor=gta.tensor, offset=gta.offset + h * CH,
                                ap=[gta.ap[0], [EWP, K], [2, CH // 2], [1, 2]],
                            ),
                            in1=bass.AP(
                                tensor=pex[:].tensor,
                                offset=pex[:].offset + h * K * 2,
                                ap=[pex[:].ap[0], [2, K], [0, CH // 2],
                                    [1, 2]],
                            ),
                            op=OP.mult,
                        )
                    state[g]["tht"] = tht

                def stage_heavy(g):
                    KA, KB = cfg.ka[g], cfg.kb[g]
                    K = KA + KB
                    pex = state[g]["pex"]
                    tht = state[g]["tht"]
                    # binary-tree reduce over k (fp16 DVE 2x adds); ping-pong
                    # tht <-> thB
                    thB = th.tile([128, HEADS * ((K + 1) // 2) * CH], F16,
                                  tag="thB")
                    opre = sm.tile([128, HC], F16, tag="opre")
                    cur, alt = (tht, K * CH), (thB, ((K + 1) // 2) * CH)
                    n = K
                    while n > 1:
                        (cur_t, HS), (alt_t, HSa) = cur, alt
                        ca_, aa_ = cur_t[:], alt_t[:]
                        if n == 2:
                            nc_b.vector.tensor_tensor(
                                out=bass.AP(
                                    tensor=opre[:].tensor, offset=opre[:].offset,
                                    ap=[opre[:].ap[0], [CH, HEADS], [1, CH]],
                                ),
                                in0=bass.AP(
                                    tensor=ca_.tensor, offset=ca_.offset,
                                    ap=[ca_.ap[0], [HS, HEADS], [1, CH]],
                                ),
                                in1=bass.AP(
                                    tensor=ca_.tensor, offset=ca_.offset + CH,
                                    ap=[ca_.ap[0], [HS, HEADS], [1, CH]],
                                ),
                                op=OP.add,
                            )
                            n = 1
                            continue
                        if n % 2 == 1:
                            # fold straggler (slot n-1) into slot 0 in place
                            nc_b.vector.tensor_tensor(
                                out=bass.AP(
                                    tensor=ca_.tensor, offset=ca_.offset,
                                    ap=[ca_.ap[0], [HS, HEADS], [1, CH]],
                                ),
                                in0=bass.AP(
                                    tensor=ca_.tensor, offset=ca_.offset,
                                    ap=[ca_.ap[0], [HS, HEADS], [1, CH]],
                                ),
                                in1=bass.AP(
                                    tensor=ca_.tensor,
                                    offset=ca_.offset + (n - 1) * CH,
                                    ap=[ca_.ap[0], [HS, HEADS], [1, CH]],
                                ),
                                op=OP.add,
                            )
                            n -= 1
                            continue
                        half = n // 2
                        nc_b.vector.tensor_tensor(
                            out=bass.AP(
                                tensor=aa_.tensor, offset=aa_.offset,
                                ap=[aa_.ap[0], [HSa, HEADS], [CH, half],
                                    [1, CH]],
                            ),
                            in0=bass.AP(
                                tensor=ca_.tensor, offset=ca_.offset,
                                ap=[ca_.ap[0], [HS, HEADS], [2 * CH, half],
                                    [1, CH]],
                            ),
                            in1=bass.AP(
                                tensor=ca_.tensor, offset=ca_.offset + CH,
                                ap=[ca_.ap[0], [HS, HEADS], [2 * CH, half],
                                    [1, CH]],
                            ),
                            op=OP.add,
                        )
                        n = half
                        cur, alt = alt, cur

                    # bias already folded into the table rows (rank-1
                    # matmul in the dense phase; softmax weights sum to 1)
                    outb = opre
                    # elu = relu(x) + exp(min(x,0)) - 1.  Layer 1 computes
                    # it exactly; layer 2 computes elu+1 = min(exp(x),1) +
                    # relu(x) (exp overflows for x>~88? no: clamp via min
                    # AFTER exp is exact since exp(x)>=1 iff x>=0) and the -1
                    # is folded into the host-adjusted fc bias.
                    rl = sm.tile([128, HC], F16, tag="rl")
                    nc_b.scalar.activation(out=rl[:], in_=outb[:], func=ACT.Relu)
                    em = sm.tile([128, HC], F16, tag="em")
                    elu = sm.tile([128, HC], F16, tag="elu")
                    # elu = rl - Relu(1 - exp(x)): exact for all x
                    # (x>0: Relu term is 0 and elu=rl=x; fp16 exp overflow to
                    # +inf is safe since Relu(1-inf)=0)
                    nc_b.scalar.activation(out=em[:], in_=outb[:],
                                           func=ACT.Exp)
                    t1 = sm.tile([128, HC], F16, tag="em1")
                    nc_b.scalar.activation(out=t1[:], in_=em[:],
                                           func=ACT.Relu, scale=-1.0,
                                           bias=1.0)
                    nc_b.vector.tensor_tensor(
                        out=elu[:], in0=rl[:], in1=t1[:], op=OP.subtract,
                    )
                    if layer == 1:
                        et = sm.tile([128, 2, 128], F16, tag="et")
                        for half_i in range(2):
                            ptr = pt.tile([128, 128], F16, space="PSUM",
                                          tag="ptr")
                            nc_b.tensor.transpose(
                                out=ptr[:],
                                in_=elu[:, half_i * 128 : (half_i + 1) * 128],
                                identity=idt[:],
                            )
                            nc_b.scalar.copy(out=et[:, half_i, :],
                                             in_=ptr[:])
                        nc_b.sync.dma_start(
                            out=bass.AP(
                                tensor=elu1T[:, :].tensor,
                                offset=g * 128,
                                ap=[[NP // NC, 128], [128 * (NP // NC), 2],
                                    [1, 128]],
                            ),
                            in_=et[:],
                        )
                    else:
                        fsc = sm.tile([128, HC], F32, tag="xm")
                        nc_b.vector.scalar_tensor_tensor(
                            out=fsc[:], in0=elu[:], scalar=1.0, in1=fcws[:],
                            op0=OP.bypass, op1=OP.mult,
                            accum_out=zall[:, g : g + 1],
                        )
                    del state[g]

                stage_gather(0)
                if G > 1:
                    stage_gather(1)
                stage_prep(0)
                for g in range(G):
                    if g + 1 < G:
                        stage_prep(g + 1)
                    if g + 2 < G:
                        stage_gather(g + 2)
                    stage_mult(g)
                    stage_heavy(g)

            import os
            phases = os.environ.get("KM_PHASES", "h1,e1,cc,h2,e2").split(",")
            marks = {}

            def mark(label):
                marks[label] = len(nc_b.inst_map)

            nc_b._phase_marks = marks
            mark("setup_end")
            if "h1" in phases:
                h_phase(1)
            mark("h1_end")
            if "e1" in phases:
                edge_phase(1)
            mark("e1_end")
            if "cc" in phases:
                nc_b.gpsimd.collective_compute(
                    "AllGather",
                    OP.bypass,
                    replica_groups=[list(range(NC))],
                    ins=[elu1T[:, :].opt()],
                    outs=[gath[:, :].opt()],
                )
            mark("cc_end")
            if "h2" in phases:
                h_phase(2)
            mark("h2_end")
            if "e2" in phases:
                edge_phase(2)
            mark("e2_end")

            # final: y = sigmoid(z + fc_b), transposed out
            if "e2" not in phases:
                nc_b.vector.memset(zall[:], 0.0)
            zsig = cst.tile([128, G], F16)
            nc_b.scalar.activation(
                out=zsig[:], in_=zall[:], func=ACT.Sigmoid,
                bias=fcbs[:, 0:1], scale=1.0,
            )
            pz = ph.tile([G, 128], F16, space="PSUM", tag="pz")
            nc_b.tensor.transpose(out=pz[:], in_=zsig[:], identity=idt[:])
            yT = cst.tile([G, 128], F32)  # copy converts f16 psum -> f32
            nc_b.vector.tensor_copy(out=yT[:], in_=pz[:])
            nc_b.sync.dma_start(
                out=bass.AP(
                    tensor=y[:, :].tensor, offset=0, ap=[[128, G], [1, 128]]
                ),
                in_=yT[:],
            )
    nc_b.finalize()
    return nc_b


def make_block_diag(a):
    """a: [H, C] -> [H*C, H] block diagonal."""
    out = np.zeros((HC, HEADS), np.float32)
    for h in range(HEADS):
        out[h * CH : (h + 1) * CH, h] = a[h]
    return out


def _aug(W, a_src, a_dst):
    """[W | W@As_bd | W@Ad_bd | pad] as fp16, width PAY."""
    W = np.asarray(W, np.float32)
    aug = np.zeros((W.shape[0], PAY), np.float32)
    aug[:, 0:HC] = W
    aug[:, HC : HC + HEADS] = W @ make_block_diag(np.asarray(a_src, np.float32))
    aug[:, HC + HEADS : HC + 2 * HEADS] = W @ make_block_diag(
        np.asarray(a_dst, np.float32)
    )
    return aug.astype(np.float16)


def build_inputs(cfg: Cfg, layout, x, W1, a_src1, a_dst1, b1, W2, a_src2,
                 a_dst2, b2, fc_w, fc_b):
    NP = cfg.np_
    node_of_sp = layout["node_of_sp"]
    xs = np.zeros((NP, DIN), np.float32)
    valid = node_of_sp >= 0
    xs[valid] = np.asarray(x, np.float32)[node_of_sp[valid]]
    xT = np.ascontiguousarray(xs.T).astype(np.float16)

    dumrow = np.zeros((3, EWP), np.float16)
    dumrow[0:2, AS_OFF : AS_OFF + HEADS] = DUM_AS

    base = dict(
        xT=xT,
        W1AUG=_aug(W1, a_src1, a_dst1),
        W2AUG=_aug(W2, a_src2, a_dst2),
        bmt=np.ones((1, 128), np.float16),
        br1=np.concatenate(
            [np.asarray(b1, np.float16).reshape(1, HC),
             np.zeros((1, PAY - HC), np.float16)], axis=1),
        br2=np.concatenate(
            [np.asarray(b2, np.float16).reshape(1, HC),
             np.zeros((1, PAY - HC), np.float16)], axis=1),
        fcwt=np.broadcast_to(
            np.asarray(fc_w, np.float32).reshape(1, HC), (128, HC)
        ).copy(),
        fcbt=np.full((128, 1), np.float32(np.asarray(fc_b).reshape(-1)[0])),
        identh=np.eye(128, dtype=np.float16),
        dumrow=dumrow,
    )
    in_maps = []
    for c in range(cfg.nc):
        m = dict(base)
        m["idx"] = layout["idx"][c]
        in_maps.append(m)
    return in_maps


def assemble_output(cfg: Cfg, layout, results):
    node_of_sp = layout["node_of_sp"]
    yfull = np.zeros((cfg.n_real, 1), np.float32)
    for c in range(cfg.nc):
        yc = results[c]["y"].reshape(-1)       # [PC] local order (g*128 + p)
        loc = np.arange(cfg.pc)
        sp = (loc // 128) * cfg.stratum + c * 128 + (loc % 128)
        nodes = node_of_sp[sp]
        ok = nodes >= 0
        yfull[nodes[ok], 0] = yc[ok]
    return yfull


def _absorb_device_wedge():
    """Run a trivial 8-core kernel; a crashed prior session can leave the
    NeuronCores in NRT_EXEC_UNIT_UNRECOVERABLE state for the next session,
    which a fresh trivial execution clears."""
    try:
        from concourse.bass_utils import run_bass_kernel_spmd

        nc_t = bacc.Bacc(None, num_devices=8)
        a = nc_t.dram_tensor("a", [128, 128], F32, kind="ExternalInput")
        o = nc_t.dram_tensor("o", [128, 128], F32, kind="ExternalOutput")
        with tile.TileContext(nc_t) as tc:
            with tc.tile_pool(name="sb", bufs=1) as sb:
                t = sb.tile([128, 128], F32)
                nc_t.sync.dma_start(out=t[:], in_=a[:, :])
                nc_t.sync.dma_start(out=o[:, :], in_=t[:])
        nc_t.finalize()
        run_bass_kernel_spmd(
            nc_t, [{"a": np.zeros((128, 128), np.float32)}] * 8,
            core_ids=list(range(8)),
        )
    except Exception:
        pass


def kernel(**inputs):
    from concourse.bass_utils import run_bass_kernel_spmd

    cfg = Cfg()
    layout = build_layout(inputs["edge_index"], cfg)
    in_maps = build_inputs(
        cfg, layout,
        inputs["x"], inputs["W1"], inputs["a_src1"], inputs["a_dst1"],
        inputs["b1"], inputs["W2"], inputs["a_src2"], inputs["a_dst2"],
        inputs["b2"], inputs["fc_w"], inputs["fc_b"],
    )
    nc_b = build_program(cfg, shared_out=True)
    last_err = None
    for attempt in range(3):
        try:
            res = run_bass_kernel_spmd(
                nc_b, in_maps, core_ids=list(range(cfg.nc))
            )
            return assemble_output(cfg, layout, res.results)
        except Exception as e:  # wedged device from a prior crashed session
            last_err = e
            _absorb_device_wedge()
    raise last_err


if __name__ == "__main__":
    pass


# revision 51
# speedup vs baseline: 1.0282x; 1.0096x over previous
"""Two-layer GAT (PyG GATConv-style) on 8 Trainium2 NeuronCores via Bass/Tile.

Strategy (dst-sharded, data-parallel, fp16 data path):
  - Nodes are assigned to (core, group) "strata": all 8 cores' group g hold
    nodes with similar in-degree statistics (snake-ordered by (cA, cB)), so a
    single SPMD program with uniform per-group slot counts serves all cores.
  - Each core redundantly computes the dense part h = x @ W (plus fused
    alpha_src/alpha_dst columns) for ALL nodes and writes a DRAM row table
    (fp16 row, 768B pitch = [h(256) | a_src(4) | a_dst(4) | pad]); the edge
    phase (gather h[src], per-dst softmax over incoming edges, weighted sum)
    runs only on the core's own 1/8 of destination nodes via dma_gather with
    host-precomputed int16 index lists (slot-major, per-dst-row layout).
  - int16 gather indices only reach 32767, so the table is split into an A
    range and a B range (split aligned to a stratum boundary); every dst row's
    slots are [A-slots | B-slots] with per-group uniform counts KA/KB. Slot 0
    of a node's own side is its self-loop, which supplies alpha_dst.
  - Edge math: e = leaky(as+ad) (no max-shift; dummy slots use as=-60000 so
    exp underflows to 0), p = exp(e), den = sum_k p, alpha = p/den fused into
    an Act-engine per-head expansion to [k, c] fp16, weighted products via a
    DVE 2x-mode tensor_tensor, then a binary-tree fp16 reduction over slots.
  - Between layers, each core's elu(out1)^T block is AllGathered (fp16) so
    layer 2's replicated dense phase can read every node's features.
  - Final sigmoid(fc) output is computed per-core and unpermuted on the host.
"""

import sys

sys.path.insert(0, "/opt/trn_rl_repo")

from dataclasses import dataclass, field

import numpy as np

import concourse.bass as bass
import concourse.bacc as bacc
import concourse.tile as tile
from concourse import mybir
from concourse.library_config import mlp as MLP_LIB

F32 = mybir.dt.float32
F16 = mybir.dt.float16
I16 = mybir.dt.int16
AX = mybir.AxisListType
OP = mybir.AluOpType
ACT = mybir.ActivationFunctionType

HEADS = 4
CH = 64
HC = HEADS * CH          # 256
DIN = 128
NEG_SLOPE = 0.2
EWP = 384                # table row pitch in fp16 elems (768 B, %256==0)
PAY = 272                # payload elems per row: h(256) | as(4) | ad(4) | pad8
AS_OFF = 256             # alpha_src column offset within a row
AD_OFF = 260             # alpha_dst column offset
DUM_AS = -60000.0        # dummy-slot alpha_src (finite; exp underflows to 0)


@dataclass
class Cfg:
    n_real: int = 50000
    nc: int = 8                  # cores
    g: int = 49                  # groups (strata) per core
    split_strat: int = 31        # strata in table range A
    ka: list = field(default_factory=list)   # per-group A slots
    kb: list = field(default_factory=list)   # per-group B slots

    @property
    def stratum(self):
        return self.nc * 128

    @property
    def np_(self):
        return self.g * self.stratum

    @property
    def pc(self):
        return self.g * 128

    @property
    def split_sp(self):
        return self.split_strat * self.stratum

    @property
    def nb(self):
        return self.np_ - self.split_sp

    @property
    def rows(self):
        return self.np_ + 3      # dumA + dumB + padself

    @property
    def row_b0(self):
        return self.split_sp + 1   # dummyB row

    @property
    def padself_idx(self):
        return self.nb + 1         # local B index of the pad-self row


def build_layout(edge_index, cfg: Cfg):
    """Host-side graph layout. Returns per-core index arrays + permutation."""
    n = cfg.n_real
    src = np.asarray(edge_index[0], dtype=np.int64)
    dst = np.asarray(edge_index[1], dtype=np.int64)
    src = np.concatenate([src, np.arange(n, dtype=np.int64)])
    dst = np.concatenate([dst, np.arange(n, dtype=np.int64)])
    deg = np.bincount(dst, minlength=n)

    split = cfg.split_sp
    order1 = np.argsort(-deg, kind="stable")
    a_nodes = order1[:split]
    b_nodes = order1[split:]
    is_a = np.zeros(n, bool)
    is_a[a_nodes] = True
    ca = np.bincount(dst[is_a[src]], minlength=n)
    cb = deg - ca

    def snake(nodes):
        out = []
        flip = False
        avals = ca[nodes]
        for v in np.sort(np.unique(avals))[::-1]:
            grp = nodes[avals == v]
            grp = grp[np.argsort(cb[grp], kind="stable")]
            if flip:
                grp = grp[::-1]
            flip = not flip
            out.append(grp)
        return np.concatenate(out) if out else nodes

    seq = np.concatenate([snake(a_nodes), snake(b_nodes)])
    # node_of_sp: storage position -> node (-1 = pad). pads go at the end.
    node_of_sp = np.full(cfg.np_, -1, np.int64)
    node_of_sp[: len(seq)] = seq
    sp_of = np.full(n, -1, np.int64)
    sp_of[seq] = np.arange(len(seq))

    # per-group slot counts
    ka = np.zeros(cfg.g, np.int64)
    kb = np.zeros(cfg.g, np.int64)
    for s in range(cfg.g):
        nodes = node_of_sp[s * cfg.stratum : (s + 1) * cfg.stratum]
        nodes = nodes[nodes >= 0]
        if len(nodes):
            ka[s] = ca[nodes].max()
            kb[s] = cb[nodes].max()
    ka = np.maximum(ka, 1)
    kb = np.maximum(kb, 1)
    # pads live in the last stratum; their pad-self slot is B slot 0
    cfg.ka = ka.tolist()
    cfg.kb = kb.tolist()

    # group edges by dst
    eorder = np.argsort(dst, kind="stable")
    s_sorted = src[eorder]
    starts = np.searchsorted(dst, np.arange(n + 1), sorter=eorder)

    sp_src = sp_of[s_sorted]           # storage pos of each edge's src
    src_is_a = sp_src < split
    idx_a_val = (sp_src + 1).astype(np.int64)            # A-range row index
    idx_b_val = (sp_src - split + 1).astype(np.int64)    # B-range local row

    def pack(flat):
        # idx element i lives at [i % 16, i // 16]; the two Q7 DGE cores read
        # partitions 0-15 and 16-31 respectively, so replicate into both.
        ln = len(flat)
        f = np.zeros((128, ln // 16), np.int16)
        ii = np.arange(ln)
        f[ii % 16, ii // 16] = flat
        f[16 + (ii % 16), ii // 16] = flat
        return f

    idx_cores = []
    for c in range(cfg.nc):
        flats = []
        for s in range(cfg.g):
            KA, KB = int(ka[s]), int(kb[s])
            ma = np.zeros((KA, 128), np.int64)   # [slot, partition]
            mb = np.zeros((KB, 128), np.int64)
            base_sp = s * cfg.stratum + c * 128
            for p in range(128):
                node = node_of_sp[base_sp + p]
                if node < 0:
                    # pad position: pad-self row keeps den positive
                    mb[0, p] = cfg.padself_idx
                    continue
                lo, hi = starts[node], starts[node + 1]
                aa = idx_a_val[lo:hi][src_is_a[lo:hi]]
                bb = idx_b_val[lo:hi][~src_is_a[lo:hi]]
                # self-loop first in its own side (supplies alpha_dst)
                own_sp = base_sp + p
                if own_sp < split:
                    self_idx = own_sp + 1
                    w = np.where(aa == self_idx)[0]
                    if len(w) == 0:
                        raise RuntimeError("self-loop missing in A list")
                    aa[[0, w[0]]] = aa[[w[0], 0]]
                else:
                    self_idx = own_sp - split + 1
                    w = np.where(bb == self_idx)[0]
                    if len(w) == 0:
                        raise RuntimeError("self-loop missing in B list")
                    bb[[0, w[0]]] = bb[[w[0], 0]]
                ma[: len(aa), p] = aa
                mb[: len(bb), p] = bb
            flats.append(
                np.concatenate([pack(ma.reshape(-1)), pack(mb.reshape(-1))],
                               axis=1)
            )
        idx_cores.append(np.concatenate(flats, axis=1))

    return dict(node_of_sp=node_of_sp, sp_of=sp_of, idx=idx_cores)


def build_program(cfg: Cfg, shared_out: bool = True):
    nc_b = bacc.Bacc(None, num_devices=cfg.nc)
    NP, G, NC = cfg.np_, cfg.g, cfg.nc
    NT = NP // 128                 # global tiles (stratum-major)
    SPLIT_T = cfg.split_sp // 128  # first B tile
    R = cfg.rows
    ROW_B0 = cfg.row_b0
    PC = cfg.pc
    sumK8 = 8 * int(np.sum(cfg.ka) + np.sum(cfg.kb))

    xT = nc_b.dram_tensor("xT", [DIN, NP], F16, kind="ExternalInput")
    W1AUG = nc_b.dram_tensor("W1AUG", [DIN, PAY], F16, kind="ExternalInput")
    W2AUG = nc_b.dram_tensor("W2AUG", [HC, PAY], F16, kind="ExternalInput")
    bmt = nc_b.dram_tensor("bmt", [1, 128], F16, kind="ExternalInput")
    br1 = nc_b.dram_tensor("br1", [1, PAY], F16, kind="ExternalInput")
    br2 = nc_b.dram_tensor("br2", [1, PAY], F16, kind="ExternalInput")
    fcwt = nc_b.dram_tensor("fcwt", [128, HC], F32, kind="ExternalInput")
    fcbt = nc_b.dram_tensor("fcbt", [128, 1], F32, kind="ExternalInput")
    identh = nc_b.dram_tensor("identh", [128, 128], F16, kind="ExternalInput")
    dumrow = nc_b.dram_tensor("dumrow", [3, EWP], F16, kind="ExternalInput")
    idx = nc_b.dram_tensor("idx", [128, sumK8], I16, kind="ExternalInput")
    y = nc_b.dram_tensor("y", [PC, 1], F32, kind="ExternalOutput")

    table1 = nc_b.dram_tensor("table1", [R, EWP], F16)
    table2 = nc_b.dram_tensor("table2", [R, EWP], F16)
    elu1T = nc_b.dram_tensor("elu1T", [HC, NP // NC], F16)
    gath = nc_b.dram_tensor(
        "gath", [NC * HC, NP // NC], F16,
        addr_space="Shared" if shared_out else "Local",
    )

    with tile.TileContext(nc_b) as tc:
        ctxmgrs = [
            tc.tile_pool(name="cst", bufs=1),
            tc.tile_pool(name="lw", bufs=3),
            tc.tile_pool(name="hp", bufs=3),
            tc.tile_pool(name="ge", bufs=3),
            tc.tile_pool(name="px", bufs=2),
            tc.tile_pool(name="th", bufs=1),
            tc.tile_pool(name="sm", bufs=2),
            tc.tile_pool(name="ix", bufs=2),
            tc.tile_pool(name="ph", bufs=1, space="PSUM"),
            tc.tile_pool(name="pt", bufs=2, space="PSUM"),
        ]
        import contextlib

        with contextlib.ExitStack() as st:
            cst, lw, hp, ge, px, th, sm, ix, ph, pt = [
                st.enter_context(m) for m in ctxmgrs
            ]
            nc_b.gpsimd.load_library(MLP_LIB)

            # ---- constants ----
            idt = cst.tile([128, 128], F16)
            nc_b.sync.dma_start(out=idt[:], in_=identh[:, :])
            bms = cst.tile([1, 128], F16)
            nc_b.sync.dma_start(out=bms[:], in_=bmt[:, :])
            br1s = cst.tile([1, PAY], F16)
            nc_b.sync.dma_start(out=br1s[:], in_=br1[:, :])
            br2s = cst.tile([1, PAY], F16)
            nc_b.sync.dma_start(out=br2s[:], in_=br2[:, :])
            fcws = cst.tile([128, HC], F32)
            nc_b.sync.dma_start(out=fcws[:], in_=fcwt[:, :])
            fcbs = cst.tile([128, 1], F32)
            nc_b.sync.dma_start(out=fcbs[:], in_=fcbt[:, :])
            w1s = cst.tile([128, PAY], F16)
            nc_b.sync.dma_start(out=w1s[:], in_=W1AUG[:, :])
            w2s = cst.tile([128, 2, PAY], F16)
            nc_b.sync.dma_start(out=w2s[:, 0, :], in_=W2AUG[0:128, :])
            nc_b.sync.dma_start(out=w2s[:, 1, :], in_=W2AUG[128:HC, :])
            zall = cst.tile([128, G], F32)

            # dummy + pad-self rows for both tables (DRAM -> DRAM)
            for tab in (table1, table2):
                nc_b.sync.dma_start(out=tab[0:1, :], in_=dumrow[0:1, :])
                nc_b.sync.dma_start(
                    out=tab[ROW_B0 : ROW_B0 + 1, :], in_=dumrow[1:2, :]
                )
                nc_b.sync.dma_start(out=tab[R - 1 : R, :], in_=dumrow[2:3, :])

            # ---- dense phase (replicated): h|alphas -> table ----
            def h_phase(layer):
                tab = table1 if layer == 1 else table2
                cp_rr = [0]

                def copy_psum(dst_ap, src_ap):
                    e = cp_rr[0] % 2
                    cp_rr[0] += 1
                    if e == 0:
                        nc_b.scalar.copy(out=dst_ap, in_=src_ap)
                    else:
                        nc_b.vector.tensor_copy(out=dst_ap, in_=src_ap)

                def write_rows(ht_slice, row0, nt):
                    # ht_slice[:, j, :] (j in 0..nt) -> rows row0 + j*128 + p
                    nc_b.sync.dma_start(
                        out=bass.AP(
                            tensor=tab[:, :].tensor,
                            offset=row0 * EWP,
                            ap=[[EWP, 128], [128 * EWP, nt], [1, PAY]],
                        ),
                        in_=ht_slice,
                    )

                if layer == 1:
                    TB = 12
                    t0 = 0
                    while t0 < NT:
                        nt = min(TB, NT - t0)
                        lh = lw.tile([128, TB * 128], F16, tag="gh")
                        nc_b.sync.dma_start(
                            out=lh[:, 0 : nt * 128],
                            in_=xT[:, t0 * 128 : (t0 + nt) * 128],
                        )
                        ht = hp.tile([128, TB, PAY], F16, tag="ht")
                        for j in range(nt):
                            pj = ph.tile([128, PAY], F32, space="PSUM",
                                         tag=f"pj{j % 5}")
                            nc_b.tensor.matmul(
                                out=pj[:], lhsT=lh[:, j * 128 : (j + 1) * 128],
                                rhs=w1s[:], start=True, stop=False,
                            )
                            nc_b.tensor.matmul(
                                out=pj[:], lhsT=bms[:], rhs=br1s[:],
                                start=False, stop=True,
                            )
                            copy_psum(ht[:, j, :], pj[:])
                        # contiguous row runs (split at the A/B boundary)
                        runs = []
                        for j in range(nt):
                            t = t0 + j
                            sh = 1 if t < SPLIT_T else 2
                            if runs and runs[-1][2] == sh:
                                runs[-1][1] += 1
                            else:
                                runs.append([j, 1, sh])
                        for j0, nj, sh in runs:
                            write_rows(ht[:, j0 : j0 + nj, :],
                                       (t0 + j0) * 128 + sh, nj)
                        t0 += nt
                else:
                    TB = 11
                    for c in range(NC):
                        s0 = 0
                        while s0 < G:
                            ns = min(TB, G - s0)
                            gh = lw.tile([128, 2, TB * 128], F16, tag="gh")
                            nc_b.sync.dma_start(
                                out=gh[:, :, 0 : ns * 128],
                                in_=bass.AP(
                                    tensor=gath[:, :].tensor,
                                    offset=(c * HC) * (NP // NC) + s0 * 128,
                                    ap=[[NP // NC, 128],
                                        [128 * (NP // NC), 2],
                                        [1, ns * 128]],
                                ),
                            )
                            ht = hp.tile([128, TB, PAY], F16, tag="ht")
                            for j in range(ns):
                                pj = ph.tile([128, PAY], F32, space="PSUM",
                                             tag=f"pj{j % 5}")
                                nc_b.tensor.matmul(
                                    out=pj[:],
                                    lhsT=gh[:, 0, j * 128 : (j + 1) * 128],
                                    rhs=w2s[:, 0, :], start=True, stop=False,
                                )
                                nc_b.tensor.matmul(
                                    out=pj[:],
                                    lhsT=gh[:, 1, j * 128 : (j + 1) * 128],
                                    rhs=w2s[:, 1, :], start=False, stop=False,
                                )
                                nc_b.tensor.matmul(
                                    out=pj[:], lhsT=bms[:], rhs=br2s[:],
                                    start=False, stop=True,
                                )
                                copy_psum(ht[:, j, :], pj[:])
                            runs = []
                            for j in range(ns):
                                sv = s0 + j
                                sh = 1 if sv < cfg.split_strat else 2
                                if runs and runs[-1][2] == sh:
                                    runs[-1][1] += 1
                                else:
                                    runs.append([j, 1, sh])
                            for j0, nj, sh in runs:
                                sv = s0 + j0
                                nc_b.sync.dma_start(
                                    out=bass.AP(
                                        tensor=tab[:, :].tensor,
                                        offset=(sv * cfg.stratum + c * 128 + sh)
                                        * EWP,
                                        ap=[[EWP, 128],
                                            [cfg.stratum * EWP, nj],
                                            [1, PAY]],
                                    ),
                                    in_=ht[:, j0 : j0 + nj, :],
                                )
                            s0 += ns

            # ---- edge phase ----
            # Software-pipelined: gathers run 2 groups ahead, softmax prep
            # (e/leaky/exp/den/recip/alpha-expansion) 1 group ahead of the
            # heavy weighted-sum work, so no engine head-of-line blocks.
            GCH = 8

            def edge_phase(layer):
                tab = table1 if layer == 1 else table2
                offs = []
                off = 0
                for g in range(G):
                    offs.append(off)
                    off += 8 * (cfg.ka[g] + cfg.kb[g])
                state = {}

                def stage_gather(g):
                    KA, KB = cfg.ka[g], cfg.kb[g]
                    K = KA + KB
                    ixg = ix.tile([128, 8 * K], I16, tag="ixg")
                    nc_b.sync.dma_start(
                        out=ixg[:], in_=idx[:, offs[g] : offs[g] + 8 * K]
                    )
                    gt = ge.tile([128, K * EWP], F16, tag="gt")
                    gta = gt[:]
                    for base, kn, ioff, in_ap in (
                        (0, KA, 0, tab[0:ROW_B0, :]),
                        (KA, KB, 8 * KA, tab[ROW_B0:R, :]),
                    ):
                        for c0 in range(0, kn, GCH):
                            cw = min(GCH, kn - c0)
                            nc_b.gpsimd.dma_gather(
                                out_ap=bass.AP(
                                    tensor=gta.tensor,
                                    offset=gta.offset + (base + c0) * EWP,
                                    ap=[gta.ap[0], [EWP, cw], [1, EWP]],
                                ),
                                in_ap=in_ap,
                                idxs_ap=ixg[:, ioff + 8 * c0 : ioff + 8 * (c0 + cw)],
                                num_idxs=128 * cw,
                                num_idxs_reg=128 * cw,
                                elem_size=EWP,
                            )
                    state[g] = dict(gt=gt)

                def stage_prep(g):
                    KA, KB = cfg.ka[g], cfg.kb[g]
                    K = KA + KB
                    gta = state[g]["gt"][:]
                    # alpha_dst from own self-loop slot (slot 0 of own side)
                    ad_off = AD_OFF if g < cfg.split_strat else KA * EWP + AD_OFF
                    # e[p, h*K+k] = as(slot k, h) + ad(h)   [Act x4,
                    # ad supplied as a per-partition bias column]
                    e = sm.tile([128, HEADS * K], F32, tag="e")
                    for h in range(HEADS):
                        nc_b.scalar.activation(
                            out=e[:, h * K : (h + 1) * K],
                            in_=bass.AP(
                                tensor=gta.tensor,
                                offset=gta.offset + AS_OFF + h,
                                ap=[gta.ap[0], [EWP, K]],
                            ),
                            func=ACT.Identity,
                            bias=bass.AP(
                                tensor=gta.tensor,
                                offset=gta.offset + ad_off + h,
                                ap=[gta.ap[0], [1, 1]],
                            ),
                        )
                    # leaky relu [DVE]
                    el = sm.tile([128, HEADS * K], F32, tag="el")
                    nc_b.vector.scalar_tensor_tensor(
                        out=el[:], in0=e[:], scalar=NEG_SLOPE, in1=e[:],
                        op0=OP.mult, op1=OP.max,
                    )
                    # p = exp(el) [Act]; den = sum_k p [DVE]; rden [DVE]
                    p = sm.tile([128, HEADS * K], F32, tag="p")
                    nc_b.scalar.activation(out=p[:], in_=el[:], func=ACT.Exp)
                    den = sm.tile([128, HEADS], F32, tag="den")
                    nc_b.vector.tensor_reduce(
                        out=den[:],
                        in_=bass.AP(
                            tensor=p[:].tensor, offset=p[:].offset,
                            ap=[p[:].ap[0], [K, HEADS], [1, K]],
                        ),
                        axis=AX.X, op=OP.add,
                    )
                    rden = sm.tile([128, HEADS], F32, tag="rden")
                    nc_b.vector.reciprocal(out=rden[:], in_=den[:])
                    # pexp[p, h, k, 0:2] = alpha = p * rden_h  (fp16) [Act]
                    # Only 2 copies per alpha: the multiply broadcasts over
                    # the middle (c_hi) dim; DVE 2x only requires the LAST
                    # dim packed.
                    pex = px.tile([128, HEADS, K, 2], F16, tag="pex")
                    for h in range(HEADS):
                        nc_b.scalar.activation(
                            out=bass.AP(
                                tensor=pex[:].tensor,
                                offset=pex[:].offset + h * K * 2,
                                ap=[pex[:].ap[0], [2, K], [1, 2]],
                            ),
                            in_=bass.AP(
                                tensor=p[:].tensor, offset=p[:].offset + h * K,
                                ap=[p[:].ap[0], [1, K], [0, 2]],
                            ),
                            func=ACT.Copy,
                            scale=rden[:, h : h + 1],
                        )
                    state[g]["pex"] = pex

                def stage_mult(g):
                    KA, KB = cfg.ka[g], cfg.kb[g]
                    K = KA + KB
                    gta = state[g]["gt"][:]
                    pex = state[g]["pex"]
                    # tht[p, h, k, c] = alpha * h_src  (fp16, DVE 2x);
                    # split per head so each starts as soon as that head's
                    # alpha expansion lands
                    tht = th.tile([128, HEADS * K * CH], F16, tag="tht")
                    for h in range(HEADS):
                        nc_b.vector.tensor_tensor(
                            out=bass.AP(
                                tensor=tht[:].tensor,
                                offset=tht[:].offset + h * K * CH,
                                ap=[tht[:].ap[0], [CH, K], [2, CH // 2],
                                    [1, 2]],
                            ),
                            in0=bass.AP(
                                tensor=gta.tensor, offset=gta.offset + h * CH,
                                ap=[gta.ap[0], [EWP, K], [2, CH // 2], [1, 2]],
                            ),
                            in1=bass.AP(
                                tensor=pex[:].tensor,
                                offset=pex[:].offset + h * K * 2,
                                ap=[pex[:].ap[0], [2, K], [0, CH // 2],
                                    [1, 2]],
                            ),
                            op=OP.mult,
                        )
                    state[g]["tht"] = tht

                def stage_heavy(g):
                    KA, KB = cfg.ka[g], cfg.kb[g]
                    K = KA + KB
                    pex = state[g]["pex"]
                    tht = state[g]["tht"]
                    # binary-tree reduce over k (fp16 DVE 2x adds); ping-pong
                    # tht <-> thB
                    thB = th.tile([128, HEADS * ((K + 1) // 2) * CH], F16,
                                  tag="thB")
                    opre = sm.tile([128, HC], F16, tag="opre")
                    cur, alt = (tht, K * CH), (thB, ((K + 1) // 2) * CH)
                    n = K
                    while n > 1:
                        (cur_t, HS), (alt_t, HSa) = cur, alt
                        ca_, aa_ = cur_t[:], alt_t[:]
                        if n == 2:
                            nc_b.vector.tensor_tensor(
                                out=bass.AP(
                                    tensor=opre[:].tensor, offset=opre[:].offset,
                                    ap=[opre[:].ap[0], [CH, HEADS], [1, CH]],
                                ),
                                in0=bass.AP(
                                    tensor=ca_.tensor, offset=ca_.offset,
                                    ap=[ca_.ap[0], [HS, HEADS], [1, CH]],
                                ),
                                in1=bass.AP(
                                    tensor=ca_.tensor, offset=ca_.offset + CH,
                                    ap=[ca_.ap[0], [HS, HEADS], [1, CH]],
                                ),
                                op=OP.add,
                            )
                            n = 1
                            continue
                        if n % 2 == 1:
                            # fold straggler (slot n-1) into slot 0 in place
                            nc_b.vector.tensor_tensor(
                                out=bass.AP(
                                    tensor=ca_.tensor, offset=ca_.offset,
                                    ap=[ca_.ap[0], [HS, HEADS], [1, CH]],
                                ),
                                in0=bass.AP(
                                    tensor=ca_.tensor, offset=ca_.offset,
                                    ap=[ca_.ap[0], [HS, HEADS], [1, CH]],
                                ),
                                in1=bass.AP(
                                    tensor=ca_.tensor,
                                    offset=ca_.offset + (n - 1) * CH,
                                    ap=[ca_.ap[0], [HS, HEADS], [1, CH]],
                                ),
                                op=OP.add,
                            )
                            n -= 1
                            continue
                        half = n // 2
                        nc_b.vector.tensor_tensor(
                            out=bass.AP(
                                tensor=aa_.tensor, offset=aa_.offset,
                                ap=[aa_.ap[0], [HSa, HEADS], [CH, half],
                                    [1, CH]],
                            ),
                            in0=bass.AP(
                                tensor=ca_.tensor, offset=ca_.offset,
                                ap=[ca_.ap[0], [HS, HEADS], [2 * CH, half],
                                    [1, CH]],
                            ),
                            in1=bass.AP(
                                tensor=ca_.tensor, offset=ca_.offset + CH,
                                ap=[ca_.ap[0], [HS, HEADS], [2 * CH, half],
                                    [1, CH]],
                            ),
                            op=OP.add,
                        )
                        n = half
                        cur, alt = alt, cur

                    # bias already folded into the table rows (rank-1
                    # matmul in the dense phase; softmax weights sum to 1)
                    outb = opre
                    # elu = relu(x) + exp(min(x,0)) - 1.  Layer 1 computes
                    # it exactly; layer 2 computes elu+1 = min(exp(x),1) +
                    # relu(x) (exp overflows for x>~88? no: clamp via min
                    # AFTER exp is exact since exp(x)>=1 iff x>=0) and the -1
                    # is folded into the host-adjusted fc bias.
                    rl = sm.tile([128, HC], F16, tag="rl")
                    nc_b.scalar.activation(out=rl[:], in_=outb[:], func=ACT.Relu)
                    em = sm.tile([128, HC], F16, tag="em")
                    elu = sm.tile([128, HC], F16, tag="elu")
                    # elu = rl - Relu(1 - exp(x)): exact for all x
                    # (x>0: Relu term is 0 and elu=rl=x; fp16 exp overflow to
                    # +inf is safe since Relu(1-inf)=0)
                    nc_b.scalar.activation(out=em[:], in_=outb[:],
                                           func=ACT.Exp)
                    t1 = sm.tile([128, HC], F16, tag="em1")
                    nc_b.scalar.activation(out=t1[:], in_=em[:],
                                           func=ACT.Relu, scale=-1.0,
                                           bias=1.0)
                    nc_b.vector.tensor_tensor(
                        out=elu[:], in0=rl[:], in1=t1[:], op=OP.subtract,
                    )
                    if layer == 1:
                        et = sm.tile([128, 2, 128], F16, tag="et")
                        for half_i in range(2):
                            ptr = pt.tile([128, 128], F16, space="PSUM",
                                          tag="ptr")
                            nc_b.tensor.transpose(
                                out=ptr[:],
                                in_=elu[:, half_i * 128 : (half_i + 1) * 128],
                                identity=idt[:],
                            )
                            nc_b.scalar.copy(out=et[:, half_i, :],
                                             in_=ptr[:])
                        nc_b.sync.dma_start(
                            out=bass.AP(
                                tensor=elu1T[:, :].tensor,
                                offset=g * 128,
                                ap=[[NP // NC, 128], [128 * (NP // NC), 2],
                                    [1, 128]],
                            ),
                            in_=et[:],
                        )
                    else:
                        fsc = sm.tile([128, HC], F32, tag="xm")
                        nc_b.vector.scalar_tensor_tensor(
                            out=fsc[:], in0=elu[:], scalar=1.0, in1=fcws[:],
                            op0=OP.bypass, op1=OP.mult,
                            accum_out=zall[:, g : g + 1],
                        )
                    del state[g]

                stage_gather(0)
                if G > 1:
                    stage_gather(1)
                stage_prep(0)
                for g in range(G):
                    if g + 1 < G:
                        stage_prep(g + 1)
                    if g + 2 < G:
                        stage_gather(g + 2)
                    stage_mult(g)
                    stage_heavy(g)

            import os
            phases = os.environ.get("KM_PHASES", "h1,e1,cc,h2,e2").split(",")
            marks = {}

            def mark(label):
                marks[label] = len(nc_b.inst_map)

            nc_b._phase_marks = marks
            mark("setup_end")
            if "h1" in phases:
                h_phase(1)
            mark("h1_end")
            if "e1" in phases:
                edge_phase(1)
            mark("e1_end")
            if "cc" in phases:
                nc_b.gpsimd.collective_compute(
                    "AllGather",
                    OP.bypass,
                    replica_groups=[list(range(NC))],
                    ins=[elu1T[:, :].opt()],
                    outs=[gath[:, :].opt()],
                )
            mark("cc_end")
            if "h2" in phases:
                h_phase(2)
            mark("h2_end")
            if "e2" in phases:
                edge_phase(2)
            mark("e2_end")

            # final: y = sigmoid(z + fc_b), transposed out
            if "e2" not in phases:
                nc_b.vector.memset(zall[:], 0.0)
            zsig = cst.tile([128, G], F16)
            nc_b.scalar.activation(
                out=zsig[:], in_=zall[:], func=ACT.Sigmoid,
                bias=fcbs[:, 0:1], scale=1.0,
            )
            pz = ph.tile([G, 128], F16, space="PSUM", tag="pz")
            nc_b.tensor.transpose(out=pz[:], in_=zsig[:], identity=idt[:])
            yT = cst.tile([G, 128], F32)  # copy converts f16 psum -> f32
            nc_b.vector.tensor_copy(out=yT[:], in_=pz[:])
            nc_b.sync.dma_start(
                out=bass.AP(
                    tensor=y[:, :].tensor, offset=0, ap=[[128, G], [1, 128]]
                ),
                in_=yT[:],
            )
    nc_b.finalize()
    return nc_b


def make_block_diag(a):
    """a: [H, C] -> [H*C, H] block diagonal."""
    out = np.zeros((HC, HEADS), np.float32)
    for h in range(HEADS):
        out[h * CH : (h + 1) * CH, h] = a[h]
    return out


def _aug(W, a_src, a_dst):
    """[W | W@As_bd | W@Ad_bd | pad] as fp16, width PAY."""
    W = np.asarray(W, np.float32)
    aug = np.zeros((W.shape[0], PAY), np.float32)
    aug[:, 0:HC] = W
    aug[:, HC : HC + HEADS] = W @ make_block_diag(np.asarray(a_src, np.float32))
    aug[:, HC + HEADS : HC + 2 * HEADS] = W @ make_block_diag(
        np.asarray(a_dst, np.float32)
    )
    return aug.astype(np.float16)


def build_inputs(cfg: Cfg, layout, x, W1, a_src1, a_dst1, b1, W2, a_src2,
                 a_dst2, b2, fc_w, fc_b):
    NP = cfg.np_
    node_of_sp = layout["node_of_sp"]
    xs = np.zeros((NP, DIN), np.float32)
    valid = node_of_sp >= 0
    xs[valid] = np.asarray(x, np.float32)[node_of_sp[valid]]
    xT = np.ascontiguousarray(xs.T).astype(np.float16)

    dumrow = np.zeros((3, EWP), np.float16)
    dumrow[0:2, AS_OFF : AS_OFF + HEADS] = DUM_AS

    base = dict(
        xT=xT,
        W1AUG=_aug(W1, a_src1, a_dst1),
        W2AUG=_aug(W2, a_src2, a_dst2),
        bmt=np.ones((1, 128), np.float16),
        br1=np.concatenate(
            [np.asarray(b1, np.float16).reshape(1, HC),
             np.zeros((1, PAY - HC), np.float16)], axis=1),
        br2=np.concatenate(
            [np.asarray(b2, np.float16).reshape(1, HC),
             np.zeros((1, PAY - HC), np.float16)], axis=1),
        fcwt=np.broadcast_to(
            np.asarray(fc_w, np.float32).reshape(1, HC), (128, HC)
        ).copy(),
        fcbt=np.full((128, 1), np.float32(np.asarray(fc_b).reshape(-1)[0])),
        identh=np.eye(128, dtype=np.float16),
        dumrow=dumrow,
    )
    in_maps = []
    for c in range(cfg.nc):
        m = dict(base)
        m["idx"] = layout["idx"][c]
        in_maps.append(m)
    return in_maps


def assemble_output(cfg: Cfg, layout, results):
    node_of_sp = layout["node_of_sp"]
    yfull = np.zeros((cfg.n_real, 1), np.float32)
    for c in range(cfg.nc):
        yc = results[c]["y"].reshape(-1)       # [PC] local order (g*128 + p)
        loc = np.arange(cfg.pc)
        sp = (loc // 128) * cfg.stratum + c * 128 + (loc % 128)
        nodes = node_of_sp[sp]
        ok = nodes >= 0
        yfull[nodes[ok], 0] = yc[ok]
    return yfull


def _absorb_device_wedge():
    """Run a trivial 8-core kernel; a crashed prior session can leave the
    NeuronCores in NRT_EXEC_UNIT_UNRECOVERABLE state for the next session,
    which a fresh trivial execution clears."""
    try:
        from concourse.bass_utils import run_bass_kernel_spmd

        nc_t = bacc.Bacc(None, num_devices=8)
        a = nc_t.dram_tensor("a", [128, 128], F32, kind="ExternalInput")
        o = nc_t.dram_tensor("o", [128, 128], F32, kind="ExternalOutput")
        with tile.TileContext(nc_t) as tc:
            with tc.tile_pool(name="sb", bufs=1) as sb:
                t = sb.tile([128, 128], F32)
                nc_t.sync.dma_start(out=t[:], in_=a[:, :])
                nc_t.sync.dma_start(out=o[:, :], in_=t[:])
        nc_t.finalize()
        run_bass_kernel_spmd(
            nc_t, [{"a": np.zeros((128, 128), np.float32)}] * 8,
            core_ids=list(range(8)),
        )
    except Exception:
        pass


def kernel(**inputs):
    from concourse.bass_utils import run_bass_kernel_spmd

    cfg = Cfg()
    layout = build_layout(inputs["edge_index"], cfg)
    in_maps = build_inputs(
        cfg, layout,
        inputs["x"], inputs["W1"], inputs["a_src1"], inputs["a_dst1"],
        inputs["b1"], inputs["W2"], inputs["a_src2"], inputs["a_dst2"],
        inputs["b2"], inputs["fc_w"], inputs["fc_b"],
    )
    nc_b = build_program(cfg, shared_out=True)
    last_err = None
    for attempt in range(3):
        try:
            res = run_bass_kernel_spmd(
                nc_b, in_maps, core_ids=list(range(cfg.nc))
            )
            return assemble_output(cfg, layout, res.results)
        except Exception as e:  # wedged device from a prior crashed session
            last_err = e
            _absorb_device_wedge()
    raise last_err


if __name__ == "__main__":
    pass
